# revision 1
# baseline (speedup 1.0000x reference)
import numpy as np
import ml_dtypes

import concourse.bass as bass
from bass_rust import InstructionNameOrderedSet
import concourse.mybir as mybir
from concourse import tile
from concourse.bass_utils import run_bass_kernel_spmd

BF16 = mybir.dt.bfloat16
F32 = mybir.dt.float32
AF = mybir.ActivationFunctionType

B, S, DIM, H, D = 2, 2048, 1024, 16, 64
WIN = 512
HPC = 4          # heads per core
NCORES = 8
NSB = S // 128   # 16 seq blocks
NKC = DIM // 128  # 8 contraction chunks
STRIPW = 640     # 128 keys attend to <=640 queries (dist 0..512 + 127)
PCOLS = 2 * HPC * D + 2 * HPC + HPC * D + HPC  # q(2x:a,b? no) placeholder

_nc_cache = {}


def _patched_drain(self, tick_clock, wait_clock):
    # Tail drain: walrus limits sync waits per instruction, so convert the
    # multi-wait drain into a chain of single-wait sem waits on SyncE.
    from concourse.vector_clock import ScopedClock

    nc = self.nc
    probe = mybir.InstNoOp(name="__drain_probe", engine=mybir.EngineType.SP, ins=[], outs=[])
    wait_clock.add_sem_waits(probe, ScopedClock({None: tick_clock.global_clock}))
    id2h = {h.num: h for h in self.sems.allocated().values()}
    si = getattr(probe, "sync_info", None)
    if si is not None:
        for w in si.on_wait:
            h = id2h.get(w.id)
            if h is not None:
                nc.sync.wait_ge(h, w.wait_value)
    nc.sync.drain()
    nc.all_engine_barrier()
    popped = nc._tile_sem_poison_stack.pop()
    assert popped is self._sem_poison
    nc.clear_and_free_semaphores(list(self.sems.allocated().values()))
    nc.all_engine_barrier()


tile.TileContext._drain_and_barrier = _patched_drain


def build_nc():
    if "nc" in _nc_cache:
        return _nc_cache["nc"]
    nc = bass.Bass()

    # ---- DRAM I/O (per-core shapes; SPMD same program) ----
    tokT_d = nc.dram_tensor("tokT", [DIM, S], BF16, kind="ExternalInput")
    # packed stationary weights per contraction chunk:
    # cols: [wq 256 | wk 256 | wqrot 256 | wkrot 256 | wg 4 | wv 256 | wmix 4]
    WTOT = 1288
    wall_d = nc.dram_tensor("wall", [NKC, 128, WTOT], BF16, kind="ExternalInput")
    wo_d = nc.dram_tensor("wo", [2, 128, DIM], BF16, kind="ExternalInput")
    vr_d = nc.dram_tensor("vr", [HPC, NSB, 128, D], BF16, kind="ExternalInput")
    cos_d = nc.dram_tensor("cosm", [128, S], BF16, kind="ExternalInput")
    sin_d = nc.dram_tensor("sinm", [128, S], BF16, kind="ExternalInput")
    mask_d = nc.dram_tensor("masks", [NSB, 128, STRIPW], BF16, kind="ExternalInput")
    id4_d = nc.dram_tensor("id4", [4, 4], BF16, kind="ExternalInput")
    out_d = nc.dram_tensor("out", [S, DIM], BF16, kind="ExternalOutput")

    with tile.TileContext(nc) as tc:
        with (
            tc.tile_pool(name="big", bufs=1) as big,
            tc.tile_pool(name="stg", bufs=2) as stg,
            tc.tile_pool(name="pp", bufs=2, space=bass.MemorySpace.PSUM) as pp,
        ):
            # ---- resident SBUF slabs ----
            tok = big.tile([128, NKC * S], BF16, tag="tok")          # 32KB/p
            wsl = big.tile([128, NKC * WTOT], BF16, tag="wsl")       # 12KB/p
            wo_sb = big.tile([128, 2 * DIM], BF16, tag="wo")         # 4KB/p
            cosm = big.tile([128, S], BF16, tag="cos")
            sinm = big.tile([128, S], BF16, tag="sin")
            msl = big.tile([128, NSB * STRIPW], BF16, tag="msl")     # 20KB/p
            vrs = [big.tile([128, NSB * D], BF16, tag=f"vr{h}", name=f"vr{h}") for h in range(HPC)]
            vaug = [big.tile([128, NSB * 65], BF16, tag=f"va{h}", name=f"va{h}") for h in range(HPC)]
            mixs = big.tile([128, 64], F32, tag="mix")               # sigmoid(mix)
            mixr = big.tile([128, 64], F32, tag="mixr")               # mix sigmoid [seq128, sb*4+h]
            gate_raw = big.tile([4, S], F32, tag="gateraw")
            gate1 = big.tile([1, HPC * S], BF16, tag="gate1")
            gsig = big.tile([32, S], BF16, tag="gsig")
            ones1 = big.tile([1, 64], BF16, tag="ones1")
            id4 = big.tile([4, 4], BF16, tag="id4")
            qkslab = big.tile([128, 8 * S], BF16, tag="qkslab")
            qraw = [qkslab[:, 0 * S : 1 * S], qkslab[:, 1 * S : 2 * S]]
            kraw = [qkslab[:, 2 * S : 3 * S], qkslab[:, 3 * S : 4 * S]]
            qrot = [qkslab[:, 4 * S : 5 * S], qkslab[:, 5 * S : 6 * S]]
            krot = [qkslab[:, 6 * S : 7 * S], qkslab[:, 7 * S : 8 * S]]
            qro, kro = qraw, kraw  # roped in place
            # PT ring: 5 live strips per head
            pts = [big.tile([128, 5 * STRIPW], BF16, tag=f"pt{h}", name=f"pt{h}") for h in range(HPC)]
            outg = [big.tile([128, S], BF16, tag=f"og{p}", name=f"og{p}") for p in range(2)]
            vtmp = big.tile([128, D], F32, tag="vtmp")
            dmy = big.tile([1, 128], F32, tag="dmy")
            dmyc = [0]

            pend = []

            def guard(inst):
                if pend:
                    s = InstructionNameOrderedSet()
                    for n in pend:
                        s.add(n)
                    inst.ins.add_nosync_dependencies_from(s)
                    pend.clear()
                return inst

            def absorb(*aps):
                for ap in aps:
                    i = dmyc[0] % 128
                    dmyc[0] += 1
                    ii = nc.vector.tensor_copy(dmy[0:1, i : i + 1], ap[0:1, 0:1])
                    pend.append(ii.ins.name)

            dmyA = big.tile([1, 128], F32, tag="dmyA")
            dmyAc = [0]

            def absorb_act(ap):
                i = dmyAc[0] % 128
                dmyAc[0] += 1
                ii = nc.scalar.copy(dmyA[0:1, i : i + 1], ap[0:1, 0:1])
                pend.append(ii.ins.name)

            bcb = big.tile([32, 1024], BF16, tag="bcb")
            bcbc = [0]
            crumb_st = {"last": None}

            def crumb(src_ap):
                crumb_st["last"] = src_ap[0:1, 0:1]

            def pe_absorb(ap=None):
                ap = ap if ap is not None else crumb_st["last"]
                if ap is None:
                    return
                if ap.partition_size() >= 32 and ap.dtype == BF16:
                    ii = nc.tensor.ldweights(ap[0:32, 0:1])
                else:
                    i = bcbc[0] % 1024
                    bcbc[0] += 1
                    nc.vector.tensor_copy(bcb[0:1, i : i + 1], ap[0:1, 0:1])
                    ii = nc.tensor.ldweights(bcb[0:32, i : i + 1])
                pend.append(ii.ins.name)

            # ---- loads ----
            tokT_dv = tokT_d.rearrange("(k p) s -> k p s", p=128)
            for kc in range(NKC):
                nc.gpsimd.dma_start(out=tok[:, kc * S : kc * S + S], in_=tokT_dv[kc])
                nc.gpsimd.dma_start(
                    out=wsl[:, kc * WTOT : kc * WTOT + WTOT], in_=wall_d[kc]
                )
            for kc in range(2):
                nc.gpsimd.dma_start(
                    out=wo_sb[:, kc * DIM : kc * DIM + DIM], in_=wo_d[kc]
                )
            nc.gpsimd.dma_start(out=cosm[:], in_=cos_d[:])
            nc.gpsimd.dma_start(out=sinm[:], in_=sin_d[:])
            for kb in range(NSB):
                nc.gpsimd.dma_start(
                    out=msl[:, kb * STRIPW : kb * STRIPW + STRIPW], in_=mask_d[kb]
                )
            for h in range(HPC):
                for sb in range(NSB):
                    nc.gpsimd.dma_start(
                        out=vrs[h][:, sb * D : sb * D + D], in_=vr_d[h, sb]
                    )

            nc.vector.memset(ones1[:], 1.0)
            nc.gpsimd.dma_start(out=id4[:], in_=id4_d[:])
            absorb(cosm, sinm)
            for kb in range(NSB):
                absorb(msl[:, kb * STRIPW : kb * STRIPW + 1])

            def wchunk(kc, c0, c1):
                return wsl[:, kc * WTOT + c0 : kc * WTOT + c1]

            def tchunk(kc, s0, s1):
                return tok[:, kc * S + s0 : kc * S + s1]

            # ---- phase 1: T-orient projections: q, k (dual use), gate ----
            NS = 4  # seq chunks of 512
            for dest, c0 in (
                (qraw[0], 0), (qraw[1], 128), (kraw[0], 256), (kraw[1], 384),
                (qrot[0], 512), (qrot[1], 640), (krot[0], 768), (krot[1], 896),
            ):
                for ns in range(NS):
                    ps = pp.tile([128, 512], F32, tag="ps1", name="psA")
                    pe_absorb()
                    for kc in range(NKC):
                        guard(nc.tensor.matmul(
                            ps[:],
                            wchunk(kc, c0, c0 + 128),
                            tchunk(kc, ns * 512, ns * 512 + 512),
                            start=(kc == 0),
                            stop=(kc == NKC - 1),
                        ))
                    nc.vector.tensor_copy(dest[:, ns * 512 : ns * 512 + 512], ps[:])
                    crumb(dest[:, ns * 512 : ns * 512 + 512])
            # gate: M=4
            for ns in range(NS):
                ps = pp.tile([4, 512], F32, tag="ps1", name="psG")
                pe_absorb()
                for kc in range(NKC):
                    guard(nc.tensor.matmul(
                        ps[:],
                        wchunk(kc, 1024, 1028),
                        tchunk(kc, ns * 512, ns * 512 + 512),
                        start=(kc == 0),
                        stop=(kc == NKC - 1),
                    ))
                nc.vector.tensor_copy(gate_raw[:, ns * 512 : ns * 512 + 512], ps[:])
                crumb(gate_raw[:, ns * 512 : ns * 512 + 512])
            absorb_act(gate_raw[0:1, 0:1])
            guard(nc.scalar.activation(gsig[0:4, :], gate_raw[:], AF.Sigmoid))
            for h in range(HPC):
                for ns in range(4):
                    gps = pp.tile([1, 512], F32, tag="ps1", name="gps")
                    pe_absorb(gsig)
                    guard(nc.tensor.matmul(
                        gps[:], id4[:, h : h + 1],
                        gsig[0:4, ns * 512 : ns * 512 + 512],
                        start=True, stop=True,
                    ))
                    absorb(gps[0:1, 0:1])
                    guard(nc.vector.tensor_copy(
                        gate1[0:1, h * S + ns * 512 : h * S + ns * 512 + 512], gps[:]
                    ))

            # ---- phase 2: v + mix (natural orient) ----
            for sb in range(NSB):
                ps = pp.tile([128, 260], F32, tag="ps1", name="psV")
                pe_absorb()
                for kc in range(NKC):
                    guard(nc.tensor.matmul(
                        ps[:],
                        tchunk(kc, sb * 128, sb * 128 + 128),
                        wchunk(kc, 1028, 1288),
                        start=(kc == 0),
                        stop=(kc == NKC - 1),
                    ))
                nc.vector.tensor_copy(mixr[:, sb * 4 : sb * 4 + 4], ps[:, 256:260])
                absorb_act(mixr[0:1, sb * 4 : sb * 4 + 1])
                guard(nc.scalar.activation(
                    mixs[:, sb * 4 : sb * 4 + 4], mixr[:, sb * 4 : sb * 4 + 4], AF.Sigmoid
                ))
                for h in range(HPC):
                    absorb(vrs[h][:, sb * D : sb * D + D])
                    guard(nc.vector.tensor_sub(
                        vtmp[:],
                        vrs[h][:, sb * D : sb * D + D],
                        ps[:, h * D : h * D + D],
                    ))
                    absorb(mixs[:, sb * 4 + h : sb * 4 + h + 1])
                    # v' = mix*(vr - v) + v
                    guard(nc.vector.scalar_tensor_tensor(
                        vaug[h][:, sb * 65 : sb * 65 + 64],
                        vtmp[:],
                        mixs[:, sb * 4 + h : sb * 4 + h + 1],
                        ps[:, h * D : h * D + D],
                        mybir.AluOpType.mult,
                        mybir.AluOpType.add,
                    ))
                    nc.vector.memset(vaug[h][:, sb * 65 + 64 : sb * 65 + 65], 1.0)
                crumb(vaug[HPC - 1][:, sb * 65 : sb * 65 + 64])

            # ---- phase 1b: RoPE via half-swap + cos/sin maps ----
            for raw, rot in (
                (qraw[0], qrot[0]),
                (qraw[1], qrot[1]),
                (kraw[0], krot[0]),
                (kraw[1], krot[1]),
            ):
                nc.vector.tensor_mul(rot[:], rot[:], sinm[:])
                nc.vector.tensor_mul(raw[:], raw[:], cosm[:])
                nc.vector.tensor_add(raw[:], raw[:], rot[:])
                crumb(raw[:])

            # ---- phase 3: attention ----
            ptw_hist, ring_hist, fbs_hist, og_hist = [], [], [], []
            for kb in range(NSB):
                Wn = min(STRIPW, S - kb * 128)
                for h in range(HPC):
                    p, hh = divmod(h, 2)
                    b0 = hh * 64
                    ptv = pts[h][:, (kb % 5) * STRIPW : (kb % 5) * STRIPW + STRIPW]
                    sim = pp.tile([128, STRIPW], F32, tag="psS", name="psS")
                    pe_absorb(kro[p])
                    pe_absorb(qro[p])
                    if len(ptw_hist) >= 1:
                        pe_absorb(ptw_hist[-1])
                    for c0 in range(0, Wn, 512):
                        c1 = min(c0 + 512, Wn)
                        guard(nc.tensor.matmul(
                            sim[:, c0:c1],
                            kro[p][b0 : b0 + 64, kb * 128 : kb * 128 + 128],
                            qro[p][b0 : b0 + 64, kb * 128 + c0 : kb * 128 + c1],
                            start=True,
                            stop=True,
                        ))
                    ptw = stg.tile([128, STRIPW], BF16, tag="ptw", name="ptw", bufs=2)
                    if ring_hist:
                        absorb_act(ring_hist[-1][0:1, 0:1])
                    absorb_act(sim[0:1, 0:1])
                    guard(nc.scalar.activation(ptw[:, 0:Wn], sim[:, 0:Wn], AF.Exp))
                    ptw_hist.append(ptw)
                    absorb(ptw[0:1, 0:1], ptv[0:1, 0:1])
                    guard(nc.vector.tensor_mul(
                        ptv[:, 0:Wn],
                        ptw[:, 0:Wn],
                        msl[:, kb * STRIPW : kb * STRIPW + Wn],
                    ))
                    ring_hist.append(ptv)
                    # AV for q-block qb = kb
                    av = pp.tile([65, 128], F32, tag="psAV", name="psAV", bufs=1)
                    pe_absorb(ptv)
                    if og_hist:
                        pe_absorb(og_hist[-1])
                    if fbs_hist:
                        pe_absorb(fbs_hist[-1][0:1, 0:1])
                    srcs = list(range(max(0, kb - 4), kb + 1))
                    for j, sc in enumerate(srcs):
                        off = (kb - sc) * 128
                        psrc = pts[h][:, (sc % 5) * STRIPW + off : (sc % 5) * STRIPW + off + 128]
                        guard(nc.tensor.matmul(
                            av[:],
                            vaug[h][:, sc * 65 : sc * 65 + 65],
                            psrc,
                            start=(j == 0),
                            stop=(j == len(srcs) - 1),
                        ))
                    # normalize + gate
                    rec_sb = big.tile([1, 128], F32, tag="recsb", name="recsb")
                    f_row = big.tile([1, 128], BF16, tag="frow", name="frow")
                    gsl = gate1[0:1, h * S + kb * 128 : h * S + kb * 128 + 128]
                    nc.vector.reciprocal(rec_sb[:], av[64:65, :])
                    absorb(gsl)
                    guard(nc.vector.tensor_mul(f_row[:], rec_sb[:], gsl))
                    pe_absorb(f_row[0:1, 0:1])
                    if fbs_hist:
                        pe_absorb(fbs_hist[-1][0:1, 0:1])
                    fps = pp.tile([64, 128], F32, tag="fps", name="fps", bufs=1)
                    guard(nc.tensor.matmul(fps[:], ones1[:], f_row[:], start=True, stop=True))
                    fbs = stg.tile([64, 128], F32, tag="fbs", name="fbs", bufs=1)
                    nc.vector.tensor_copy(fbs[:], fps[:])
                    fbs_hist.append(fbs)
                    guard(nc.vector.tensor_mul(
                        outg[p][b0 : b0 + 64, kb * 128 : kb * 128 + 128],
                        av[0:64, :],
                        fbs[:],
                    ))
                    og_hist.append(outg[p][b0 : b0 + 1, kb * 128 : kb * 128 + 1])

            # ---- phase 4: Wo ----
            ost_hist = []
            crumb(outg[0][:, S - 128 : S])
            crumb(outg[1][:, S - 128 : S])
            for g8 in range(4):
                slab = qkslab[:, g8 * 4 * DIM : (g8 + 1) * 4 * DIM]
                for j in range(4):
                    sb = g8 * 4 + j
                    for half in range(2):
                        ps = pp.tile([128, 512], F32, tag="ps1", name="psO")
                        pe_absorb()
                        if ost_hist:
                            pe_absorb(ost_hist[-1])
                        for kc in range(2):
                            guard(nc.tensor.matmul(
                                ps[:],
                                outg[kc][:, sb * 128 : sb * 128 + 128],
                                wo_sb[:, kc * DIM + half * 512 : kc * DIM + half * 512 + 512],
                                start=(kc == 0),
                                stop=(kc == 1),
                            ))
                        dst = slab[:, j * DIM + half * 512 : j * DIM + half * 512 + 512]
                        absorb(ps[0:1, 0:1])
                        guard(nc.vector.tensor_copy(dst, ps[:]))
                        ost_hist.append(dst)
                nc.sync.dma_start(
                    out=out_d[g8 * 512 : g8 * 512 + 512, :].rearrange(
                        "(sb p) d -> p sb d", p=128
                    ),
                    in_=slab.rearrange("p (sb d) -> p sb d", d=DIM),
                )

    _nc_cache["nc"] = nc
    return nc


def vrs_cat(nc, vrs, sb):
    return None


WTOT = 1288


def _prep_core(tokens, value_residual, episode_ids, Wq, Wkv, Wo, Wg, Wmix, b, g):
    bf = ml_dtypes.bfloat16
    hs = slice(4 * g, 4 * g + 4)
    perm = np.concatenate([np.arange(0, D, 2), np.arange(1, D, 2)])
    scale = D ** -0.5

    tokT = np.ascontiguousarray(tokens[b].T).astype(bf)                    # [DIM,S]

    swap = np.concatenate([np.arange(32, 64), np.arange(0, 32)])
    wq4 = Wq.reshape(DIM, H, D)[:, hs][:, :, perm] * scale                 # [DIM,4,64]
    wk4 = Wkv[:, : H * D].reshape(DIM, H, D)[:, hs][:, :, perm]
    wq = wq4.reshape(DIM, 256)
    wk = wk4.reshape(DIM, 256)
    wqr = wq4[:, :, swap].reshape(DIM, 256)
    wkr = wk4[:, :, swap].reshape(DIM, 256)
    wv = Wkv[:, H * D :].reshape(DIM, H, D)[:, hs].reshape(DIM, 256)
    wg = Wg[:, hs]
    wm = Wmix[:, hs]
    wall = np.concatenate([wq, wk, wqr, wkr, wg, wv, wm], axis=1).astype(bf)
    wall = np.ascontiguousarray(wall.reshape(NKC, 128, WTOT))

    wo = Wo.reshape(H, D, DIM)[hs].reshape(256, DIM).astype(bf)
    wo = np.ascontiguousarray(wo.reshape(2, 128, DIM))

    vr = value_residual[b, hs].astype(bf)                                  # [4,S,D]
    vr = np.ascontiguousarray(vr.reshape(HPC, NSB, 128, D))

    pos = np.arange(S, dtype=np.float64)
    invf = 1.0 / (10000.0 ** (np.arange(0, D, 2, dtype=np.float64) / D))   # [32]
    ang = pos[None, :] * invf[:, None]                                     # [32,S]
    c32, s32 = np.cos(ang), np.sin(ang)
    cosm = np.tile(c32, (4, 1)).astype(bf)                                 # [128,S]
    sgn = np.concatenate([-s32, s32], axis=0)                              # [64,S]
    sinm = np.tile(sgn, (2, 1)).astype(bf)

    ids = np.asarray(episode_ids[b])
    # ee[k] = last index with same episode id
    ee = np.searchsorted(ids, ids, side="right") - 1                       # [S]
    kk = np.arange(S)
    ub = np.minimum(kk + WIN, ee)                                          # [S]
    masks = np.zeros((NSB, 128, STRIPW), dtype=bf)
    for kb in range(NSB):
        k = kb * 128 + np.arange(128)
        q = kb * 128 + np.arange(STRIPW)
        live = (q[None, :] >= k[:, None]) & (q[None, :] <= ub[k][:, None]) & (
            q[None, :] < S
        )
        masks[kb] = live.astype(bf)

    return {
        "tokT": tokT, "wall": wall, "wo": wo, "vr": vr,
        "cosm": cosm, "sinm": sinm, "masks": masks, "id4": np.eye(4, dtype=bf),
    }


def kernel(tokens, value_residual, episode_ids, Wq, Wkv, Wo, Wg, Wmix):
    nc = build_nc()
    in_maps = []
    for c in range(NCORES):
        b, g = divmod(c, 4)
        in_maps.append(
            _prep_core(tokens, value_residual, episode_ids, Wq, Wkv, Wo, Wg, Wmix, b, g)
        )
    res = run_bass_kernel_spmd(nc, in_maps, core_ids=list(range(NCORES))).results
    out = np.zeros((B, S, DIM), dtype=np.float32)
    for c in range(NCORES):
        out[c // 4] += res[c]["out"].astype(np.float32)
    return out



# revision 3
# speedup vs baseline: 4.0486x; 4.0486x over previous
import numpy as np
import ml_dtypes

import jax
import concourse.bass as bass
from bass_rust import InstructionNameOrderedSet
import concourse.mybir as mybir
from concourse import tile
from concourse import bass2jax

BF16 = mybir.dt.bfloat16
F32 = mybir.dt.float32
AF = mybir.ActivationFunctionType

B, S, DIM, H, D = 2, 2048, 1024, 16, 64
WIN = 512
HPC = 4          # heads per core
NCORES = 8
NSB = S // 128   # 16 seq blocks
NKC = DIM // 128  # 8 contraction chunks
STRIPW = 640     # 128 keys attend to <=640 queries (dist 0..512 + 127)
WUP = 776        # uploaded weight cols per kc chunk: wq 256 | wk 256 | wg 4 | wv 256 | wm 4
WTOT = 1288      # sbuf slab cols per kc chunk: wq | wk | wqrot | wkrot | wg | wv | wm

_nc_cache = {}


def _patched_drain(self, tick_clock, wait_clock):
    # Tail drain: walrus limits sync waits per instruction, so convert the
    # multi-wait drain into a chain of single-wait sem waits on SyncE.
    from concourse.vector_clock import ScopedClock

    nc = self.nc
    probe = mybir.InstNoOp(name="__drain_probe", engine=mybir.EngineType.SP, ins=[], outs=[])
    wait_clock.add_sem_waits(probe, ScopedClock({None: tick_clock.global_clock}))
    id2h = {h.num: h for h in self.sems.allocated().values()}
    si = getattr(probe, "sync_info", None)
    if si is not None:
        for w in si.on_wait:
            h = id2h.get(w.id)
            if h is not None:
                nc.sync.wait_ge(h, w.wait_value)
    nc.sync.drain()
    nc.all_engine_barrier()
    popped = nc._tile_sem_poison_stack.pop()
    assert popped is self._sem_poison
    nc.clear_and_free_semaphores(list(self.sems.allocated().values()))
    nc.all_engine_barrier()


tile.TileContext._drain_and_barrier = _patched_drain


def _consts():
    bf = ml_dtypes.bfloat16
    pos = np.arange(S, dtype=np.float64)
    invf = 1.0 / (10000.0 ** (np.arange(0, D, 2, dtype=np.float64) / D))   # [32]
    ang = pos[None, :] * invf[:, None]                                     # [32,S]
    c32, s32 = np.cos(ang), np.sin(ang)
    cosm = np.tile(c32, (4, 1)).astype(bf)                                 # [128,S]
    sgn = np.concatenate([-s32, s32], axis=0)                              # [64,S]
    sinm = np.tile(sgn, (2, 1)).astype(bf)
    p = np.arange(128)
    j = np.arange(STRIPW)
    win = ((j[None, :] >= p[:, None]) & (j[None, :] - p[:, None] <= WIN)).astype(np.float32)
    iotaw = np.ascontiguousarray(np.broadcast_to(j[None, :].astype(np.float32), (128, STRIPW)))
    return cosm, sinm, win, iotaw


def build_nc():
    if "nc" in _nc_cache:
        return _nc_cache["nc"]
    nc = bass.Bass(num_devices=NCORES)

    # ---- per-core external I/O (core c = 4*b + g: batch b, head-group g) ----
    toksh_d = nc.dram_tensor("toksh", [DIM, 512], BF16, kind="ExternalInput")
    wallsh_d = nc.dram_tensor("wallsh", [4, 128, WUP], BF16, kind="ExternalInput")
    wosh_d = nc.dram_tensor("wosh", [128, DIM], BF16, kind="ExternalInput")
    vr_d = nc.dram_tensor("vr", [HPC, NSB, 128, D], BF16, kind="ExternalInput")
    ubt_d = nc.dram_tensor("ubt", [128, NSB], F32, kind="ExternalInput")
    out_d = nc.dram_tensor("out", [512, DIM], BF16, kind="ExternalOutput")

    # ---- NEFF-embedded constants (shipped at model load, not per call) ----
    cosm_np, sinm_np, win_np, iotaw_np = _consts()
    cos_d = nc.inline_tensor(cosm_np, name="cosk")
    sin_d = nc.inline_tensor(sinm_np, name="sink")
    win_d = nc.inline_tensor(win_np, name="wink")
    iota_d = nc.inline_tensor(iotaw_np, name="iotak")
    id4_d = nc.inline_tensor(np.eye(4, dtype=ml_dtypes.bfloat16), name="id4k")

    # ---- internal DRAM for collectives ----
    tok_stage = nc.dram_tensor("tok_stage", [DIM, 512], BF16, kind="Internal")
    wall_stage = nc.dram_tensor("wall_stage", [4, 128, WUP], BF16, kind="Internal")
    wo_stage = nc.dram_tensor("wo_stage", [128, DIM], BF16, kind="Internal")
    tok_g = nc.dram_tensor("tok_g", [4, DIM, 512], BF16, kind="Internal")
    wall_g = nc.dram_tensor("wall_g", [NKC, 128, WUP], BF16, kind="Internal")
    wo_g = nc.dram_tensor("wo_g", [2, 128, DIM], BF16, kind="Internal")
    pout_d = nc.dram_tensor("pout", [S, DIM], BF16, kind="Internal")
    rs_d = nc.dram_tensor("rsout", [512, DIM], BF16, kind="Internal")

    s_stage = nc.alloc_semaphore("s_stage")
    s_ag = nc.alloc_semaphore("s_ag")
    s_rs = nc.alloc_semaphore("s_rs")
    s_fin = nc.alloc_semaphore("s_fin")

    G4 = [[0, 1, 2, 3], [4, 5, 6, 7]]        # batch groups (head-parallel)
    G2 = [[0, 4], [1, 5], [2, 6], [3, 7]]    # same head-group pairs across batches

    # ---- pre-tile: stage inputs to internal DRAM, gather across cores ----
    nc.gpsimd.dma_start(out=tok_stage[:], in_=toksh_d[:]).then_inc(s_stage, 16)
    nc.gpsimd.dma_start(out=wall_stage[:], in_=wallsh_d[:]).then_inc(s_stage, 16)
    nc.gpsimd.dma_start(out=wo_stage[:], in_=wosh_d[:]).then_inc(s_stage, 16)
    nc.gpsimd.wait_ge(s_stage, 48)
    nc.gpsimd.collective_compute(
        "AllGather", mybir.AluOpType.bypass, replica_groups=G4,
        ins=[tok_stage[:].opt()], outs=[tok_g[:].opt()],
    ).then_inc(s_ag, 1)
    nc.gpsimd.collective_compute(
        "AllGather", mybir.AluOpType.bypass, replica_groups=G2,
        ins=[wall_stage[:].opt()], outs=[wall_g[:].opt()],
    ).then_inc(s_ag, 1)
    nc.gpsimd.collective_compute(
        "AllGather", mybir.AluOpType.bypass, replica_groups=G2,
        ins=[wo_stage[:].opt()], outs=[wo_g[:].opt()],
    ).then_inc(s_ag, 1)
    nc.gpsimd.wait_ge(s_ag, 3)

    with tile.TileContext(nc) as tc:
        with (
            tc.tile_pool(name="big", bufs=1) as big,
            tc.tile_pool(name="stg", bufs=2) as stg,
            tc.tile_pool(name="pp", bufs=2, space=bass.MemorySpace.PSUM) as pp,
        ):
            # ---- resident SBUF slabs ----
            tok = big.tile([128, NKC * S], BF16, tag="tok")          # 32KB/p
            wsl = big.tile([128, NKC * WTOT], BF16, tag="wsl")       # 20KB/p
            wo_sb = big.tile([128, 2 * DIM], BF16, tag="wo")         # 4KB/p
            cosm = big.tile([128, S], BF16, tag="cos")
            sinm = big.tile([128, S], BF16, tag="sin")
            msl = big.tile([128, NSB * STRIPW], BF16, tag="msl")     # 20KB/p
            winsb = big.tile([128, STRIPW], F32, tag="winsb")
            iotasb = big.tile([128, STRIPW], F32, tag="iotasb")
            ubl = big.tile([128, NSB], F32, tag="ubl")
            vrs = [big.tile([128, NSB * D], BF16, tag=f"vr{h}", name=f"vr{h}") for h in range(HPC)]
            vaug = [big.tile([128, NSB * 65], BF16, tag=f"va{h}", name=f"va{h}") for h in range(HPC)]
            mixs = big.tile([128, 64], F32, tag="mix")               # sigmoid(mix)
            mixr = big.tile([128, 64], F32, tag="mixr")              # mix raw [seq128, sb*4+h]
            gate_raw = big.tile([4, S], F32, tag="gateraw")
            gate1 = big.tile([1, HPC * S], BF16, tag="gate1")
            gsig = big.tile([32, S], BF16, tag="gsig")
            ones1 = big.tile([1, 64], BF16, tag="ones1")
            id4 = big.tile([4, 4], BF16, tag="id4")
            qkslab = big.tile([128, 8 * S], BF16, tag="qkslab")
            qraw = [qkslab[:, 0 * S : 1 * S], qkslab[:, 1 * S : 2 * S]]
            kraw = [qkslab[:, 2 * S : 3 * S], qkslab[:, 3 * S : 4 * S]]
            qrot = [qkslab[:, 4 * S : 5 * S], qkslab[:, 5 * S : 6 * S]]
            krot = [qkslab[:, 6 * S : 7 * S], qkslab[:, 7 * S : 8 * S]]
            qro, kro = qraw, kraw  # roped in place
            # PT ring: 5 live strips per head
            pts = [big.tile([128, 5 * STRIPW], BF16, tag=f"pt{h}", name=f"pt{h}") for h in range(HPC)]
            outg = [big.tile([128, S], BF16, tag=f"og{p}", name=f"og{p}") for p in range(2)]
            vtmp = big.tile([128, D], F32, tag="vtmp")
            dmy = big.tile([1, 128], F32, tag="dmy")
            dmyc = [0]

            pend = []

            def guard(inst):
                if pend:
                    s = InstructionNameOrderedSet()
                    for n in pend:
                        s.add(n)
                    inst.ins.add_nosync_dependencies_from(s)
                    pend.clear()
                return inst

            def absorb(*aps):
                for ap in aps:
                    i = dmyc[0] % 128
                    dmyc[0] += 1
                    ii = nc.vector.tensor_copy(dmy[0:1, i : i + 1], ap[0:1, 0:1])
                    pend.append(ii.ins.name)

            dmyA = big.tile([1, 128], F32, tag="dmyA")
            dmyAc = [0]

            def absorb_act(ap):
                i = dmyAc[0] % 128
                dmyAc[0] += 1
                ii = nc.scalar.copy(dmyA[0:1, i : i + 1], ap[0:1, 0:1])
                pend.append(ii.ins.name)

            bcb = big.tile([32, 1024], BF16, tag="bcb")
            bcbc = [0]
            crumb_st = {"last": None}

            def crumb(src_ap):
                crumb_st["last"] = src_ap[0:1, 0:1]

            def pe_absorb(ap=None):
                ap = ap if ap is not None else crumb_st["last"]
                if ap is None:
                    return
                if ap.partition_size() >= 32 and ap.dtype == BF16:
                    ii = nc.tensor.ldweights(ap[0:32, 0:1])
                else:
                    i = bcbc[0] % 1024
                    bcbc[0] += 1
                    nc.vector.tensor_copy(bcb[0:1, i : i + 1], ap[0:1, 0:1])
                    ii = nc.tensor.ldweights(bcb[0:32, i : i + 1])
                pend.append(ii.ins.name)

            # ---- loads (from gathered internal DRAM + inline consts) ----
            for kc in range(NKC):
                for q in range(4):
                    nc.gpsimd.dma_start(
                        out=tok[:, kc * S + q * 512 : kc * S + q * 512 + 512],
                        in_=tok_g[q, kc * 128 : kc * 128 + 128, :],
                    )
                # wq|wk -> slab cols 0:512; wg|wv|wm -> slab cols 1024:1288
                nc.gpsimd.dma_start(
                    out=wsl[:, kc * WTOT : kc * WTOT + 512], in_=wall_g[kc][:, 0:512]
                )
                nc.gpsimd.dma_start(
                    out=wsl[:, kc * WTOT + 1024 : kc * WTOT + 1288],
                    in_=wall_g[kc][:, 512:776],
                )
            for kc in range(2):
                nc.gpsimd.dma_start(
                    out=wo_sb[:, kc * DIM : kc * DIM + DIM], in_=wo_g[kc]
                )
            nc.gpsimd.dma_start(out=cosm[:], in_=cos_d[:])
            nc.gpsimd.dma_start(out=sinm[:], in_=sin_d[:])
            nc.gpsimd.dma_start(out=winsb[:], in_=win_d[:])
            nc.gpsimd.dma_start(out=iotasb[:], in_=iota_d[:])
            nc.gpsimd.dma_start(out=ubl[:], in_=ubt_d[:])
            for h in range(HPC):
                for sb in range(NSB):
                    nc.gpsimd.dma_start(
                        out=vrs[h][:, sb * D : sb * D + D], in_=vr_d[h, sb]
                    )

            nc.vector.memset(ones1[:], 1.0)
            nc.gpsimd.dma_start(out=id4[:], in_=id4_d[:])
            absorb(cosm, sinm)

            # ---- masks on device: msl = win * (iota <= ub_local) ----
            absorb(winsb, iotasb, ubl)
            for kb in range(NSB):
                guard(nc.vector.scalar_tensor_tensor(
                    msl[:, kb * STRIPW : kb * STRIPW + STRIPW],
                    iotasb[:],
                    ubl[:, kb : kb + 1],
                    winsb[:],
                    mybir.AluOpType.is_le,
                    mybir.AluOpType.mult,
                ))

            # ---- rot weights on device: wqrot/wkrot = 32-col half-swap of wq/wk ----
            for kc in range(NKC):
                base = kc * WTOT
                for h in range(HPC):
                    for dst0, src0 in ((0, 32), (32, 0)):
                        nc.vector.tensor_copy(
                            wsl[:, base + 512 + h * 64 + dst0 : base + 512 + h * 64 + dst0 + 32],
                            wsl[:, base + h * 64 + src0 : base + h * 64 + src0 + 32],
                        )
                        nc.vector.tensor_copy(
                            wsl[:, base + 768 + h * 64 + dst0 : base + 768 + h * 64 + dst0 + 32],
                            wsl[:, base + 256 + h * 64 + src0 : base + 256 + h * 64 + src0 + 32],
                        )

            def wchunk(kc, c0, c1):
                return wsl[:, kc * WTOT + c0 : kc * WTOT + c1]

            def tchunk(kc, s0, s1):
                return tok[:, kc * S + s0 : kc * S + s1]

            # ---- phase 1: T-orient projections: q, k (dual use), gate ----
            NS = 4  # seq chunks of 512
            for dest, c0 in (
                (qraw[0], 0), (qraw[1], 128), (kraw[0], 256), (kraw[1], 384),
                (qrot[0], 512), (qrot[1], 640), (krot[0], 768), (krot[1], 896),
            ):
                for ns in range(NS):
                    ps = pp.tile([128, 512], F32, tag="ps1", name="psA")
                    pe_absorb()
                    for kc in range(NKC):
                        guard(nc.tensor.matmul(
                            ps[:],
                            wchunk(kc, c0, c0 + 128),
                            tchunk(kc, ns * 512, ns * 512 + 512),
                            start=(kc == 0),
                            stop=(kc == NKC - 1),
                        ))
                    nc.vector.tensor_copy(dest[:, ns * 512 : ns * 512 + 512], ps[:])
                    crumb(dest[:, ns * 512 : ns * 512 + 512])
            # gate: M=4
            for ns in range(NS):
                ps = pp.tile([4, 512], F32, tag="ps1", name="psG")
                pe_absorb()
                for kc in range(NKC):
                    guard(nc.tensor.matmul(
                        ps[:],
                        wchunk(kc, 1024, 1028),
                        tchunk(kc, ns * 512, ns * 512 + 512),
                        start=(kc == 0),
                        stop=(kc == NKC - 1),
                    ))
                nc.vector.tensor_copy(gate_raw[:, ns * 512 : ns * 512 + 512], ps[:])
                crumb(gate_raw[:, ns * 512 : ns * 512 + 512])
            absorb_act(gate_raw[0:1, 0:1])
            guard(nc.scalar.activation(gsig[0:4, :], gate_raw[:], AF.Sigmoid))
            for h in range(HPC):
                for ns in range(4):
                    gps = pp.tile([1, 512], F32, tag="ps1", name="gps")
                    pe_absorb(gsig)
                    guard(nc.tensor.matmul(
                        gps[:], id4[:, h : h + 1],
                        gsig[0:4, ns * 512 : ns * 512 + 512],
                        start=True, stop=True,
                    ))
                    absorb(gps[0:1, 0:1])
                    guard(nc.vector.tensor_copy(
                        gate1[0:1, h * S + ns * 512 : h * S + ns * 512 + 512], gps[:]
                    ))

            # ---- phase 2: v + mix (natural orient) ----
            for sb in range(NSB):
                ps = pp.tile([128, 260], F32, tag="ps1", name="psV")
                pe_absorb()
                for kc in range(NKC):
                    guard(nc.tensor.matmul(
                        ps[:],
                        tchunk(kc, sb * 128, sb * 128 + 128),
                        wchunk(kc, 1028, 1288),
                        start=(kc == 0),
                        stop=(kc == NKC - 1),
                    ))
                nc.vector.tensor_copy(mixr[:, sb * 4 : sb * 4 + 4], ps[:, 256:260])
                absorb_act(mixr[0:1, sb * 4 : sb * 4 + 1])
                guard(nc.scalar.activation(
                    mixs[:, sb * 4 : sb * 4 + 4], mixr[:, sb * 4 : sb * 4 + 4], AF.Sigmoid
                ))
                for h in range(HPC):
                    absorb(vrs[h][:, sb * D : sb * D + D])
                    guard(nc.vector.tensor_sub(
                        vtmp[:],
                        vrs[h][:, sb * D : sb * D + D],
                        ps[:, h * D : h * D + D],
                    ))
                    absorb(mixs[:, sb * 4 + h : sb * 4 + h + 1])
                    # v' = mix*(vr - v) + v
                    guard(nc.vector.scalar_tensor_tensor(
                        vaug[h][:, sb * 65 : sb * 65 + 64],
                        vtmp[:],
                        mixs[:, sb * 4 + h : sb * 4 + h + 1],
                        ps[:, h * D : h * D + D],
                        mybir.AluOpType.mult,
                        mybir.AluOpType.add,
                    ))
                    nc.vector.memset(vaug[h][:, sb * 65 + 64 : sb * 65 + 65], 1.0)
                crumb(vaug[HPC - 1][:, sb * 65 : sb * 65 + 64])

            # ---- phase 1b: RoPE via half-swap + cos/sin maps ----
            for raw, rot in (
                (qraw[0], qrot[0]),
                (qraw[1], qrot[1]),
                (kraw[0], krot[0]),
                (kraw[1], krot[1]),
            ):
                nc.vector.tensor_mul(rot[:], rot[:], sinm[:])
                nc.vector.tensor_mul(raw[:], raw[:], cosm[:])
                nc.vector.tensor_add(raw[:], raw[:], rot[:])
                crumb(raw[:])

            # ---- phase 3: attention ----
            ptw_hist, ring_hist, fbs_hist, og_hist = [], [], [], []
            for kb in range(NSB):
                Wn = min(STRIPW, S - kb * 128)
                for h in range(HPC):
                    p, hh = divmod(h, 2)
                    b0 = hh * 64
                    ptv = pts[h][:, (kb % 5) * STRIPW : (kb % 5) * STRIPW + STRIPW]
                    sim = pp.tile([128, STRIPW], F32, tag="psS", name="psS")
                    pe_absorb(kro[p])
                    pe_absorb(qro[p])
                    if len(ptw_hist) >= 1:
                        pe_absorb(ptw_hist[-1])
                    for c0 in range(0, Wn, 512):
                        c1 = min(c0 + 512, Wn)
                        guard(nc.tensor.matmul(
                            sim[:, c0:c1],
                            kro[p][b0 : b0 + 64, kb * 128 : kb * 128 + 128],
                            qro[p][b0 : b0 + 64, kb * 128 + c0 : kb * 128 + c1],
                            start=True,
                            stop=True,
                        ))
                    ptw = stg.tile([128, STRIPW], BF16, tag="ptw", name="ptw", bufs=2)
                    if ring_hist:
                        absorb_act(ring_hist[-1][0:1, 0:1])
                    absorb_act(sim[0:1, 0:1])
                    guard(nc.scalar.activation(ptw[:, 0:Wn], sim[:, 0:Wn], AF.Exp))
                    ptw_hist.append(ptw)
                    absorb(ptw[0:1, 0:1], ptv[0:1, 0:1])
                    guard(nc.vector.tensor_mul(
                        ptv[:, 0:Wn],
                        ptw[:, 0:Wn],
                        msl[:, kb * STRIPW : kb * STRIPW + Wn],
                    ))
                    ring_hist.append(ptv)
                    # AV for q-block qb = kb
                    av = pp.tile([65, 128], F32, tag="psAV", name="psAV", bufs=1)
                    pe_absorb(ptv)
                    if og_hist:
                        pe_absorb(og_hist[-1])
                    if fbs_hist:
                        pe_absorb(fbs_hist[-1][0:1, 0:1])
                    srcs = list(range(max(0, kb - 4), kb + 1))
                    for j, sc in enumerate(srcs):
                        off = (kb - sc) * 128
                        psrc = pts[h][:, (sc % 5) * STRIPW + off : (sc % 5) * STRIPW + off + 128]
                        guard(nc.tensor.matmul(
                            av[:],
                            vaug[h][:, sc * 65 : sc * 65 + 65],
                            psrc,
                            start=(j == 0),
                            stop=(j == len(srcs) - 1),
                        ))
                    # normalize + gate
                    rec_sb = big.tile([1, 128], F32, tag="recsb", name="recsb")
                    f_row = big.tile([1, 128], BF16, tag="frow", name="frow")
                    gsl = gate1[0:1, h * S + kb * 128 : h * S + kb * 128 + 128]
                    nc.vector.reciprocal(rec_sb[:], av[64:65, :])
                    absorb(gsl)
                    guard(nc.vector.tensor_mul(f_row[:], rec_sb[:], gsl))
                    pe_absorb(f_row[0:1, 0:1])
                    if fbs_hist:
                        pe_absorb(fbs_hist[-1][0:1, 0:1])
                    fps = pp.tile([64, 128], F32, tag="fps", name="fps", bufs=1)
                    guard(nc.tensor.matmul(fps[:], ones1[:], f_row[:], start=True, stop=True))
                    fbs = stg.tile([64, 128], F32, tag="fbs", name="fbs", bufs=1)
                    nc.vector.tensor_copy(fbs[:], fps[:])
                    fbs_hist.append(fbs)
                    guard(nc.vector.tensor_mul(
                        outg[p][b0 : b0 + 64, kb * 128 : kb * 128 + 128],
                        av[0:64, :],
                        fbs[:],
                    ))
                    og_hist.append(outg[p][b0 : b0 + 1, kb * 128 : kb * 128 + 1])

            # ---- phase 4: Wo -> partial out (internal DRAM) ----
            ost_hist = []
            crumb(outg[0][:, S - 128 : S])
            crumb(outg[1][:, S - 128 : S])
            for g8 in range(4):
                slab = qkslab[:, g8 * 4 * DIM : (g8 + 1) * 4 * DIM]
                for j in range(4):
                    sb = g8 * 4 + j
                    for half in range(2):
                        ps = pp.tile([128, 512], F32, tag="ps1", name="psO")
                        pe_absorb()
                        if ost_hist:
                            pe_absorb(ost_hist[-1])
                        for kc in range(2):
                            guard(nc.tensor.matmul(
                                ps[:],
                                outg[kc][:, sb * 128 : sb * 128 + 128],
                                wo_sb[:, kc * DIM + half * 512 : kc * DIM + half * 512 + 512],
                                start=(kc == 0),
                                stop=(kc == 1),
                            ))
                        dst = slab[:, j * DIM + half * 512 : j * DIM + half * 512 + 512]
                        absorb(ps[0:1, 0:1])
                        guard(nc.vector.tensor_copy(dst, ps[:]))
                        ost_hist.append(dst)
                nc.sync.dma_start(
                    out=pout_d[g8 * 512 : g8 * 512 + 512, :].rearrange(
                        "(sb p) d -> p sb d", p=128
                    ),
                    in_=slab.rearrange("p (sb d) -> p sb d", d=DIM),
                )

    # ---- post-tile (drain guarantees all DMAs done): reduce partials ----
    nc.gpsimd.collective_compute(
        "ReduceScatter", mybir.AluOpType.add, replica_groups=G4,
        ins=[pout_d[:].opt()], outs=[rs_d[:].opt()],
    ).then_inc(s_rs, 1)
    nc.gpsimd.wait_ge(s_rs, 1)
    nc.gpsimd.dma_start(out=out_d[:], in_=rs_d[:]).then_inc(s_fin, 16)
    nc.sync.wait_ge(s_fin, 16)

    _nc_cache["nc"] = nc
    return nc


def _prep_all(tokens, value_residual, episode_ids, Wq, Wkv, Wo, Wg, Wmix):
    bf = ml_dtypes.bfloat16
    perm = np.concatenate([np.arange(0, D, 2), np.arange(1, D, 2)])
    scale = D ** -0.5

    tokens = np.asarray(tokens)
    tokT = [np.ascontiguousarray(tokens[b].T).astype(bf) for b in range(B)]   # [DIM,S]

    Wqp = np.asarray(Wq).reshape(DIM, H, D)[:, :, perm] * scale
    Wkp = np.asarray(Wkv)[:, : H * D].reshape(DIM, H, D)[:, :, perm]
    Wvp = np.asarray(Wkv)[:, H * D :].reshape(DIM, H, D)
    Wop = np.asarray(Wo).reshape(H, D, DIM)
    Wgp, Wmp = np.asarray(Wg), np.asarray(Wmix)
    packs = []
    for g in range(4):
        hs = slice(4 * g, 4 * g + 4)
        wall = np.concatenate(
            [
                Wqp[:, hs].reshape(DIM, 256),
                Wkp[:, hs].reshape(DIM, 256),
                Wgp[:, hs],
                Wvp[:, hs].reshape(DIM, 256),
                Wmp[:, hs],
            ],
            axis=1,
        ).astype(bf)                                                          # [DIM, 776]
        wall = np.ascontiguousarray(wall.reshape(NKC, 128, WUP))
        wo = np.ascontiguousarray(
            Wop[hs].reshape(256, DIM).astype(bf).reshape(2, 128, DIM)
        )
        packs.append((wall, wo))

    ubts = []
    kk = np.arange(S)
    for b in range(B):
        ids = np.asarray(episode_ids[b])
        ee = np.searchsorted(ids, ids, side="right") - 1                      # [S]
        ubl = (ee.astype(np.float32) - (kk // 128 * 128).astype(np.float32))
        ubts.append(np.ascontiguousarray(ubl.reshape(NSB, 128).T))            # [128,NSB]

    vrs = np.asarray(value_residual).astype(bf)

    in_maps = []
    for c in range(NCORES):
        b, g = divmod(c, 4)
        wall, wo = packs[g]
        in_maps.append({
            "toksh": np.ascontiguousarray(tokT[b][:, g * 512 : (g + 1) * 512]),
            "wallsh": np.ascontiguousarray(wall[4 * b : 4 * b + 4]),
            "wosh": wo[b],
            "vr": np.ascontiguousarray(vrs[b, 4 * g : 4 * g + 4].reshape(HPC, NSB, 128, D)),
            "ubt": ubts[b],
        })
    return in_maps


# ---- cached PJRT dispatch: jit built once, zero output-buffers device-resident ----
_runner_cache = {}


def _get_runner(nc):
    if "fn" in _runner_cache:
        return _runner_cache
    from jax.experimental.shard_map import shard_map

    bass2jax.install_neuronx_cc_hook()
    partition_name = nc.partition_id_tensor.name if nc.partition_id_tensor else None
    in_names, out_names, out_avals = [], [], []
    for alloc in nc.m.functions[0].allocations:
        if not isinstance(alloc, mybir.MemoryLocationSet):
            continue
        name = alloc.memorylocations[0].name
        if alloc.kind == "ExternalInput":
            if name != partition_name:
                in_names.append(name)
        elif alloc.kind == "ExternalOutput":
            out_avals.append(
                jax.core.ShapedArray(tuple(alloc.tensor_shape), mybir.dt.np(alloc.dtype))
            )
            out_names.append(name)
    in_names_all = list(in_names) + list(out_names)
    if partition_name is not None:
        in_names_all.append(partition_name)

    def _body(*args):
        operands = list(args)
        if partition_name is not None:
            operands.append(bass2jax.partition_id_tensor())
        outs = bass2jax._bass_exec_p.bind(
            *operands,
            out_avals=tuple(out_avals),
            in_names=tuple(in_names_all),
            out_names=tuple(out_names),
            lowering_input_output_aliases=(),
            sim_require_finite=True,
            sim_require_nnan=True,
            nc=nc,
        )
        return tuple(outs)

    devices = jax.devices()[:NCORES]
    mesh = jax.sharding.Mesh(np.asarray(devices), ("core",))
    P = jax.sharding.PartitionSpec
    n_in = len(in_names) + len(out_names)
    fn = jax.jit(
        shard_map(
            _body, mesh=mesh, in_specs=(P("core"),) * n_in,
            out_specs=(P("core"),) * len(out_names), check_rep=False,
        ),
        keep_unused=True,
    )
    sh = jax.sharding.NamedSharding(mesh, P("core"))
    zeros_dev = [
        jax.device_put(np.zeros((NCORES * a.shape[0], *a.shape[1:]), a.dtype), sh)
        for a in out_avals
    ]
    jax.block_until_ready(zeros_dev)
    _runner_cache.update(fn=fn, in_names=in_names, zeros_dev=zeros_dev)
    return _runner_cache


def _execute(nc, in_maps):
    r = _get_runner(nc)
    concat_in = [
        np.concatenate([np.asarray(m[name]) for m in in_maps], axis=0)
        for name in r["in_names"]
    ]
    outs = r["fn"](*concat_in, *r["zeros_dev"])
    out_full = np.asarray(outs[0]).reshape(NCORES, 512, DIM)
    res = np.zeros((B, S, DIM), dtype=np.float32)
    for c in range(NCORES):
        b, rk = divmod(c, 4)
        res[b, rk * 512 : (rk + 1) * 512] = out_full[c].astype(np.float32)
    return res


def kernel(tokens, value_residual, episode_ids, Wq, Wkv, Wo, Wg, Wmix):
    nc = build_nc()
    in_maps = _prep_all(tokens, value_residual, episode_ids, Wq, Wkv, Wo, Wg, Wmix)
    return _execute(nc, in_maps)


# revision 9
# speedup vs baseline: 4.4437x; 1.0976x over previous
import numpy as np
import ml_dtypes

import jax
import concourse.bass as bass
from bass_rust import InstructionNameOrderedSet
import concourse.mybir as mybir
from concourse import tile
from concourse import bass2jax

BF16 = mybir.dt.bfloat16
F32 = mybir.dt.float32
FP8 = mybir.dt.float8e4
AF = mybir.ActivationFunctionType

B, S, DIM, H, D = 2, 2048, 1024, 16, 64
WIN = 512
HPC = 4          # heads per core
NCORES = 8
NSB = S // 128   # 16 seq blocks
NKC = DIM // 128  # 8 contraction chunks
STRIPW = 640     # 128 keys attend to <=640 queries (dist 0..512 + 127)
WUP = 768        # uploaded weight cols per kc chunk: wq 256 | wk 256 | wv 256 (fp8)
WSLW = 1280      # sbuf slab cols per kc chunk: wq | wk | wqrot | wkrot | wv
QKSC = 8.0       # wq/wk upload prescale (avoids fp8 subnormals)
EXPSC = 1.0 / (QKSC * QKSC * 8.0)   # folds both prescales + D**-0.5 into exp

# bf16 blob layout (elements): tok | wall | wo | vr | ubq | ubr | gate1 | mraw
TOK_N = DIM * 512                 # 524288
WALL_N = 4 * 128 * WUP            # 393216
WO_N = 128 * DIM                  # 131072
VR_N = HPC * S * D                # 524288
UB_N = 128 * NSB                  # 2048
GR_N = 4 * S                      # 8192
MR_N = 128 * 64                   # 8192
BLOB16_N = TOK_N + WALL_N + WO_N + VR_N + 2 * UB_N + GR_N + MR_N

_nc_cache = {}


def _patched_drain(self, tick_clock, wait_clock):
    # Tail drain: walrus limits sync waits per instruction, so convert the
    # multi-wait drain into a chain of single-wait sem waits on SyncE.
    from concourse.vector_clock import ScopedClock

    nc = self.nc
    probe = mybir.InstNoOp(name="__drain_probe", engine=mybir.EngineType.SP, ins=[], outs=[])
    wait_clock.add_sem_waits(probe, ScopedClock({None: tick_clock.global_clock}))
    id2h = {h.num: h for h in self.sems.allocated().values()}
    si = getattr(probe, "sync_info", None)
    if si is not None:
        for w in si.on_wait:
            h = id2h.get(w.id)
            if h is not None:
                nc.sync.wait_ge(h, w.wait_value)
    nc.sync.drain()
    nc.all_engine_barrier()
    popped = nc._tile_sem_poison_stack.pop()
    assert popped is self._sem_poison
    nc.clear_and_free_semaphores(list(self.sems.allocated().values()))
    nc.all_engine_barrier()


tile.TileContext._drain_and_barrier = _patched_drain


def _consts():
    bf = ml_dtypes.bfloat16
    pos = np.arange(S, dtype=np.float64)
    invf = 1.0 / (10000.0 ** (np.arange(0, D, 2, dtype=np.float64) / D))   # [32]
    ang = pos[None, :] * invf[:, None]                                     # [32,S]
    c32, s32 = np.cos(ang), np.sin(ang)
    cosm = np.tile(c32, (4, 1)).astype(bf)                                 # [128,S]
    sgn = np.concatenate([-s32, s32], axis=0)                              # [64,S]
    sinm = np.tile(sgn, (2, 1)).astype(bf)
    p = np.arange(128)
    j = np.arange(STRIPW)
    win = ((j[None, :] >= p[:, None]) & (j[None, :] - p[:, None] <= WIN)).astype(np.float32)
    iotaw = np.ascontiguousarray(np.broadcast_to(j[None, :].astype(np.float32), (128, STRIPW)))
    return cosm, sinm, win, iotaw


def build_nc():
    if "nc" in _nc_cache:
        return _nc_cache["nc"]
    nc = bass.Bass(num_devices=NCORES)

    # ---- per-core external I/O (core c = 4*b + g: batch b, head-group g) ----
    # blob16: tok quarter [DIM,512] | wall half | wo half | vr | ubq | ubr | gate1 | mraw
    blob16_d = nc.dram_tensor("blob16", [BLOB16_N], BF16, kind="ExternalInput")
    out_d = nc.dram_tensor("out", [512, DIM], BF16, kind="ExternalOutput")

    # ---- NEFF-embedded constants (shipped at model load, not per call) ----
    cosm_np, sinm_np, win_np, iotaw_np = _consts()
    cos_d = nc.inline_tensor(cosm_np, name="cosk")
    sin_d = nc.inline_tensor(sinm_np, name="sink")
    win_d = nc.inline_tensor(win_np, name="wink")
    iota_d = nc.inline_tensor(iotaw_np, name="iotak")

    # ---- internal DRAM for collectives ----
    tok_stage = nc.dram_tensor("tok_stage", [TOK_N], BF16, kind="Internal")
    wall_stage = nc.dram_tensor("wall_stage", [WALL_N], BF16, kind="Internal")
    wo_stage = nc.dram_tensor("wo_stage", [WO_N], BF16, kind="Internal")
    tok_g = nc.dram_tensor("tok_g", [4 * TOK_N], BF16, kind="Internal")
    wall_g = nc.dram_tensor("wall_g", [2 * WALL_N], BF16, kind="Internal")
    wo_g = nc.dram_tensor("wo_g", [2 * WO_N], BF16, kind="Internal")
    pout_d = nc.dram_tensor("pout", [S, DIM], BF16, kind="Internal")
    rs_d = nc.dram_tensor("rsout", [512, DIM], BF16, kind="Internal")

    s_stage = nc.alloc_semaphore("s_stage")
    s_ag = nc.alloc_semaphore("s_ag")
    s_rs = nc.alloc_semaphore("s_rs")
    s_fin = nc.alloc_semaphore("s_fin")

    G4 = [[0, 1, 2, 3], [4, 5, 6, 7]]        # batch groups (head-parallel)
    G2 = [[0, 4], [1, 5], [2, 6], [3, 7]]    # same head-group pairs across batches

    # ---- pre-tile: stage inputs to internal DRAM, gather across cores ----
    nc.gpsimd.dma_start(out=tok_stage[:], in_=blob16_d[0:TOK_N]).then_inc(s_stage, 16)
    nc.gpsimd.dma_start(
        out=wall_stage[:], in_=blob16_d[TOK_N : TOK_N + WALL_N]
    ).then_inc(s_stage, 16)
    nc.gpsimd.dma_start(
        out=wo_stage[:], in_=blob16_d[TOK_N + WALL_N : TOK_N + WALL_N + WO_N]
    ).then_inc(s_stage, 16)
    nc.gpsimd.wait_ge(s_stage, 48)
    nc.gpsimd.collective_compute(
        "AllGather", mybir.AluOpType.bypass, replica_groups=G4,
        ins=[tok_stage[:].opt()], outs=[tok_g[:].opt()],
    ).then_inc(s_ag, 1)
    nc.gpsimd.collective_compute(
        "AllGather", mybir.AluOpType.bypass, replica_groups=G2,
        ins=[wall_stage[:].opt()], outs=[wall_g[:].opt()],
    ).then_inc(s_ag, 1)
    nc.gpsimd.collective_compute(
        "AllGather", mybir.AluOpType.bypass, replica_groups=G2,
        ins=[wo_stage[:].opt()], outs=[wo_g[:].opt()],
    ).then_inc(s_ag, 1)
    nc.gpsimd.wait_ge(s_ag, 3)

    with tile.TileContext(nc) as tc:
        with (
            tc.tile_pool(name="big", bufs=1) as big,
            tc.tile_pool(name="stg", bufs=2) as stg,
            tc.tile_pool(name="pp", bufs=2, space=bass.MemorySpace.PSUM) as pp,
        ):
            # ---- resident SBUF slabs ----
            tok = big.tile([128, NKC * S], BF16, tag="tok")          # 32KB/p
            wsl = big.tile([128, NKC * WSLW], BF16, tag="wsl")       # 20KB/p
            wo_sb = big.tile([128, 2 * DIM], BF16, tag="wo")         # 4KB/p
            cosm = big.tile([128, S], BF16, tag="cos")
            sinm = big.tile([128, S], BF16, tag="sin")
            msl = big.tile([128, NSB * STRIPW], BF16, tag="msl")     # 20KB/p
            winsb = big.tile([128, STRIPW], F32, tag="winsb")
            iotasb = big.tile([128, STRIPW], F32, tag="iotasb")
            ubq_t = big.tile([128, NSB], BF16, tag="ubq")
            ubr_t = big.tile([128, NSB], BF16, tag="ubr")
            ubl = big.tile([128, NSB], F32, tag="ubl")
            vrs = [big.tile([128, NSB * D], BF16, tag=f"vr{h}", name=f"vr{h}") for h in range(HPC)]
            vaug = [big.tile([128, NSB * 65], BF16, tag=f"va{h}", name=f"va{h}") for h in range(HPC)]
            mixs = big.tile([128, 64], F32, tag="mix")               # sigmoid(mix)
            mixr = big.tile([128, 64], BF16, tag="mixr")             # uploaded raw mix
            gate1 = big.tile([1, HPC * S], BF16, tag="gate1")        # uploaded sigmoid(gate)
            ones1 = big.tile([1, 64], BF16, tag="ones1")
            qkslab = big.tile([128, 8 * S], BF16, tag="qkslab")
            qraw = [qkslab[:, 0 * S : 1 * S], qkslab[:, 1 * S : 2 * S]]
            kraw = [qkslab[:, 2 * S : 3 * S], qkslab[:, 3 * S : 4 * S]]
            qrot = [qkslab[:, 4 * S : 5 * S], qkslab[:, 5 * S : 6 * S]]
            krot = [qkslab[:, 6 * S : 7 * S], qkslab[:, 7 * S : 8 * S]]
            qro, kro = qraw, kraw  # roped in place
            # PT ring: 5 live strips per head
            pts = [big.tile([128, 5 * STRIPW], BF16, tag=f"pt{h}", name=f"pt{h}") for h in range(HPC)]
            outg = [big.tile([128, S], BF16, tag=f"og{p}", name=f"og{p}") for p in range(2)]
            vtmp = big.tile([128, D], F32, tag="vtmp")
            dmy = big.tile([1, 128], F32, tag="dmy")
            dmyc = [0]

            pend = []

            def guard(inst):
                if pend:
                    s = InstructionNameOrderedSet()
                    for n in pend:
                        s.add(n)
                    inst.ins.add_nosync_dependencies_from(s)
                    pend.clear()
                return inst

            def absorb(*aps):
                for ap in aps:
                    i = dmyc[0] % 128
                    dmyc[0] += 1
                    ii = nc.vector.tensor_copy(dmy[0:1, i : i + 1], ap[0:1, 0:1])
                    pend.append(ii.ins.name)

            dmyA = big.tile([1, 128], F32, tag="dmyA")
            dmyAc = [0]

            def absorb_act(ap):
                i = dmyAc[0] % 128
                dmyAc[0] += 1
                ii = nc.scalar.copy(dmyA[0:1, i : i + 1], ap[0:1, 0:1])
                pend.append(ii.ins.name)

            bcb = big.tile([32, 1024], BF16, tag="bcb")
            bcbc = [0]
            crumb_st = {"last": None}

            def crumb(src_ap):
                crumb_st["last"] = src_ap[0:1, 0:1]

            def pe_absorb(ap=None):
                ap = ap if ap is not None else crumb_st["last"]
                if ap is None:
                    return
                if ap.partition_size() >= 32 and ap.dtype == BF16:
                    ii = nc.tensor.ldweights(ap[0:32, 0:1])
                else:
                    i = bcbc[0] % 1024
                    bcbc[0] += 1
                    nc.vector.tensor_copy(bcb[0:1, i : i + 1], ap[0:1, 0:1])
                    ii = nc.tensor.ldweights(bcb[0:32, i : i + 1])
                pend.append(ii.ins.name)

            # ---- loads (from gathered internal DRAM + inline consts) ----
            # tokens: one strided DMA per gathered quarter
            for q in range(4):
                nc.gpsimd.dma_start(
                    out=tok.rearrange("p (k qq s) -> p k qq s", k=NKC, qq=4, s=512)[
                        :, :, q, :
                    ],
                    in_=tok_g[q * TOK_N : (q + 1) * TOK_N].rearrange(
                        "(k p s) -> p k s", k=NKC, p=128, s=512
                    ),
                )
            for kc in range(NKC):
                src = wall_g[kc * 128 * WUP : (kc + 1) * 128 * WUP].rearrange(
                    "(p c) -> p c", p=128, c=WUP
                )
                nc.gpsimd.dma_start(
                    out=wsl[:, kc * WSLW : kc * WSLW + 512], in_=src[:, 0:512]
                )
                nc.gpsimd.dma_start(
                    out=wsl[:, kc * WSLW + 1024 : kc * WSLW + 1280], in_=src[:, 512:768]
                )
            for half in range(2):
                nc.gpsimd.dma_start(
                    out=wo_sb[:, half * DIM : half * DIM + DIM],
                    in_=wo_g[half * WO_N : (half + 1) * WO_N].rearrange(
                        "(p d) -> p d", p=128, d=DIM
                    ),
                )
            nc.gpsimd.dma_start(out=cosm[:], in_=cos_d[:])
            nc.gpsimd.dma_start(out=sinm[:], in_=sin_d[:])
            nc.gpsimd.dma_start(out=winsb[:], in_=win_d[:])
            nc.gpsimd.dma_start(out=iotasb[:], in_=iota_d[:])
            off = TOK_N + WALL_N + WO_N + VR_N
            nc.gpsimd.dma_start(
                out=ubq_t[:],
                in_=blob16_d[off : off + UB_N].rearrange("(p n) -> p n", p=128, n=NSB),
            )
            off += UB_N
            nc.gpsimd.dma_start(
                out=ubr_t[:],
                in_=blob16_d[off : off + UB_N].rearrange("(p n) -> p n", p=128, n=NSB),
            )
            off += UB_N
            nc.gpsimd.dma_start(
                out=gate1[:],
                in_=blob16_d[off : off + GR_N].rearrange("(o x) -> o x", o=1, x=GR_N),
            )
            off += GR_N
            nc.gpsimd.dma_start(
                out=mixr[:],
                in_=blob16_d[off : off + MR_N].rearrange("(p n) -> p n", p=128, n=64),
            )
            # vr: one strided DMA per head
            vr0 = TOK_N + WALL_N + WO_N
            for h in range(HPC):
                nc.gpsimd.dma_start(
                    out=vrs[h].rearrange("p (sb d) -> p sb d", sb=NSB, d=D),
                    in_=blob16_d[vr0 + h * NSB * 128 * D : vr0 + (h + 1) * NSB * 128 * D]
                    .rearrange("(sb p d) -> p sb d", sb=NSB, p=128, d=D),
                )

            nc.vector.memset(ones1[:], 1.0)
            absorb(cosm, sinm, gate1)
            absorb_act(mixr[0:1, 0:1])

            # ---- ub reconstruct + masks on device: msl = win * (iota <= ub) ----
            absorb(ubq_t, ubr_t)
            guard(nc.vector.scalar_tensor_tensor(
                ubl[:], ubq_t[:], 256.0, ubr_t[:],
                mybir.AluOpType.mult, mybir.AluOpType.add,
            ))
            absorb(winsb, iotasb)
            for kb in range(NSB):
                guard(nc.vector.scalar_tensor_tensor(
                    msl[:, kb * STRIPW : kb * STRIPW + STRIPW],
                    iotasb[:],
                    ubl[:, kb : kb + 1],
                    winsb[:],
                    mybir.AluOpType.is_le,
                    mybir.AluOpType.mult,
                ))

            # ---- rot weights on device: wqrot/wkrot = 32-col half-swap of wq/wk ----
            for kc in range(NKC):
                base = kc * WSLW
                for h in range(HPC):
                    for dst0, src0 in ((0, 32), (32, 0)):
                        nc.vector.tensor_copy(
                            wsl[:, base + 512 + h * 64 + dst0 : base + 512 + h * 64 + dst0 + 32],
                            wsl[:, base + h * 64 + src0 : base + h * 64 + src0 + 32],
                        )
                        nc.vector.tensor_copy(
                            wsl[:, base + 768 + h * 64 + dst0 : base + 768 + h * 64 + dst0 + 32],
                            wsl[:, base + 256 + h * 64 + src0 : base + 256 + h * 64 + src0 + 32],
                        )

            def wchunk(kc, c0, c1):
                return wsl[:, kc * WSLW + c0 : kc * WSLW + c1]

            def tchunk(kc, s0, s1):
                return tok[:, kc * S + s0 : kc * S + s1]

            # ---- phase 1: T-orient projections: q, k (dual use) ----
            NS = 4  # seq chunks of 512
            for dest, c0 in (
                (qraw[0], 0), (qraw[1], 128), (kraw[0], 256), (kraw[1], 384),
                (qrot[0], 512), (qrot[1], 640), (krot[0], 768), (krot[1], 896),
            ):
                for ns in range(NS):
                    ps = pp.tile([128, 512], F32, tag="ps1", name="psA")
                    pe_absorb()
                    for kc in range(NKC):
                        guard(nc.tensor.matmul(
                            ps[:],
                            wchunk(kc, c0, c0 + 128),
                            tchunk(kc, ns * 512, ns * 512 + 512),
                            start=(kc == 0),
                            stop=(kc == NKC - 1),
                        ))
                    nc.vector.tensor_copy(dest[:, ns * 512 : ns * 512 + 512], ps[:])
                    crumb(dest[:, ns * 512 : ns * 512 + 512])

            # ---- phase 2: v (natural orient) + lerp with value residual ----
            for sb in range(NSB):
                ps = pp.tile([128, 256], F32, tag="ps1", name="psV")
                pe_absorb()
                for kc in range(NKC):
                    guard(nc.tensor.matmul(
                        ps[:],
                        tchunk(kc, sb * 128, sb * 128 + 128),
                        wchunk(kc, 1024, 1280),
                        start=(kc == 0),
                        stop=(kc == NKC - 1),
                    ))
                absorb_act(mixr[0:1, sb * 4 : sb * 4 + 1])
                guard(nc.scalar.activation(
                    mixs[:, sb * 4 : sb * 4 + 4], mixr[:, sb * 4 : sb * 4 + 4], AF.Sigmoid
                ))
                for h in range(HPC):
                    absorb(vrs[h][:, sb * D : sb * D + D])
                    guard(nc.vector.tensor_sub(
                        vtmp[:],
                        vrs[h][:, sb * D : sb * D + D],
                        ps[:, h * D : h * D + D],
                    ))
                    absorb(mixs[:, sb * 4 + h : sb * 4 + h + 1])
                    # v' = mix*(vr - v) + v
                    guard(nc.vector.scalar_tensor_tensor(
                        vaug[h][:, sb * 65 : sb * 65 + 64],
                        vtmp[:],
                        mixs[:, sb * 4 + h : sb * 4 + h + 1],
                        ps[:, h * D : h * D + D],
                        mybir.AluOpType.mult,
                        mybir.AluOpType.add,
                    ))
                    nc.vector.memset(vaug[h][:, sb * 65 + 64 : sb * 65 + 65], 1.0)
                crumb(vaug[HPC - 1][:, sb * 65 : sb * 65 + 64])

            # ---- phase 1b: RoPE via half-swap + cos/sin maps ----
            for raw, rot in (
                (qraw[0], qrot[0]),
                (qraw[1], qrot[1]),
                (kraw[0], krot[0]),
                (kraw[1], krot[1]),
            ):
                nc.vector.tensor_mul(rot[:], rot[:], sinm[:])
                nc.vector.tensor_mul(raw[:], raw[:], cosm[:])
                nc.vector.tensor_add(raw[:], raw[:], rot[:])
                crumb(raw[:])

            # ---- phase 3: attention ----
            ptw_hist, ring_hist, fbs_hist, og_hist = [], [], [], []
            for kb in range(NSB):
                Wn = min(STRIPW, S - kb * 128)
                for h in range(HPC):
                    p, hh = divmod(h, 2)
                    b0 = hh * 64
                    ptv = pts[h][:, (kb % 5) * STRIPW : (kb % 5) * STRIPW + STRIPW]
                    sim = pp.tile([128, STRIPW], F32, tag="psS", name="psS")
                    pe_absorb(kro[p])
                    pe_absorb(qro[p])
                    if len(ptw_hist) >= 1:
                        pe_absorb(ptw_hist[-1])
                    for c0 in range(0, Wn, 512):
                        c1 = min(c0 + 512, Wn)
                        guard(nc.tensor.matmul(
                            sim[:, c0:c1],
                            kro[p][b0 : b0 + 64, kb * 128 : kb * 128 + 128],
                            qro[p][b0 : b0 + 64, kb * 128 + c0 : kb * 128 + c1],
                            start=True,
                            stop=True,
                        ))
                    ptw = stg.tile([128, STRIPW], BF16, tag="ptw", name="ptw", bufs=2)
                    if ring_hist:
                        absorb_act(ring_hist[-1][0:1, 0:1])
                    absorb_act(sim[0:1, 0:1])
                    guard(nc.scalar.activation(
                        ptw[:, 0:Wn], sim[:, 0:Wn], AF.Exp, scale=EXPSC
                    ))
                    ptw_hist.append(ptw)
                    absorb(ptw[0:1, 0:1], ptv[0:1, 0:1])
                    guard(nc.vector.tensor_mul(
                        ptv[:, 0:Wn],
                        ptw[:, 0:Wn],
                        msl[:, kb * STRIPW : kb * STRIPW + Wn],
                    ))
                    ring_hist.append(ptv)
                    # AV for q-block qb = kb
                    av = pp.tile([65, 128], F32, tag="psAV", name="psAV", bufs=1)
                    pe_absorb(ptv)
                    if og_hist:
                        pe_absorb(og_hist[-1])
                    if fbs_hist:
                        pe_absorb(fbs_hist[-1][0:1, 0:1])
                    srcs = list(range(max(0, kb - 4), kb + 1))
                    for j, sc in enumerate(srcs):
                        off2 = (kb - sc) * 128
                        psrc = pts[h][:, (sc % 5) * STRIPW + off2 : (sc % 5) * STRIPW + off2 + 128]
                        guard(nc.tensor.matmul(
                            av[:],
                            vaug[h][:, sc * 65 : sc * 65 + 65],
                            psrc,
                            start=(j == 0),
                            stop=(j == len(srcs) - 1),
                        ))
                    # normalize + gate
                    rec_sb = big.tile([1, 128], F32, tag="recsb", name="recsb")
                    f_row = big.tile([1, 128], BF16, tag="frow", name="frow")
                    gsl = gate1[0:1, h * S + kb * 128 : h * S + kb * 128 + 128]
                    nc.vector.reciprocal(rec_sb[:], av[64:65, :])
                    absorb(gsl)
                    guard(nc.vector.tensor_mul(f_row[:], rec_sb[:], gsl))
                    pe_absorb(f_row[0:1, 0:1])
                    if fbs_hist:
                        pe_absorb(fbs_hist[-1][0:1, 0:1])
                    fps = pp.tile([64, 128], F32, tag="fps", name="fps", bufs=1)
                    guard(nc.tensor.matmul(fps[:], ones1[:], f_row[:], start=True, stop=True))
                    fbs = stg.tile([64, 128], F32, tag="fbs", name="fbs", bufs=1)
                    nc.vector.tensor_copy(fbs[:], fps[:])
                    fbs_hist.append(fbs)
                    guard(nc.vector.tensor_mul(
                        outg[p][b0 : b0 + 64, kb * 128 : kb * 128 + 128],
                        av[0:64, :],
                        fbs[:],
                    ))
                    og_hist.append(outg[p][b0 : b0 + 1, kb * 128 : kb * 128 + 1])

            # ---- phase 4: Wo -> partial out (internal DRAM) ----
            ost_hist = []
            crumb(outg[0][:, S - 128 : S])
            crumb(outg[1][:, S - 128 : S])
            for g8 in range(4):
                slab = qkslab[:, g8 * 4 * DIM : (g8 + 1) * 4 * DIM]
                for j in range(4):
                    sb = g8 * 4 + j
                    for half in range(2):
                        ps = pp.tile([128, 512], F32, tag="ps1", name="psO")
                        pe_absorb()
                        if ost_hist:
                            pe_absorb(ost_hist[-1])
                        for kc in range(2):
                            guard(nc.tensor.matmul(
                                ps[:],
                                outg[kc][:, sb * 128 : sb * 128 + 128],
                                wo_sb[:, kc * DIM + half * 512 : kc * DIM + half * 512 + 512],
                                start=(kc == 0),
                                stop=(kc == 1),
                            ))
                        dst = slab[:, j * DIM + half * 512 : j * DIM + half * 512 + 512]
                        absorb(ps[0:1, 0:1])
                        guard(nc.vector.tensor_copy(dst, ps[:]))
                        ost_hist.append(dst)
                nc.sync.dma_start(
                    out=pout_d[g8 * 512 : g8 * 512 + 512, :].rearrange(
                        "(sb p) d -> p sb d", p=128
                    ),
                    in_=slab.rearrange("p (sb d) -> p sb d", d=DIM),
                )

    # ---- post-tile (drain guarantees all DMAs done): reduce partials ----
    nc.gpsimd.collective_compute(
        "ReduceScatter", mybir.AluOpType.add, replica_groups=G4,
        ins=[pout_d[:].opt()], outs=[rs_d[:].opt()],
    ).then_inc(s_rs, 1)
    nc.gpsimd.wait_ge(s_rs, 1)
    nc.gpsimd.dma_start(out=out_d[:], in_=rs_d[:]).then_inc(s_fin, 16)
    nc.sync.wait_ge(s_fin, 16)

    _nc_cache["nc"] = nc
    return nc


def _prep_all(tokens, value_residual, episode_ids, Wq, Wkv, Wo, Wg, Wmix):
    bf = ml_dtypes.bfloat16
    perm = np.concatenate([np.arange(0, D, 2), np.arange(1, D, 2)])

    tokens = np.asarray(tokens, dtype=np.float32)
    tokT = [np.ascontiguousarray(tokens[b].T).astype(bf) for b in range(B)]   # [DIM,S]

    Wqp = np.asarray(Wq).reshape(DIM, H, D)[:, :, perm] * QKSC
    Wkp = np.asarray(Wkv)[:, : H * D].reshape(DIM, H, D)[:, :, perm] * QKSC
    Wvp = np.asarray(Wkv)[:, H * D :].reshape(DIM, H, D)
    Wop = np.asarray(Wo).reshape(H, D, DIM)
    packs = []
    for g in range(4):
        hs = slice(4 * g, 4 * g + 4)
        wall = np.concatenate(
            [
                Wqp[:, hs].reshape(DIM, 256),
                Wkp[:, hs].reshape(DIM, 256),
                Wvp[:, hs].reshape(DIM, 256),
            ],
            axis=1,
        ).astype(bf)                                                          # [DIM, 768]
        wall = np.ascontiguousarray(wall.reshape(NKC, 128, WUP))
        wo = np.ascontiguousarray(
            Wop[hs].reshape(256, DIM).astype(bf).reshape(2, 128, DIM)
        )
        packs.append((wall, wo))

    # gate/mix projections on host (tiny, keeps sigmoid paths off fp8)
    graw_all = np.einsum("bnd,dh->bhn", tokens, np.asarray(Wg, dtype=np.float32))   # [B,H,S]
    gate1_all = 1.0 / (1.0 + np.exp(-graw_all))
    mraw_all = np.einsum("bnd,dh->bnh", tokens, np.asarray(Wmix, dtype=np.float32)) # [B,S,H]

    ubqs, ubrs = [], []
    kk = np.arange(S)
    for b in range(B):
        ids = np.asarray(episode_ids[b])
        ee = np.searchsorted(ids, ids, side="right") - 1                      # [S]
        ubl = ee - (kk // 128) * 128                                          # int
        q = ubl // 256
        r = ubl - 256 * q
        ubqs.append(np.ascontiguousarray(q.reshape(NSB, 128).T.astype(bf)))   # [128,NSB]
        ubrs.append(np.ascontiguousarray(r.reshape(NSB, 128).T.astype(bf)))

    vr16 = np.asarray(value_residual).astype(bf)

    in_maps = []
    for c in range(NCORES):
        b, g = divmod(c, 4)
        wall, wo = packs[g]
        hs = slice(4 * g, 4 * g + 4)

        g1 = gate1_all[b, hs].astype(bf)                                      # [4,S]
        mraw = np.ascontiguousarray(
            mraw_all[b, :, hs].reshape(NSB, 128, 4).transpose(1, 0, 2).reshape(128, 64)
        ).astype(bf)                                                          # [128,64]
        blob16 = np.concatenate([
            tokT[b][:, g * 512 : (g + 1) * 512].ravel(),
            wall[4 * b : 4 * b + 4].ravel(),
            wo[b].ravel(),
            vr16[b, hs].ravel(),
            ubqs[b].ravel(), ubrs[b].ravel(),
            g1.ravel(), mraw.ravel(),
        ])
        in_maps.append({"blob16": blob16})
    return in_maps


# ---- cached PJRT dispatch: jit built once, zero output-buffers device-resident ----
_runner_cache = {}


def _get_runner(nc):
    if "fn" in _runner_cache:
        return _runner_cache
    from jax.experimental.shard_map import shard_map

    bass2jax.install_neuronx_cc_hook()
    partition_name = nc.partition_id_tensor.name if nc.partition_id_tensor else None
    in_names, out_names, out_avals = [], [], []
    for alloc in nc.m.functions[0].allocations:
        if not isinstance(alloc, mybir.MemoryLocationSet):
            continue
        name = alloc.memorylocations[0].name
        if alloc.kind == "ExternalInput":
            if name != partition_name:
                in_names.append(name)
        elif alloc.kind == "ExternalOutput":
            out_avals.append(
                jax.core.ShapedArray(tuple(alloc.tensor_shape), mybir.dt.np(alloc.dtype))
            )
            out_names.append(name)
    in_names_all = list(in_names) + list(out_names)
    if partition_name is not None:
        in_names_all.append(partition_name)

    def _body(*args):
        operands = list(args)
        if partition_name is not None:
            operands.append(bass2jax.partition_id_tensor())
        outs = bass2jax._bass_exec_p.bind(
            *operands,
            out_avals=tuple(out_avals),
            in_names=tuple(in_names_all),
            out_names=tuple(out_names),
            lowering_input_output_aliases=(),
            sim_require_finite=True,
            sim_require_nnan=True,
            nc=nc,
        )
        return tuple(outs)

    devices = jax.devices()[:NCORES]
    mesh = jax.sharding.Mesh(np.asarray(devices), ("core",))
    P = jax.sharding.PartitionSpec
    n_in = len(in_names) + len(out_names)
    fn = jax.jit(
        shard_map(
            _body, mesh=mesh, in_specs=(P("core"),) * n_in,
            out_specs=(P("core"),) * len(out_names), check_rep=False,
        ),
        keep_unused=True,
    )
    sh = jax.sharding.NamedSharding(mesh, P("core"))
    zeros_dev = [
        jax.device_put(np.zeros((NCORES * a.shape[0], *a.shape[1:]), a.dtype), sh)
        for a in out_avals
    ]
    jax.block_until_ready(zeros_dev)
    _runner_cache.update(fn=fn, in_names=in_names, zeros_dev=zeros_dev)
    return _runner_cache


def _execute(nc, in_maps):
    r = _get_runner(nc)
    concat_in = [
        np.concatenate([np.asarray(m[name]) for m in in_maps], axis=0)
        for name in r["in_names"]
    ]
    outs = r["fn"](*concat_in, *r["zeros_dev"])
    out_full = np.asarray(outs[0]).reshape(NCORES, 512, DIM)
    res = np.zeros((B, S, DIM), dtype=np.float32)
    for c in range(NCORES):
        b, rk = divmod(c, 4)
        res[b, rk * 512 : (rk + 1) * 512] = out_full[c].astype(np.float32)
    return res


def kernel(tokens, value_residual, episode_ids, Wq, Wkv, Wo, Wg, Wmix):
    nc = build_nc()
    in_maps = _prep_all(tokens, value_residual, episode_ids, Wq, Wkv, Wo, Wg, Wmix)
    return _execute(nc, in_maps)


# revision 14
# speedup vs baseline: 6.0155x; 1.3537x over previous
import numpy as np
import ml_dtypes

import jax
import concourse.bass as bass
from bass_rust import InstructionNameOrderedSet
import concourse.mybir as mybir
from concourse import tile
from concourse import bass2jax

BF16 = mybir.dt.bfloat16
F32 = mybir.dt.float32
I8 = mybir.dt.int8
AF = mybir.ActivationFunctionType

B, S, DIM, H, D = 2, 2048, 1024, 16, 64
WIN = 512
HPC = 4          # heads per core
NCORES = 8
NSB = S // 128   # 16 seq blocks
NKC = DIM // 128  # 8 contraction chunks
STRIPW = 640     # 128 keys attend to <=640 queries (dist 0..512 + 127)
WUP = 768        # uploaded weight cols per kc chunk: wq 256 | wk 256 | wv 256 (int8)
WSLW = 1280      # sbuf slab cols per kc chunk: wq | wk | wqrot | wkrot | wv

# int8 blob layout (elements): tok quarter | wall half   (per-tensor scales in blob16)
TOK_N = DIM * 512                 # 524288
WALL_N = 4 * 128 * WUP            # 393216
BLOB8_N = TOK_N + WALL_N          # 917504
# bf16 blob layout (elements): wo | vr | ubq | ubr | gate1 | mraw | cvec [128,4]
WO_N = 128 * DIM                  # 131072
VR_N = HPC * S * D                # 524288
UB_N = 128 * NSB                  # 2048
GR_N = 4 * S                     # 8192
MR_N = 128 * 64                   # 8192
CV_N = 128 * 4                    # 512
BLOB16_N = WO_N + VR_N + 2 * UB_N + GR_N + MR_N + CV_N

_nc_cache = {}


def _patched_drain(self, tick_clock, wait_clock):
    # Tail drain: walrus limits sync waits per instruction, so convert the
    # multi-wait drain into a chain of single-wait sem waits on SyncE.
    from concourse.vector_clock import ScopedClock

    nc = self.nc
    probe = mybir.InstNoOp(name="__drain_probe", engine=mybir.EngineType.SP, ins=[], outs=[])
    wait_clock.add_sem_waits(probe, ScopedClock({None: tick_clock.global_clock}))
    id2h = {h.num: h for h in self.sems.allocated().values()}
    si = getattr(probe, "sync_info", None)
    if si is not None:
        for w in si.on_wait:
            h = id2h.get(w.id)
            if h is not None:
                nc.sync.wait_ge(h, w.wait_value)
    nc.sync.drain()
    nc.all_engine_barrier()
    popped = nc._tile_sem_poison_stack.pop()
    assert popped is self._sem_poison
    nc.clear_and_free_semaphores(list(self.sems.allocated().values()))
    nc.all_engine_barrier()


tile.TileContext._drain_and_barrier = _patched_drain


def _consts():
    bf = ml_dtypes.bfloat16
    pos = np.arange(S, dtype=np.float64)
    invf = 1.0 / (10000.0 ** (np.arange(0, D, 2, dtype=np.float64) / D))   # [32]
    ang = pos[None, :] * invf[:, None]                                     # [32,S]
    c32, s32 = np.cos(ang), np.sin(ang)
    cosm = np.tile(c32, (4, 1)).astype(bf)                                 # [128,S]
    sgn = np.concatenate([-s32, s32], axis=0)                              # [64,S]
    sinm = np.tile(sgn, (2, 1)).astype(bf)
    p = np.arange(128)
    j = np.arange(STRIPW)
    win = ((j[None, :] >= p[:, None]) & (j[None, :] - p[:, None] <= WIN)).astype(np.float32)
    iotaw = np.ascontiguousarray(np.broadcast_to(j[None, :].astype(np.float32), (128, STRIPW)))
    return cosm, sinm, win, iotaw


def build_nc():
    if "nc" in _nc_cache:
        return _nc_cache["nc"]
    nc = bass.Bass(num_devices=NCORES)

    # ---- per-core external I/O (core c = 4*b + g: batch b, head-group g) ----
    # blob8: tok quarter [DIM,512] int8 | wall half [4,128,WUP] int8
    blob8_d = nc.dram_tensor("blob8", [BLOB8_N], I8, kind="ExternalInput")
    # blob16: wo half | vr | ubq | ubr | gate1 | mraw | cvec
    blob16_d = nc.dram_tensor("blob16", [BLOB16_N], BF16, kind="ExternalInput")
    out_d = nc.dram_tensor("out", [512, DIM], I8, kind="ExternalOutput")
    outsc_d = nc.dram_tensor("outsc", [128, 1], F32, kind="ExternalOutput")

    # ---- NEFF-embedded constants (shipped at model load, not per call) ----
    cosm_np, sinm_np, win_np, iotaw_np = _consts()
    cos_d = nc.inline_tensor(cosm_np, name="cosk")
    sin_d = nc.inline_tensor(sinm_np, name="sink")
    win_d = nc.inline_tensor(win_np, name="wink")
    iota_d = nc.inline_tensor(iotaw_np, name="iotak")

    # ---- internal DRAM for collectives ----
    tok_stage = nc.dram_tensor("tok_stage", [TOK_N], I8, kind="Internal")
    wall_stage = nc.dram_tensor("wall_stage", [WALL_N], I8, kind="Internal")
    wo_stage = nc.dram_tensor("wo_stage", [WO_N], BF16, kind="Internal")
    tok_g = nc.dram_tensor("tok_g", [4 * TOK_N], I8, kind="Internal")
    wall_g = nc.dram_tensor("wall_g", [2 * WALL_N], I8, kind="Internal")
    wo_g = nc.dram_tensor("wo_g", [2 * WO_N], BF16, kind="Internal")
    pout_d = nc.dram_tensor("pout", [S, DIM], BF16, kind="Internal")
    rs_d = nc.dram_tensor("rsout", [512, DIM], BF16, kind="Internal")

    s_stage = nc.alloc_semaphore("s_stage")
    s_ag = nc.alloc_semaphore("s_ag")
    s_rs = nc.alloc_semaphore("s_rs")
    s_fin = nc.alloc_semaphore("s_fin")

    G4 = [[0, 1, 2, 3], [4, 5, 6, 7]]        # batch groups (head-parallel)
    G2 = [[0, 4], [1, 5], [2, 6], [3, 7]]    # same head-group pairs across batches

    # ---- pre-tile: stage inputs to internal DRAM, gather across cores ----
    nc.gpsimd.dma_start(out=tok_stage[:], in_=blob8_d[0:TOK_N]).then_inc(s_stage, 16)
    nc.gpsimd.dma_start(
        out=wall_stage[:], in_=blob8_d[TOK_N : TOK_N + WALL_N]
    ).then_inc(s_stage, 16)
    nc.gpsimd.dma_start(out=wo_stage[:], in_=blob16_d[0:WO_N]).then_inc(s_stage, 16)
    nc.gpsimd.wait_ge(s_stage, 48)
    nc.gpsimd.collective_compute(
        "AllGather", mybir.AluOpType.bypass, replica_groups=G4,
        ins=[tok_stage[:].opt()], outs=[tok_g[:].opt()],
    ).then_inc(s_ag, 1)
    nc.gpsimd.collective_compute(
        "AllGather", mybir.AluOpType.bypass, replica_groups=G2,
        ins=[wall_stage[:].opt()], outs=[wall_g[:].opt()],
    ).then_inc(s_ag, 1)
    nc.gpsimd.collective_compute(
        "AllGather", mybir.AluOpType.bypass, replica_groups=G2,
        ins=[wo_stage[:].opt()], outs=[wo_g[:].opt()],
    ).then_inc(s_ag, 1)
    nc.gpsimd.wait_ge(s_ag, 3)

    with tile.TileContext(nc) as tc:
        with (
            tc.tile_pool(name="big", bufs=1) as big,
            tc.tile_pool(name="stg", bufs=2) as stg,
            tc.tile_pool(name="pp", bufs=2, space=bass.MemorySpace.PSUM) as pp,
        ):
            # ---- resident SBUF slabs ----
            tok = big.tile([128, NKC * S], BF16, tag="tok")          # 32KB/p
            wsl = big.tile([128, NKC * WSLW], BF16, tag="wsl")       # 20KB/p
            wo_sb = big.tile([128, 2 * DIM], BF16, tag="wo")         # 4KB/p
            cosm = big.tile([128, S], BF16, tag="cos")
            sinm = big.tile([128, S], BF16, tag="sin")
            tokq8 = big.tile([128, NKC * S], I8, tag="tokq8")        # 16KB/p
            wslq8 = big.tile([128, NKC * WUP], I8, tag="wslq8")      # 6KB/p
            winsb = big.tile([128, STRIPW], F32, tag="winsb")
            iotasb = big.tile([128, STRIPW], F32, tag="iotasb")
            ubq_t = big.tile([128, NSB], BF16, tag="ubq")
            ubr_t = big.tile([128, NSB], BF16, tag="ubr")
            ubl = big.tile([128, NSB], F32, tag="ubl")
            vrs = [big.tile([128, NSB * D], BF16, tag=f"vr{h}", name=f"vr{h}") for h in range(HPC)]
            vaug = [big.tile([128, NSB * 65], BF16, tag=f"va{h}", name=f"va{h}") for h in range(HPC)]
            mixs = big.tile([128, 64], F32, tag="mix")               # sigmoid(mix)
            mixr = big.tile([128, 64], BF16, tag="mixr")             # uploaded raw mix
            gate1 = big.tile([1, HPC * S], BF16, tag="gate1")        # uploaded sigmoid(gate)
            cvec = big.tile([128, 4], BF16, tag="cvec")              # scale consts hi/lo
            cexp = big.tile([128, 1], F32, tag="cexp")               # exp logit scale
            csv = big.tile([128, 1], F32, tag="csv")                 # v dequant scale
            ones1 = big.tile([1, 64], BF16, tag="ones1")
            qkslab = big.tile([128, 8 * S], BF16, tag="qkslab")
            qraw = [qkslab[:, 0 * S : 1 * S], qkslab[:, 1 * S : 2 * S]]
            kraw = [qkslab[:, 2 * S : 3 * S], qkslab[:, 3 * S : 4 * S]]
            qrot = [qkslab[:, 4 * S : 5 * S], qkslab[:, 5 * S : 6 * S]]
            krot = [qkslab[:, 6 * S : 7 * S], qkslab[:, 7 * S : 8 * S]]
            qro, kro = qraw, kraw  # roped in place
            # PT ring: 5 live strips per head
            pts = [big.tile([128, 5 * STRIPW], BF16, tag=f"pt{h}", name=f"pt{h}") for h in range(HPC)]
            outg = [big.tile([128, S], BF16, tag=f"og{p}", name=f"og{p}") for p in range(2)]
            vtmp = big.tile([128, D], F32, tag="vtmp")
            dmy = big.tile([1, 128], F32, tag="dmy")
            dmyc = [0]

            pend = []

            def guard(inst):
                if pend:
                    s = InstructionNameOrderedSet()
                    for n in pend:
                        s.add(n)
                    inst.ins.add_nosync_dependencies_from(s)
                    pend.clear()
                return inst

            def absorb(*aps):
                for ap in aps:
                    i = dmyc[0] % 128
                    dmyc[0] += 1
                    ii = nc.vector.tensor_copy(dmy[0:1, i : i + 1], ap[0:1, 0:1])
                    pend.append(ii.ins.name)

            dmyA = big.tile([1, 128], F32, tag="dmyA")
            dmyAc = [0]

            def absorb_act(ap):
                i = dmyAc[0] % 128
                dmyAc[0] += 1
                ii = nc.scalar.copy(dmyA[0:1, i : i + 1], ap[0:1, 0:1])
                pend.append(ii.ins.name)

            bcb = big.tile([32, 1024], BF16, tag="bcb")
            bcbc = [0]
            crumb_st = {"last": None}

            def crumb(src_ap):
                crumb_st["last"] = src_ap[0:1, 0:1]

            def pe_absorb(ap=None):
                ap = ap if ap is not None else crumb_st["last"]
                if ap is None:
                    return
                if ap.partition_size() >= 32 and ap.dtype == BF16:
                    ii = nc.tensor.ldweights(ap[0:32, 0:1])
                else:
                    i = bcbc[0] % 1024
                    bcbc[0] += 1
                    nc.vector.tensor_copy(bcb[0:1, i : i + 1], ap[0:1, 0:1])
                    ii = nc.tensor.ldweights(bcb[0:32, i : i + 1])
                pend.append(ii.ins.name)

            # ---- loads (from gathered internal DRAM + inline consts) ----
            # tokens/weights: int8 staged resident, converted to bf16 on DVE
            tokv = tok.rearrange("p (k qq s) -> p k qq s", k=NKC, qq=4, s=512)
            tkv8 = tokq8.rearrange("p (k qq s) -> p k qq s", k=NKC, qq=4, s=512)
            for q in range(4):
                nc.gpsimd.dma_start(
                    out=tkv8[:, :, q, :],
                    in_=tok_g[q * TOK_N : (q + 1) * TOK_N].rearrange(
                        "(k p s) -> p k s", k=NKC, p=128, s=512
                    ),
                )
            for q in range(4):
                absorb(tokq8[:, q * 512 : q * 512 + 1])
                guard(nc.vector.tensor_copy(tokv[:, :, q, :], tkv8[:, :, q, :]))
            for kc in range(NKC):
                nc.gpsimd.dma_start(
                    out=wslq8[:, kc * WUP : (kc + 1) * WUP],
                    in_=wall_g[kc * 128 * WUP : (kc + 1) * 128 * WUP].rearrange(
                        "(p c) -> p c", p=128, c=WUP
                    ),
                )
            for kc in range(NKC):
                absorb(wslq8[:, kc * WUP : kc * WUP + 1])
                guard(nc.vector.tensor_copy(
                    wsl[:, kc * WSLW : kc * WSLW + 512],
                    wslq8[:, kc * WUP : kc * WUP + 512],
                ))
                guard(nc.vector.tensor_copy(
                    wsl[:, kc * WSLW + 1024 : kc * WSLW + 1280],
                    wslq8[:, kc * WUP + 512 : kc * WUP + 768],
                ))
            for half in range(2):
                nc.gpsimd.dma_start(
                    out=wo_sb[:, half * DIM : half * DIM + DIM],
                    in_=wo_g[half * WO_N : (half + 1) * WO_N].rearrange(
                        "(p d) -> p d", p=128, d=DIM
                    ),
                )
            nc.gpsimd.dma_start(out=cosm[:], in_=cos_d[:])
            nc.gpsimd.dma_start(out=sinm[:], in_=sin_d[:])
            nc.gpsimd.dma_start(out=winsb[:], in_=win_d[:])
            nc.gpsimd.dma_start(out=iotasb[:], in_=iota_d[:])
            off = WO_N + VR_N
            nc.gpsimd.dma_start(
                out=ubq_t[:],
                in_=blob16_d[off : off + UB_N].rearrange("(p n) -> p n", p=128, n=NSB),
            )
            off += UB_N
            nc.gpsimd.dma_start(
                out=ubr_t[:],
                in_=blob16_d[off : off + UB_N].rearrange("(p n) -> p n", p=128, n=NSB),
            )
            off += UB_N
            nc.gpsimd.dma_start(
                out=gate1[:],
                in_=blob16_d[off : off + GR_N].rearrange("(o x) -> o x", o=1, x=GR_N),
            )
            off += GR_N
            nc.gpsimd.dma_start(
                out=mixr[:],
                in_=blob16_d[off : off + MR_N].rearrange("(p n) -> p n", p=128, n=64),
            )
            # vr: one strided DMA per head
            vr0 = WO_N
            for h in range(HPC):
                nc.gpsimd.dma_start(
                    out=vrs[h].rearrange("p (sb d) -> p sb d", sb=NSB, d=D),
                    in_=blob16_d[vr0 + h * NSB * 128 * D : vr0 + (h + 1) * NSB * 128 * D]
                    .rearrange("(sb p d) -> p sb d", sb=NSB, p=128, d=D),
                )

            off += MR_N
            nc.gpsimd.dma_start(
                out=cvec[:],
                in_=blob16_d[off : off + CV_N].rearrange("(p n) -> p n", p=128, n=4),
            )
            nc.vector.memset(ones1[:], 1.0)
            absorb(cosm, sinm, gate1, cvec)
            guard(nc.vector.tensor_add(cexp[:], cvec[:, 0:1], cvec[:, 1:2]))
            guard(nc.vector.tensor_add(csv[:], cvec[:, 2:3], cvec[:, 3:4]))
            absorb_act(mixr[0:1, 0:1])
            absorb_act(cexp[0:1, 0:1])
            absorb_act(csv[0:1, 0:1])

            # ---- ub reconstruct + masks on device: msl = win * (iota <= ub) ----
            absorb(ubq_t, ubr_t)
            guard(nc.vector.scalar_tensor_tensor(
                ubl[:], ubq_t[:], 256.0, ubr_t[:],
                mybir.AluOpType.mult, mybir.AluOpType.add,
            ))
            absorb(winsb, iotasb)

            # ---- rot weights on device: wqrot/wkrot = 32-col half-swap of wq/wk ----
            for kc in range(NKC):
                base = kc * WSLW
                for h in range(HPC):
                    for dst0, src0 in ((0, 32), (32, 0)):
                        nc.vector.tensor_copy(
                            wsl[:, base + 512 + h * 64 + dst0 : base + 512 + h * 64 + dst0 + 32],
                            wsl[:, base + h * 64 + src0 : base + h * 64 + src0 + 32],
                        )
                        nc.vector.tensor_copy(
                            wsl[:, base + 768 + h * 64 + dst0 : base + 768 + h * 64 + dst0 + 32],
                            wsl[:, base + 256 + h * 64 + src0 : base + 256 + h * 64 + src0 + 32],
                        )

            def wchunk(kc, c0, c1):
                return wsl[:, kc * WSLW + c0 : kc * WSLW + c1]

            def tchunk(kc, s0, s1):
                return tok[:, kc * S + s0 : kc * S + s1]

            # ---- phase 1: T-orient projections: q, k (dual use) ----
            NS = 4  # seq chunks of 512
            for dest, c0 in (
                (qraw[0], 0), (qraw[1], 128), (kraw[0], 256), (kraw[1], 384),
                (qrot[0], 512), (qrot[1], 640), (krot[0], 768), (krot[1], 896),
            ):
                for ns in range(NS):
                    ps = pp.tile([128, 512], F32, tag="ps1", name="psA")
                    pe_absorb()
                    for kc in range(NKC):
                        guard(nc.tensor.matmul(
                            ps[:],
                            wchunk(kc, c0, c0 + 128),
                            tchunk(kc, ns * 512, ns * 512 + 512),
                            start=(kc == 0),
                            stop=(kc == NKC - 1),
                        ))
                    nc.vector.tensor_copy(dest[:, ns * 512 : ns * 512 + 512], ps[:])
                    crumb(dest[:, ns * 512 : ns * 512 + 512])

            # ---- phase 2: v (natural orient) + lerp with value residual ----
            for sb in range(NSB):
                ps = pp.tile([128, 256], F32, tag="ps1", name="psV")
                pe_absorb()
                for kc in range(NKC):
                    guard(nc.tensor.matmul(
                        ps[:],
                        tchunk(kc, sb * 128, sb * 128 + 128),
                        wchunk(kc, 1024, 1280),
                        start=(kc == 0),
                        stop=(kc == NKC - 1),
                    ))
                absorb_act(mixr[0:1, sb * 4 : sb * 4 + 1])
                guard(nc.scalar.activation(
                    mixs[:, sb * 4 : sb * 4 + 4], mixr[:, sb * 4 : sb * 4 + 4], AF.Sigmoid
                ))
                v_t = stg.tile([128, 256], F32, tag="vt", name="vt", bufs=2)
                absorb(ps[0:1, 0:1])
                guard(nc.vector.tensor_scalar(
                    v_t[:], ps[:], csv[:, 0:1], None, mybir.AluOpType.mult
                ))
                for h in range(HPC):
                    absorb(vrs[h][:, sb * D : sb * D + D])
                    guard(nc.vector.tensor_sub(
                        vtmp[:],
                        vrs[h][:, sb * D : sb * D + D],
                        v_t[:, h * D : h * D + D],
                    ))
                    absorb(mixs[:, sb * 4 + h : sb * 4 + h + 1])
                    # v' = mix*(vr - v) + v
                    guard(nc.vector.scalar_tensor_tensor(
                        vaug[h][:, sb * 65 : sb * 65 + 64],
                        vtmp[:],
                        mixs[:, sb * 4 + h : sb * 4 + h + 1],
                        v_t[:, h * D : h * D + D],
                        mybir.AluOpType.mult,
                        mybir.AluOpType.add,
                    ))
                    nc.vector.memset(vaug[h][:, sb * 65 + 64 : sb * 65 + 65], 1.0)
                crumb(vaug[HPC - 1][:, sb * 65 : sb * 65 + 64])

            # ---- phase 1b: RoPE via half-swap + cos/sin maps ----
            for raw, rot in (
                (qraw[0], qrot[0]),
                (qraw[1], qrot[1]),
                (kraw[0], krot[0]),
                (kraw[1], krot[1]),
            ):
                nc.vector.tensor_mul(rot[:], rot[:], sinm[:])
                nc.vector.tensor_mul(raw[:], raw[:], cosm[:])
                nc.vector.tensor_add(raw[:], raw[:], rot[:])
                crumb(raw[:])

            # ---- phase 3: attention ----
            ptw_hist, ring_hist, fbs_hist, og_hist = [], [], [], []
            for kb in range(NSB):
                Wn = min(STRIPW, S - kb * 128)
                mtile = stg.tile([128, STRIPW], BF16, tag="mt", name="mt", bufs=2)
                guard(nc.vector.scalar_tensor_tensor(
                    mtile[:],
                    iotasb[:],
                    ubl[:, kb : kb + 1],
                    winsb[:],
                    mybir.AluOpType.is_le,
                    mybir.AluOpType.mult,
                ))
                for h in range(HPC):
                    p, hh = divmod(h, 2)
                    b0 = hh * 64
                    ptv = pts[h][:, (kb % 5) * STRIPW : (kb % 5) * STRIPW + STRIPW]
                    sim = pp.tile([128, STRIPW], F32, tag="psS", name="psS")
                    pe_absorb(kro[p])
                    pe_absorb(qro[p])
                    if len(ptw_hist) >= 1:
                        pe_absorb(ptw_hist[-1])
                    for c0 in range(0, Wn, 512):
                        c1 = min(c0 + 512, Wn)
                        guard(nc.tensor.matmul(
                            sim[:, c0:c1],
                            kro[p][b0 : b0 + 64, kb * 128 : kb * 128 + 128],
                            qro[p][b0 : b0 + 64, kb * 128 + c0 : kb * 128 + c1],
                            start=True,
                            stop=True,
                        ))
                    ptw = stg.tile([128, STRIPW], BF16, tag="ptw", name="ptw", bufs=2)
                    if ring_hist:
                        absorb_act(ring_hist[-1][0:1, 0:1])
                    absorb_act(sim[0:1, 0:1])
                    guard(nc.scalar.activation(
                        ptw[:, 0:Wn], sim[:, 0:Wn], AF.Exp, scale=cexp[:, 0:1]
                    ))
                    ptw_hist.append(ptw)
                    absorb(ptw[0:1, 0:1], ptv[0:1, 0:1])
                    guard(nc.vector.tensor_mul(
                        ptv[:, 0:Wn],
                        ptw[:, 0:Wn],
                        mtile[:, 0:Wn],
                    ))
                    ring_hist.append(ptv)
                    # AV for q-block qb = kb
                    av = pp.tile([65, 128], F32, tag="psAV", name="psAV", bufs=1)
                    pe_absorb(ptv)
                    if og_hist:
                        pe_absorb(og_hist[-1])
                    if fbs_hist:
                        pe_absorb(fbs_hist[-1][0:1, 0:1])
                    srcs = list(range(max(0, kb - 4), kb + 1))
                    for j, sc in enumerate(srcs):
                        off2 = (kb - sc) * 128
                        psrc = pts[h][:, (sc % 5) * STRIPW + off2 : (sc % 5) * STRIPW + off2 + 128]
                        guard(nc.tensor.matmul(
                            av[:],
                            vaug[h][:, sc * 65 : sc * 65 + 65],
                            psrc,
                            start=(j == 0),
                            stop=(j == len(srcs) - 1),
                        ))
                    # normalize + gate
                    rec_sb = big.tile([1, 128], F32, tag="recsb", name="recsb")
                    f_row = big.tile([1, 128], BF16, tag="frow", name="frow")
                    gsl = gate1[0:1, h * S + kb * 128 : h * S + kb * 128 + 128]
                    nc.vector.reciprocal(rec_sb[:], av[64:65, :])
                    absorb(gsl)
                    guard(nc.vector.tensor_mul(f_row[:], rec_sb[:], gsl))
                    pe_absorb(f_row[0:1, 0:1])
                    if fbs_hist:
                        pe_absorb(fbs_hist[-1][0:1, 0:1])
                    fps = pp.tile([64, 128], F32, tag="fps", name="fps", bufs=1)
                    guard(nc.tensor.matmul(fps[:], ones1[:], f_row[:], start=True, stop=True))
                    fbs = stg.tile([64, 128], F32, tag="fbs", name="fbs", bufs=1)
                    nc.vector.tensor_copy(fbs[:], fps[:])
                    fbs_hist.append(fbs)
                    guard(nc.vector.tensor_mul(
                        outg[p][b0 : b0 + 64, kb * 128 : kb * 128 + 128],
                        av[0:64, :],
                        fbs[:],
                    ))
                    og_hist.append(outg[p][b0 : b0 + 1, kb * 128 : kb * 128 + 1])

            # ---- phase 4: Wo -> partial out (internal DRAM) ----
            ost_hist = []
            crumb(outg[0][:, S - 128 : S])
            crumb(outg[1][:, S - 128 : S])
            for g8 in range(4):
                slab = qkslab[:, g8 * 4 * DIM : (g8 + 1) * 4 * DIM]
                for j in range(4):
                    sb = g8 * 4 + j
                    for half in range(2):
                        ps = pp.tile([128, 512], F32, tag="ps1", name="psO")
                        pe_absorb()
                        if ost_hist:
                            pe_absorb(ost_hist[-1])
                        for kc in range(2):
                            guard(nc.tensor.matmul(
                                ps[:],
                                outg[kc][:, sb * 128 : sb * 128 + 128],
                                wo_sb[:, kc * DIM + half * 512 : kc * DIM + half * 512 + 512],
                                start=(kc == 0),
                                stop=(kc == 1),
                            ))
                        dst = slab[:, j * DIM + half * 512 : j * DIM + half * 512 + 512]
                        absorb(ps[0:1, 0:1])
                        guard(nc.vector.tensor_copy(dst, ps[:]))
                        ost_hist.append(dst)
                nc.sync.dma_start(
                    out=pout_d[g8 * 512 : g8 * 512 + 512, :].rearrange(
                        "(sb p) d -> p sb d", p=128
                    ),
                    in_=slab.rearrange("p (sb d) -> p sb d", d=DIM),
                )

    # ---- post-tile (drain guarantees all DMAs done): reduce partials ----
    nc.gpsimd.collective_compute(
        "ReduceScatter", mybir.AluOpType.add, replica_groups=G4,
        ins=[pout_d[:].opt()], outs=[rs_d[:].opt()],
    ).then_inc(s_rs, 1)
    nc.gpsimd.wait_ge(s_rs, 1)
    with tile.TileContext(nc) as tc2:
        with tc2.tile_pool(name="qz", bufs=1) as qz:
            rsb = qz.tile([128, 4 * DIM], BF16, tag="rsb")
            amx = qz.tile([128, 1], F32, tag="amx")
            rcpq = qz.tile([128, 1], F32, tag="rcpq")
            oi8 = qz.tile([128, 4 * DIM], I8, tag="oi8")
            nc.gpsimd.dma_start(
                out=rsb.rearrange("p (sb d) -> p sb d", d=DIM),
                in_=rs_d[:].rearrange("(sb p) d -> p sb d", p=128),
            )
            nc.vector.tensor_reduce(
                amx[:], rsb[:], mybir.AxisListType.XYZW, mybir.AluOpType.max,
                apply_absolute_value=True,
            )
            nc.vector.reciprocal(rcpq[:], amx[:])
            nc.vector.tensor_scalar(
                oi8[:], rsb[:], rcpq[:, 0:1], 127.0,
                mybir.AluOpType.mult, mybir.AluOpType.mult,
            )
            nc.sync.dma_start(
                out=out_d[:].rearrange("(sb p) d -> p sb d", p=128),
                in_=oi8.rearrange("p (sb d) -> p sb d", d=DIM),
            )
            nc.sync.dma_start(out=outsc_d[:], in_=amx[:])

    _nc_cache["nc"] = nc
    return nc


def _q8(x):
    s = float(np.abs(x).max()) / 127.0
    return np.clip(np.round(x / s), -127, 127).astype(np.int8), s


def _hilo(x):
    bf = ml_dtypes.bfloat16
    hi = np.float32(bf(x))
    lo = np.float32(bf(np.float32(x) - hi))
    return bf(hi), bf(lo)


def _prep_all(tokens, value_residual, episode_ids, Wq, Wkv, Wo, Wg, Wmix):
    bf = ml_dtypes.bfloat16
    perm = np.concatenate([np.arange(0, D, 2), np.arange(1, D, 2)])

    tokens = np.asarray(tokens, dtype=np.float32)
    st = float(np.abs(tokens).max()) / 127.0
    tokq = [
        np.ascontiguousarray(
            np.clip(np.round(tokens[b].T / st), -127, 127).astype(np.int8)
        )
        for b in range(B)
    ]                                                                         # [DIM,S] i8

    Wqp = np.asarray(Wq).reshape(DIM, H, D)[:, :, perm]
    Wkp = np.asarray(Wkv)[:, : H * D].reshape(DIM, H, D)[:, :, perm]
    Wvp = np.asarray(Wkv)[:, H * D :].reshape(DIM, H, D)
    Wop = np.asarray(Wo).reshape(H, D, DIM)
    packs = []
    for g in range(4):
        hs = slice(4 * g, 4 * g + 4)
        wq_q, swq = _q8(Wqp[:, hs].reshape(DIM, 256))
        wk_q, swk = _q8(Wkp[:, hs].reshape(DIM, 256))
        wv_q, swv = _q8(Wvp[:, hs].reshape(DIM, 256))
        wall = np.concatenate([wq_q, wk_q, wv_q], axis=1)                     # [DIM,768] i8
        wall = np.ascontiguousarray(wall.reshape(NKC, 128, WUP))
        wo = np.ascontiguousarray(
            Wop[hs].reshape(256, DIM).astype(bf).reshape(2, 128, DIM)
        )
        cexp = st * st * swq * swk * (D ** -0.5)
        csv = st * swv
        eh, el = _hilo(cexp)
        vh, vl = _hilo(csv)
        cv = np.empty((128, 4), dtype=bf)
        cv[:, 0], cv[:, 1], cv[:, 2], cv[:, 3] = eh, el, vh, vl
        packs.append((wall, wo, cv))

    # gate/mix projections on host (tiny, keeps sigmoid paths off fp8)
    graw_all = np.einsum("bnd,dh->bhn", tokens, np.asarray(Wg, dtype=np.float32))   # [B,H,S]
    gate1_all = 1.0 / (1.0 + np.exp(-graw_all))
    mraw_all = np.einsum("bnd,dh->bnh", tokens, np.asarray(Wmix, dtype=np.float32)) # [B,S,H]

    ubqs, ubrs = [], []
    kk = np.arange(S)
    for b in range(B):
        ids = np.asarray(episode_ids[b])
        ee = np.searchsorted(ids, ids, side="right") - 1                      # [S]
        ubl = ee - (kk // 128) * 128                                          # int
        q = ubl // 256
        r = ubl - 256 * q
        ubqs.append(np.ascontiguousarray(q.reshape(NSB, 128).T.astype(bf)))   # [128,NSB]
        ubrs.append(np.ascontiguousarray(r.reshape(NSB, 128).T.astype(bf)))

    vr16 = np.asarray(value_residual).astype(bf)

    in_maps = []
    for c in range(NCORES):
        b, g = divmod(c, 4)
        wall, wo, cv = packs[g]
        hs = slice(4 * g, 4 * g + 4)

        g1 = gate1_all[b, hs].astype(bf)                                      # [4,S]
        mraw = np.ascontiguousarray(
            mraw_all[b, :, hs].reshape(NSB, 128, 4).transpose(1, 0, 2).reshape(128, 64)
        ).astype(bf)                                                          # [128,64]
        blob8 = np.concatenate([
            tokq[b][:, g * 512 : (g + 1) * 512].ravel(),
            wall[4 * b : 4 * b + 4].ravel(),
        ])
        blob16 = np.concatenate([
            wo[b].ravel(),
            vr16[b, hs].ravel(),
            ubqs[b].ravel(), ubrs[b].ravel(),
            g1.ravel(), mraw.ravel(), cv.ravel(),
        ])
        in_maps.append({"blob8": blob8, "blob16": blob16})
    return in_maps


# ---- cached PJRT dispatch: jit built once, zero output-buffers device-resident ----
_runner_cache = {}


def _get_runner(nc):
    if "fn" in _runner_cache:
        return _runner_cache
    from jax.experimental.shard_map import shard_map

    bass2jax.install_neuronx_cc_hook()
    partition_name = nc.partition_id_tensor.name if nc.partition_id_tensor else None
    in_names, out_names, out_avals = [], [], []
    for alloc in nc.m.functions[0].allocations:
        if not isinstance(alloc, mybir.MemoryLocationSet):
            continue
        name = alloc.memorylocations[0].name
        if alloc.kind == "ExternalInput":
            if name != partition_name:
                in_names.append(name)
        elif alloc.kind == "ExternalOutput":
            out_avals.append(
                jax.core.ShapedArray(tuple(alloc.tensor_shape), mybir.dt.np(alloc.dtype))
            )
            out_names.append(name)
    in_names_all = list(in_names) + list(out_names)
    if partition_name is not None:
        in_names_all.append(partition_name)

    def _body(*args):
        operands = list(args)
        if partition_name is not None:
            operands.append(bass2jax.partition_id_tensor())
        outs = bass2jax._bass_exec_p.bind(
            *operands,
            out_avals=tuple(out_avals),
            in_names=tuple(in_names_all),
            out_names=tuple(out_names),
            lowering_input_output_aliases=(),
            sim_require_finite=True,
            sim_require_nnan=True,
            nc=nc,
        )
        return tuple(outs)

    devices = jax.devices()[:NCORES]
    mesh = jax.sharding.Mesh(np.asarray(devices), ("core",))
    P = jax.sharding.PartitionSpec
    n_in = len(in_names) + len(out_names)
    fn = jax.jit(
        shard_map(
            _body, mesh=mesh, in_specs=(P("core"),) * n_in,
            out_specs=(P("core"),) * len(out_names), check_rep=False,
        ),
        keep_unused=True,
    )
    sh = jax.sharding.NamedSharding(mesh, P("core"))
    zeros_dev = [
        jax.device_put(np.zeros((NCORES * a.shape[0], *a.shape[1:]), a.dtype), sh)
        for a in out_avals
    ]
    jax.block_until_ready(zeros_dev)
    _runner_cache.update(fn=fn, in_names=in_names, zeros_dev=zeros_dev)
    return _runner_cache


def _execute(nc, in_maps):
    r = _get_runner(nc)
    concat_in = [
        np.concatenate([np.asarray(m[name]) for m in in_maps], axis=0)
        for name in r["in_names"]
    ]
    outs = r["fn"](*concat_in, *r["zeros_dev"])
    outs[0].copy_to_host_async()
    outs[1].copy_to_host_async()
    oi = np.asarray(outs[0]).reshape(NCORES, 512, DIM)
    sc = np.asarray(outs[1]).reshape(NCORES, 128, 1) / 127.0
    res = np.zeros((B, S, DIM), dtype=np.float32)
    for c in range(NCORES):
        b, rk = divmod(c, 4)
        scl = np.tile(sc[c], (4, 1)).reshape(512, 1)
        res[b, rk * 512 : (rk + 1) * 512] = oi[c].astype(np.float32) * scl
    return res


def kernel(tokens, value_residual, episode_ids, Wq, Wkv, Wo, Wg, Wmix):
    nc = build_nc()
    in_maps = _prep_all(tokens, value_residual, episode_ids, Wq, Wkv, Wo, Wg, Wmix)
    return _execute(nc, in_maps)


# revision 15
# speedup vs baseline: 7.3902x; 1.2285x over previous
import numpy as np
import ml_dtypes

import jax
import concourse.bass as bass
from bass_rust import InstructionNameOrderedSet
import concourse.mybir as mybir
from concourse import tile
from concourse import bass2jax

BF16 = mybir.dt.bfloat16
F32 = mybir.dt.float32
I8 = mybir.dt.int8
AF = mybir.ActivationFunctionType

B, S, DIM, H, D = 2, 2048, 1024, 16, 64
WIN = 512
HPC = 4          # heads per core
NCORES = 8
NSB = S // 128   # 16 seq blocks
NKC = DIM // 128  # 8 contraction chunks
STRIPW = 640     # 128 keys attend to <=640 queries (dist 0..512 + 127)
WUP = 768        # uploaded weight cols per kc chunk: wq 256 | wk 256 | wv 256 (int8)
WSLW = 1280      # sbuf slab cols per kc chunk: wq | wk | wqrot | wkrot | wv

# int8 blob layout (elements): tok quarter | wall half | vr  (scales in blob16)
TOK_N = DIM * 512                 # 524288
WALL_N = 4 * 128 * WUP            # 393216
VR_N = HPC * S * D                # 524288
BLOB8_N = TOK_N + WALL_N + VR_N   # 1441792
# bf16 blob layout (elements): wo | ubq | ubr | gate1 | mraw | cvec [128,12]
WO_N = 128 * DIM                  # 131072
UB_N = 128 * NSB                  # 2048
GR_N = 4 * S                     # 8192
MR_N = 128 * 64                   # 8192
CV_N = 128 * 12                   # 1536
BLOB16_N = WO_N + 2 * UB_N + GR_N + MR_N + CV_N

_nc_cache = {}


def _patched_drain(self, tick_clock, wait_clock):
    # Tail drain: walrus limits sync waits per instruction, so convert the
    # multi-wait drain into a chain of single-wait sem waits on SyncE.
    from concourse.vector_clock import ScopedClock

    nc = self.nc
    probe = mybir.InstNoOp(name="__drain_probe", engine=mybir.EngineType.SP, ins=[], outs=[])
    wait_clock.add_sem_waits(probe, ScopedClock({None: tick_clock.global_clock}))
    id2h = {h.num: h for h in self.sems.allocated().values()}
    si = getattr(probe, "sync_info", None)
    if si is not None:
        for w in si.on_wait:
            h = id2h.get(w.id)
            if h is not None:
                nc.sync.wait_ge(h, w.wait_value)
    nc.sync.drain()
    nc.all_engine_barrier()
    popped = nc._tile_sem_poison_stack.pop()
    assert popped is self._sem_poison
    nc.clear_and_free_semaphores(list(self.sems.allocated().values()))
    nc.all_engine_barrier()


tile.TileContext._drain_and_barrier = _patched_drain


def _consts():
    bf = ml_dtypes.bfloat16
    pos = np.arange(S, dtype=np.float64)
    invf = 1.0 / (10000.0 ** (np.arange(0, D, 2, dtype=np.float64) / D))   # [32]
    ang = pos[None, :] * invf[:, None]                                     # [32,S]
    c32, s32 = np.cos(ang), np.sin(ang)
    cosm = np.tile(c32, (4, 1)).astype(bf)                                 # [128,S]
    sgn = np.concatenate([-s32, s32], axis=0)                              # [64,S]
    sinm = np.tile(sgn, (2, 1)).astype(bf)
    p = np.arange(128)
    j = np.arange(STRIPW)
    win = ((j[None, :] >= p[:, None]) & (j[None, :] - p[:, None] <= WIN)).astype(np.float32)
    iotaw = np.ascontiguousarray(np.broadcast_to(j[None, :].astype(np.float32), (128, STRIPW)))
    return cosm, sinm, win, iotaw


def build_nc():
    if "nc" in _nc_cache:
        return _nc_cache["nc"]
    nc = bass.Bass(num_devices=NCORES)

    # ---- per-core external I/O (core c = 4*b + g: batch b, head-group g) ----
    # blob8: tok quarter [DIM,512] int8 | wall half [4,128,WUP] int8
    blob8_d = nc.dram_tensor("blob8", [BLOB8_N], I8, kind="ExternalInput")
    # blob16: wo half | vr | ubq | ubr | gate1 | mraw | cvec
    blob16_d = nc.dram_tensor("blob16", [BLOB16_N], BF16, kind="ExternalInput")
    out_d = nc.dram_tensor("out", [512, DIM], I8, kind="ExternalOutput")
    outsc_d = nc.dram_tensor("outsc", [128, 1], F32, kind="ExternalOutput")

    # ---- NEFF-embedded constants (shipped at model load, not per call) ----
    cosm_np, sinm_np, win_np, iotaw_np = _consts()
    cos_d = nc.inline_tensor(cosm_np, name="cosk")
    sin_d = nc.inline_tensor(sinm_np, name="sink")
    win_d = nc.inline_tensor(win_np, name="wink")
    iota_d = nc.inline_tensor(iotaw_np, name="iotak")

    # ---- internal DRAM for collectives ----
    tok_stage = nc.dram_tensor("tok_stage", [TOK_N], I8, kind="Internal")
    wall_stage = nc.dram_tensor("wall_stage", [WALL_N], I8, kind="Internal")
    wo_stage = nc.dram_tensor("wo_stage", [WO_N], BF16, kind="Internal")
    tok_g = nc.dram_tensor("tok_g", [4 * TOK_N], I8, kind="Internal")
    wall_g = nc.dram_tensor("wall_g", [2 * WALL_N], I8, kind="Internal")
    wo_g = nc.dram_tensor("wo_g", [2 * WO_N], BF16, kind="Internal")
    pout_d = nc.dram_tensor("pout", [S, DIM], BF16, kind="Internal")
    rs_d = nc.dram_tensor("rsout", [512, DIM], BF16, kind="Internal")

    s_stage = nc.alloc_semaphore("s_stage")
    s_ag = nc.alloc_semaphore("s_ag")
    s_rs = nc.alloc_semaphore("s_rs")
    s_fin = nc.alloc_semaphore("s_fin")

    G4 = [[0, 1, 2, 3], [4, 5, 6, 7]]        # batch groups (head-parallel)
    G2 = [[0, 4], [1, 5], [2, 6], [3, 7]]    # same head-group pairs across batches

    # ---- pre-tile: stage inputs to internal DRAM, gather across cores ----
    nc.gpsimd.dma_start(out=tok_stage[:], in_=blob8_d[0:TOK_N]).then_inc(s_stage, 16)
    nc.gpsimd.dma_start(
        out=wall_stage[:], in_=blob8_d[TOK_N : TOK_N + WALL_N]
    ).then_inc(s_stage, 16)
    nc.gpsimd.dma_start(out=wo_stage[:], in_=blob16_d[0:WO_N]).then_inc(s_stage, 16)
    nc.gpsimd.wait_ge(s_stage, 48)
    nc.gpsimd.collective_compute(
        "AllGather", mybir.AluOpType.bypass, replica_groups=G4,
        ins=[tok_stage[:].opt()], outs=[tok_g[:].opt()],
    ).then_inc(s_ag, 1)
    nc.gpsimd.collective_compute(
        "AllGather", mybir.AluOpType.bypass, replica_groups=G2,
        ins=[wall_stage[:].opt()], outs=[wall_g[:].opt()],
    ).then_inc(s_ag, 1)
    nc.gpsimd.collective_compute(
        "AllGather", mybir.AluOpType.bypass, replica_groups=G2,
        ins=[wo_stage[:].opt()], outs=[wo_g[:].opt()],
    ).then_inc(s_ag, 1)
    nc.gpsimd.wait_ge(s_ag, 3)

    with tile.TileContext(nc) as tc:
        with (
            tc.tile_pool(name="big", bufs=1) as big,
            tc.tile_pool(name="stg", bufs=2) as stg,
            tc.tile_pool(name="pp", bufs=2, space=bass.MemorySpace.PSUM) as pp,
        ):
            # ---- resident SBUF slabs ----
            tok = big.tile([128, NKC * S], BF16, tag="tok")          # 32KB/p
            wsl = big.tile([128, NKC * WSLW], BF16, tag="wsl")       # 20KB/p
            wo_sb = big.tile([128, 2 * DIM], BF16, tag="wo")         # 4KB/p
            cosm = big.tile([128, S], BF16, tag="cos")
            sinm = big.tile([128, S], BF16, tag="sin")
            tokq8 = big.tile([128, NKC * S], I8, tag="tokq8")        # 16KB/p
            wslq8 = big.tile([128, NKC * WUP], I8, tag="wslq8")      # 6KB/p
            winsb = big.tile([128, STRIPW], F32, tag="winsb")
            iotasb = big.tile([128, STRIPW], F32, tag="iotasb")
            ubq_t = big.tile([128, NSB], BF16, tag="ubq")
            ubr_t = big.tile([128, NSB], BF16, tag="ubr")
            ubl = big.tile([128, NSB], F32, tag="ubl")
            vrq8 = [big.tile([128, NSB * D], I8, tag=f"vq{h}", name=f"vq{h}") for h in range(HPC)]
            vrs = [big.tile([128, NSB * D], BF16, tag=f"vr{h}", name=f"vr{h}") for h in range(HPC)]
            vaug = [big.tile([128, NSB * 65], BF16, tag=f"va{h}", name=f"va{h}") for h in range(HPC)]
            mixs = big.tile([128, 64], F32, tag="mix")               # sigmoid(mix)
            mixr = big.tile([128, 64], BF16, tag="mixr")             # uploaded raw mix
            gate1 = big.tile([1, HPC * S], BF16, tag="gate1")        # uploaded sigmoid(gate)
            cvec = big.tile([128, 12], BF16, tag="cvec")             # scale consts hi/lo
            cexp = big.tile([128, 1], F32, tag="cexp")               # exp logit scale
            csv = big.tile([128, 1], F32, tag="csv")                 # v dequant scale
            vsc = big.tile([128, 4], F32, tag="vsc")                 # vr dequant scales
            ones1 = big.tile([1, 64], BF16, tag="ones1")
            qkslab = big.tile([128, 8 * S], BF16, tag="qkslab")
            qraw = [qkslab[:, 0 * S : 1 * S], qkslab[:, 1 * S : 2 * S]]
            kraw = [qkslab[:, 2 * S : 3 * S], qkslab[:, 3 * S : 4 * S]]
            qrot = [qkslab[:, 4 * S : 5 * S], qkslab[:, 5 * S : 6 * S]]
            krot = [qkslab[:, 6 * S : 7 * S], qkslab[:, 7 * S : 8 * S]]
            qro, kro = qraw, kraw  # roped in place
            # PT ring: 5 live strips per head
            pts = [big.tile([128, 5 * STRIPW], BF16, tag=f"pt{h}", name=f"pt{h}") for h in range(HPC)]
            outg = [big.tile([128, S], BF16, tag=f"og{p}", name=f"og{p}") for p in range(2)]
            vtmp = big.tile([128, D], F32, tag="vtmp")
            dmy = big.tile([1, 128], F32, tag="dmy")
            dmyc = [0]

            pend = []

            def guard(inst):
                if pend:
                    s = InstructionNameOrderedSet()
                    for n in pend:
                        s.add(n)
                    inst.ins.add_nosync_dependencies_from(s)
                    pend.clear()
                return inst

            def absorb(*aps):
                for ap in aps:
                    i = dmyc[0] % 128
                    dmyc[0] += 1
                    ii = nc.vector.tensor_copy(dmy[0:1, i : i + 1], ap[0:1, 0:1])
                    pend.append(ii.ins.name)

            dmyA = big.tile([1, 128], F32, tag="dmyA")
            dmyAc = [0]

            def absorb_act(ap):
                i = dmyAc[0] % 128
                dmyAc[0] += 1
                ii = nc.scalar.copy(dmyA[0:1, i : i + 1], ap[0:1, 0:1])
                pend.append(ii.ins.name)

            bcb = big.tile([32, 1024], BF16, tag="bcb")
            bcbc = [0]
            crumb_st = {"last": None}

            def crumb(src_ap):
                crumb_st["last"] = src_ap[0:1, 0:1]

            def pe_absorb(ap=None):
                ap = ap if ap is not None else crumb_st["last"]
                if ap is None:
                    return
                if ap.partition_size() >= 32 and ap.dtype == BF16:
                    ii = nc.tensor.ldweights(ap[0:32, 0:1])
                else:
                    i = bcbc[0] % 1024
                    bcbc[0] += 1
                    nc.vector.tensor_copy(bcb[0:1, i : i + 1], ap[0:1, 0:1])
                    ii = nc.tensor.ldweights(bcb[0:32, i : i + 1])
                pend.append(ii.ins.name)

            # ---- loads (from gathered internal DRAM + inline consts) ----
            # tokens/weights: int8 staged resident, converted to bf16 on DVE
            tokv = tok.rearrange("p (k qq s) -> p k qq s", k=NKC, qq=4, s=512)
            tkv8 = tokq8.rearrange("p (k qq s) -> p k qq s", k=NKC, qq=4, s=512)
            for q in range(4):
                nc.gpsimd.dma_start(
                    out=tkv8[:, :, q, :],
                    in_=tok_g[q * TOK_N : (q + 1) * TOK_N].rearrange(
                        "(k p s) -> p k s", k=NKC, p=128, s=512
                    ),
                )
            for q in range(4):
                absorb(tokq8[:, q * 512 : q * 512 + 1])
                guard(nc.vector.tensor_copy(tokv[:, :, q, :], tkv8[:, :, q, :]))
            for kc in range(NKC):
                nc.gpsimd.dma_start(
                    out=wslq8[:, kc * WUP : (kc + 1) * WUP],
                    in_=wall_g[kc * 128 * WUP : (kc + 1) * 128 * WUP].rearrange(
                        "(p c) -> p c", p=128, c=WUP
                    ),
                )
            for kc in range(NKC):
                absorb(wslq8[:, kc * WUP : kc * WUP + 1])
                guard(nc.vector.tensor_copy(
                    wsl[:, kc * WSLW : kc * WSLW + 512],
                    wslq8[:, kc * WUP : kc * WUP + 512],
                ))
                guard(nc.vector.tensor_copy(
                    wsl[:, kc * WSLW + 1024 : kc * WSLW + 1280],
                    wslq8[:, kc * WUP + 512 : kc * WUP + 768],
                ))
            for half in range(2):
                nc.gpsimd.dma_start(
                    out=wo_sb[:, half * DIM : half * DIM + DIM],
                    in_=wo_g[half * WO_N : (half + 1) * WO_N].rearrange(
                        "(p d) -> p d", p=128, d=DIM
                    ),
                )
            nc.gpsimd.dma_start(out=cosm[:], in_=cos_d[:])
            nc.gpsimd.dma_start(out=sinm[:], in_=sin_d[:])
            nc.gpsimd.dma_start(out=winsb[:], in_=win_d[:])
            nc.gpsimd.dma_start(out=iotasb[:], in_=iota_d[:])
            off = WO_N
            nc.gpsimd.dma_start(
                out=ubq_t[:],
                in_=blob16_d[off : off + UB_N].rearrange("(p n) -> p n", p=128, n=NSB),
            )
            off += UB_N
            nc.gpsimd.dma_start(
                out=ubr_t[:],
                in_=blob16_d[off : off + UB_N].rearrange("(p n) -> p n", p=128, n=NSB),
            )
            off += UB_N
            nc.gpsimd.dma_start(
                out=gate1[:],
                in_=blob16_d[off : off + GR_N].rearrange("(o x) -> o x", o=1, x=GR_N),
            )
            off += GR_N
            nc.gpsimd.dma_start(
                out=mixr[:],
                in_=blob16_d[off : off + MR_N].rearrange("(p n) -> p n", p=128, n=64),
            )
            # vr: one strided int8 DMA per head, dequant on DVE
            vr0 = TOK_N + WALL_N
            for h in range(HPC):
                nc.gpsimd.dma_start(
                    out=vrq8[h].rearrange("p (sb d) -> p sb d", sb=NSB, d=D),
                    in_=blob8_d[vr0 + h * NSB * 128 * D : vr0 + (h + 1) * NSB * 128 * D]
                    .rearrange("(sb p d) -> p sb d", sb=NSB, p=128, d=D),
                )

            off += MR_N
            nc.gpsimd.dma_start(
                out=cvec[:],
                in_=blob16_d[off : off + CV_N].rearrange("(p n) -> p n", p=128, n=12),
            )
            nc.vector.memset(ones1[:], 1.0)
            absorb(cosm, sinm, gate1, cvec)
            guard(nc.vector.tensor_add(cexp[:], cvec[:, 0:1], cvec[:, 1:2]))
            guard(nc.vector.tensor_add(csv[:], cvec[:, 2:3], cvec[:, 3:4]))
            for h in range(HPC):
                guard(nc.vector.tensor_add(
                    vsc[:, h : h + 1], cvec[:, 4 + 2 * h : 5 + 2 * h],
                    cvec[:, 5 + 2 * h : 6 + 2 * h],
                ))
            for h in range(HPC):
                absorb(vrq8[h][:, 0:1])
                guard(nc.vector.tensor_scalar(
                    vrs[h][:], vrq8[h][:], vsc[:, h : h + 1], None, mybir.AluOpType.mult
                ))
            absorb_act(mixr[0:1, 0:1])
            absorb_act(cexp[0:1, 0:1])
            absorb_act(csv[0:1, 0:1])

            # ---- ub reconstruct + masks on device: msl = win * (iota <= ub) ----
            absorb(ubq_t, ubr_t)
            guard(nc.vector.scalar_tensor_tensor(
                ubl[:], ubq_t[:], 256.0, ubr_t[:],
                mybir.AluOpType.mult, mybir.AluOpType.add,
            ))
            absorb(winsb, iotasb)

            # ---- rot weights on device: wqrot/wkrot = 32-col half-swap of wq/wk ----
            for kc in range(NKC):
                base = kc * WSLW
                for h in range(HPC):
                    for dst0, src0 in ((0, 32), (32, 0)):
                        nc.vector.tensor_copy(
                            wsl[:, base + 512 + h * 64 + dst0 : base + 512 + h * 64 + dst0 + 32],
                            wsl[:, base + h * 64 + src0 : base + h * 64 + src0 + 32],
                        )
                        nc.vector.tensor_copy(
                            wsl[:, base + 768 + h * 64 + dst0 : base + 768 + h * 64 + dst0 + 32],
                            wsl[:, base + 256 + h * 64 + src0 : base + 256 + h * 64 + src0 + 32],
                        )

            def wchunk(kc, c0, c1):
                return wsl[:, kc * WSLW + c0 : kc * WSLW + c1]

            def tchunk(kc, s0, s1):
                return tok[:, kc * S + s0 : kc * S + s1]

            # ---- phase 1: T-orient projections: q, k (dual use) ----
            NS = 4  # seq chunks of 512
            for dest, c0 in (
                (qraw[0], 0), (qraw[1], 128), (kraw[0], 256), (kraw[1], 384),
                (qrot[0], 512), (qrot[1], 640), (krot[0], 768), (krot[1], 896),
            ):
                for ns in range(NS):
                    ps = pp.tile([128, 512], F32, tag="ps1", name="psA")
                    pe_absorb()
                    for kc in range(NKC):
                        guard(nc.tensor.matmul(
                            ps[:],
                            wchunk(kc, c0, c0 + 128),
                            tchunk(kc, ns * 512, ns * 512 + 512),
                            start=(kc == 0),
                            stop=(kc == NKC - 1),
                        ))
                    nc.vector.tensor_copy(dest[:, ns * 512 : ns * 512 + 512], ps[:])
                    crumb(dest[:, ns * 512 : ns * 512 + 512])

            # ---- phase 2: v (natural orient) + lerp with value residual ----
            for sb in range(NSB):
                ps = pp.tile([128, 256], F32, tag="ps1", name="psV")
                pe_absorb()
                for kc in range(NKC):
                    guard(nc.tensor.matmul(
                        ps[:],
                        tchunk(kc, sb * 128, sb * 128 + 128),
                        wchunk(kc, 1024, 1280),
                        start=(kc == 0),
                        stop=(kc == NKC - 1),
                    ))
                absorb_act(mixr[0:1, sb * 4 : sb * 4 + 1])
                guard(nc.scalar.activation(
                    mixs[:, sb * 4 : sb * 4 + 4], mixr[:, sb * 4 : sb * 4 + 4], AF.Sigmoid
                ))
                v_t = stg.tile([128, 256], F32, tag="vt", name="vt", bufs=2)
                absorb(ps[0:1, 0:1])
                guard(nc.vector.tensor_scalar(
                    v_t[:], ps[:], csv[:, 0:1], None, mybir.AluOpType.mult
                ))
                for h in range(HPC):
                    absorb(vrs[h][:, sb * D : sb * D + D])
                    guard(nc.vector.tensor_sub(
                        vtmp[:],
                        vrs[h][:, sb * D : sb * D + D],
                        v_t[:, h * D : h * D + D],
                    ))
                    absorb(mixs[:, sb * 4 + h : sb * 4 + h + 1])
                    # v' = mix*(vr - v) + v
                    guard(nc.vector.scalar_tensor_tensor(
                        vaug[h][:, sb * 65 : sb * 65 + 64],
                        vtmp[:],
                        mixs[:, sb * 4 + h : sb * 4 + h + 1],
                        v_t[:, h * D : h * D + D],
                        mybir.AluOpType.mult,
                        mybir.AluOpType.add,
                    ))
                    nc.vector.memset(vaug[h][:, sb * 65 + 64 : sb * 65 + 65], 1.0)
                crumb(vaug[HPC - 1][:, sb * 65 : sb * 65 + 64])

            # ---- phase 1b: RoPE via half-swap + cos/sin maps ----
            for raw, rot in (
                (qraw[0], qrot[0]),
                (qraw[1], qrot[1]),
                (kraw[0], krot[0]),
                (kraw[1], krot[1]),
            ):
                nc.vector.tensor_mul(rot[:], rot[:], sinm[:])
                nc.vector.tensor_mul(raw[:], raw[:], cosm[:])
                nc.vector.tensor_add(raw[:], raw[:], rot[:])
                crumb(raw[:])

            # ---- phase 3: attention ----
            ptw_hist, ring_hist, fbs_hist, og_hist = [], [], [], []
            for kb in range(NSB):
                Wn = min(STRIPW, S - kb * 128)
                mtile = stg.tile([128, STRIPW], BF16, tag="mt", name="mt", bufs=2)
                guard(nc.vector.scalar_tensor_tensor(
                    mtile[:],
                    iotasb[:],
                    ubl[:, kb : kb + 1],
                    winsb[:],
                    mybir.AluOpType.is_le,
                    mybir.AluOpType.mult,
                ))
                for h in range(HPC):
                    p, hh = divmod(h, 2)
                    b0 = hh * 64
                    ptv = pts[h][:, (kb % 5) * STRIPW : (kb % 5) * STRIPW + STRIPW]
                    sim = pp.tile([128, STRIPW], F32, tag="psS", name="psS")
                    pe_absorb(kro[p])
                    pe_absorb(qro[p])
                    if len(ptw_hist) >= 1:
                        pe_absorb(ptw_hist[-1])
                    for c0 in range(0, Wn, 512):
                        c1 = min(c0 + 512, Wn)
                        guard(nc.tensor.matmul(
                            sim[:, c0:c1],
                            kro[p][b0 : b0 + 64, kb * 128 : kb * 128 + 128],
                            qro[p][b0 : b0 + 64, kb * 128 + c0 : kb * 128 + c1],
                            start=True,
                            stop=True,
                        ))
                    ptw = stg.tile([128, STRIPW], BF16, tag="ptw", name="ptw", bufs=2)
                    if ring_hist:
                        absorb_act(ring_hist[-1][0:1, 0:1])
                    absorb_act(sim[0:1, 0:1])
                    guard(nc.scalar.activation(
                        ptw[:, 0:Wn], sim[:, 0:Wn], AF.Exp, scale=cexp[:, 0:1]
                    ))
                    ptw_hist.append(ptw)
                    absorb(ptw[0:1, 0:1], ptv[0:1, 0:1])
                    guard(nc.vector.tensor_mul(
                        ptv[:, 0:Wn],
                        ptw[:, 0:Wn],
                        mtile[:, 0:Wn],
                    ))
                    ring_hist.append(ptv)
                    # AV for q-block qb = kb
                    av = pp.tile([65, 128], F32, tag="psAV", name="psAV", bufs=1)
                    pe_absorb(ptv)
                    if og_hist:
                        pe_absorb(og_hist[-1])
                    if fbs_hist:
                        pe_absorb(fbs_hist[-1][0:1, 0:1])
                    srcs = list(range(max(0, kb - 4), kb + 1))
                    for j, sc in enumerate(srcs):
                        off2 = (kb - sc) * 128
                        psrc = pts[h][:, (sc % 5) * STRIPW + off2 : (sc % 5) * STRIPW + off2 + 128]
                        guard(nc.tensor.matmul(
                            av[:],
                            vaug[h][:, sc * 65 : sc * 65 + 65],
                            psrc,
                            start=(j == 0),
                            stop=(j == len(srcs) - 1),
                        ))
                    # normalize + gate
                    rec_sb = big.tile([1, 128], F32, tag="recsb", name="recsb")
                    f_row = big.tile([1, 128], BF16, tag="frow", name="frow")
                    gsl = gate1[0:1, h * S + kb * 128 : h * S + kb * 128 + 128]
                    nc.vector.reciprocal(rec_sb[:], av[64:65, :])
                    absorb(gsl)
                    guard(nc.vector.tensor_mul(f_row[:], rec_sb[:], gsl))
                    pe_absorb(f_row[0:1, 0:1])
                    if fbs_hist:
                        pe_absorb(fbs_hist[-1][0:1, 0:1])
                    fps = pp.tile([64, 128], F32, tag="fps", name="fps", bufs=1)
                    guard(nc.tensor.matmul(fps[:], ones1[:], f_row[:], start=True, stop=True))
                    fbs = stg.tile([64, 128], F32, tag="fbs", name="fbs", bufs=1)
                    nc.vector.tensor_copy(fbs[:], fps[:])
                    fbs_hist.append(fbs)
                    guard(nc.vector.tensor_mul(
                        outg[p][b0 : b0 + 64, kb * 128 : kb * 128 + 128],
                        av[0:64, :],
                        fbs[:],
                    ))
                    og_hist.append(outg[p][b0 : b0 + 1, kb * 128 : kb * 128 + 1])

            # ---- phase 4: Wo -> partial out (internal DRAM) ----
            ost_hist = []
            crumb(outg[0][:, S - 128 : S])
            crumb(outg[1][:, S - 128 : S])
            for g8 in range(4):
                slab = qkslab[:, g8 * 4 * DIM : (g8 + 1) * 4 * DIM]
                for j in range(4):
                    sb = g8 * 4 + j
                    for half in range(2):
                        ps = pp.tile([128, 512], F32, tag="ps1", name="psO")
                        pe_absorb()
                        if ost_hist:
                            pe_absorb(ost_hist[-1])
                        for kc in range(2):
                            guard(nc.tensor.matmul(
                                ps[:],
                                outg[kc][:, sb * 128 : sb * 128 + 128],
                                wo_sb[:, kc * DIM + half * 512 : kc * DIM + half * 512 + 512],
                                start=(kc == 0),
                                stop=(kc == 1),
                            ))
                        dst = slab[:, j * DIM + half * 512 : j * DIM + half * 512 + 512]
                        absorb(ps[0:1, 0:1])
                        guard(nc.vector.tensor_copy(dst, ps[:]))
                        ost_hist.append(dst)
                nc.sync.dma_start(
                    out=pout_d[g8 * 512 : g8 * 512 + 512, :].rearrange(
                        "(sb p) d -> p sb d", p=128
                    ),
                    in_=slab.rearrange("p (sb d) -> p sb d", d=DIM),
                )

    # ---- post-tile (drain guarantees all DMAs done): reduce partials ----
    nc.gpsimd.collective_compute(
        "ReduceScatter", mybir.AluOpType.add, replica_groups=G4,
        ins=[pout_d[:].opt()], outs=[rs_d[:].opt()],
    ).then_inc(s_rs, 1)
    nc.gpsimd.wait_ge(s_rs, 1)
    with tile.TileContext(nc) as tc2:
        with tc2.tile_pool(name="qz", bufs=1) as qz:
            rsb = qz.tile([128, 4 * DIM], BF16, tag="rsb")
            amx = qz.tile([128, 1], F32, tag="amx")
            rcpq = qz.tile([128, 1], F32, tag="rcpq")
            oi8 = qz.tile([128, 4 * DIM], I8, tag="oi8")
            nc.gpsimd.dma_start(
                out=rsb.rearrange("p (sb d) -> p sb d", d=DIM),
                in_=rs_d[:].rearrange("(sb p) d -> p sb d", p=128),
            )
            nc.vector.tensor_reduce(
                amx[:], rsb[:], mybir.AxisListType.XYZW, mybir.AluOpType.max,
                apply_absolute_value=True,
            )
            nc.vector.reciprocal(rcpq[:], amx[:])
            nc.vector.tensor_scalar(
                oi8[:], rsb[:], rcpq[:, 0:1], 127.0,
                mybir.AluOpType.mult, mybir.AluOpType.mult,
            )
            nc.sync.dma_start(
                out=out_d[:].rearrange("(sb p) d -> p sb d", p=128),
                in_=oi8.rearrange("p (sb d) -> p sb d", d=DIM),
            )
            nc.sync.dma_start(out=outsc_d[:], in_=amx[:])

    _nc_cache["nc"] = nc
    return nc


def _q8(x):
    s = float(np.abs(x).max()) / 127.0
    return np.clip(np.round(x / s), -127, 127).astype(np.int8), s


def _hilo(x):
    bf = ml_dtypes.bfloat16
    hi = np.float32(bf(x))
    lo = np.float32(bf(np.float32(x) - hi))
    return bf(hi), bf(lo)


def _prep_all(tokens, value_residual, episode_ids, Wq, Wkv, Wo, Wg, Wmix):
    bf = ml_dtypes.bfloat16
    perm = np.concatenate([np.arange(0, D, 2), np.arange(1, D, 2)])

    tokens = np.asarray(tokens, dtype=np.float32)
    st = float(np.abs(tokens).max()) / 127.0
    tokq = [
        np.ascontiguousarray(
            np.clip(np.round(tokens[b].T / st), -127, 127).astype(np.int8)
        )
        for b in range(B)
    ]                                                                         # [DIM,S] i8

    Wqp = np.asarray(Wq).reshape(DIM, H, D)[:, :, perm]
    Wkp = np.asarray(Wkv)[:, : H * D].reshape(DIM, H, D)[:, :, perm]
    Wvp = np.asarray(Wkv)[:, H * D :].reshape(DIM, H, D)
    Wop = np.asarray(Wo).reshape(H, D, DIM)
    packs = []
    for g in range(4):
        hs = slice(4 * g, 4 * g + 4)
        wq_q, swq = _q8(Wqp[:, hs].reshape(DIM, 256))
        wk_q, swk = _q8(Wkp[:, hs].reshape(DIM, 256))
        wv_q, swv = _q8(Wvp[:, hs].reshape(DIM, 256))
        wall = np.concatenate([wq_q, wk_q, wv_q], axis=1)                     # [DIM,768] i8
        wall = np.ascontiguousarray(wall.reshape(NKC, 128, WUP))
        wo = np.ascontiguousarray(
            Wop[hs].reshape(256, DIM).astype(bf).reshape(2, 128, DIM)
        )
        cexp = st * st * swq * swk * (D ** -0.5)
        csv = st * swv
        packs.append((wall, wo, cexp, csv))

    # gate/mix projections on host (tiny, keeps sigmoid paths off fp8)
    graw_all = np.einsum("bnd,dh->bhn", tokens, np.asarray(Wg, dtype=np.float32))   # [B,H,S]
    gate1_all = 1.0 / (1.0 + np.exp(-graw_all))
    mraw_all = np.einsum("bnd,dh->bnh", tokens, np.asarray(Wmix, dtype=np.float32)) # [B,S,H]

    ubqs, ubrs = [], []
    kk = np.arange(S)
    for b in range(B):
        ids = np.asarray(episode_ids[b])
        ee = np.searchsorted(ids, ids, side="right") - 1                      # [S]
        ubl = ee - (kk // 128) * 128                                          # int
        q = ubl // 256
        r = ubl - 256 * q
        ubqs.append(np.ascontiguousarray(q.reshape(NSB, 128).T.astype(bf)))   # [128,NSB]
        ubrs.append(np.ascontiguousarray(r.reshape(NSB, 128).T.astype(bf)))

    vr_f = np.asarray(value_residual, dtype=np.float32)
    svr = np.abs(vr_f).max(axis=(2, 3)) / 127.0                               # [B,H]
    vrq = np.clip(np.round(vr_f / svr[:, :, None, None]), -127, 127).astype(np.int8)

    in_maps = []
    for c in range(NCORES):
        b, g = divmod(c, 4)
        wall, wo, cexp, csv = packs[g]
        hs = slice(4 * g, 4 * g + 4)
        cv = np.empty((128, 12), dtype=bf)
        eh, el = _hilo(cexp)
        vh, vl = _hilo(csv)
        cv[:, 0], cv[:, 1], cv[:, 2], cv[:, 3] = eh, el, vh, vl
        for h in range(HPC):
            sh_, sl_ = _hilo(float(svr[b, 4 * g + h]))
            cv[:, 4 + 2 * h], cv[:, 5 + 2 * h] = sh_, sl_

        g1 = gate1_all[b, hs].astype(bf)                                      # [4,S]
        mraw = np.ascontiguousarray(
            mraw_all[b, :, hs].reshape(NSB, 128, 4).transpose(1, 0, 2).reshape(128, 64)
        ).astype(bf)                                                          # [128,64]
        blob8 = np.concatenate([
            tokq[b][:, g * 512 : (g + 1) * 512].ravel(),
            wall[4 * b : 4 * b + 4].ravel(),
            vrq[b, hs].ravel(),
        ])
        blob16 = np.concatenate([
            wo[b].ravel(),
            ubqs[b].ravel(), ubrs[b].ravel(),
            g1.ravel(), mraw.ravel(), cv.ravel(),
        ])
        in_maps.append({"blob8": blob8, "blob16": blob16})
    return in_maps


# ---- cached PJRT dispatch: jit built once, zero output-buffers device-resident ----
_runner_cache = {}


def _get_runner(nc):
    if "fn" in _runner_cache:
        return _runner_cache
    from jax.experimental.shard_map import shard_map

    bass2jax.install_neuronx_cc_hook()
    partition_name = nc.partition_id_tensor.name if nc.partition_id_tensor else None
    in_names, out_names, out_avals = [], [], []
    for alloc in nc.m.functions[0].allocations:
        if not isinstance(alloc, mybir.MemoryLocationSet):
            continue
        name = alloc.memorylocations[0].name
        if alloc.kind == "ExternalInput":
            if name != partition_name:
                in_names.append(name)
        elif alloc.kind == "ExternalOutput":
            out_avals.append(
                jax.core.ShapedArray(tuple(alloc.tensor_shape), mybir.dt.np(alloc.dtype))
            )
            out_names.append(name)
    in_names_all = list(in_names) + list(out_names)
    if partition_name is not None:
        in_names_all.append(partition_name)

    def _body(*args):
        operands = list(args)
        if partition_name is not None:
            operands.append(bass2jax.partition_id_tensor())
        outs = bass2jax._bass_exec_p.bind(
            *operands,
            out_avals=tuple(out_avals),
            in_names=tuple(in_names_all),
            out_names=tuple(out_names),
            lowering_input_output_aliases=(),
            sim_require_finite=True,
            sim_require_nnan=True,
            nc=nc,
        )
        return tuple(outs)

    devices = jax.devices()[:NCORES]
    mesh = jax.sharding.Mesh(np.asarray(devices), ("core",))
    P = jax.sharding.PartitionSpec
    n_in = len(in_names) + len(out_names)
    fn = jax.jit(
        shard_map(
            _body, mesh=mesh, in_specs=(P("core"),) * n_in,
            out_specs=(P("core"),) * len(out_names), check_rep=False,
        ),
        keep_unused=True,
    )
    sh = jax.sharding.NamedSharding(mesh, P("core"))
    zeros_dev = [
        jax.device_put(np.zeros((NCORES * a.shape[0], *a.shape[1:]), a.dtype), sh)
        for a in out_avals
    ]
    jax.block_until_ready(zeros_dev)
    _runner_cache.update(fn=fn, in_names=in_names, zeros_dev=zeros_dev)
    return _runner_cache


def _execute(nc, in_maps):
    r = _get_runner(nc)
    concat_in = [
        np.concatenate([np.asarray(m[name]) for m in in_maps], axis=0)
        for name in r["in_names"]
    ]
    outs = r["fn"](*concat_in, *r["zeros_dev"])
    outs[0].copy_to_host_async()
    outs[1].copy_to_host_async()
    oi = np.asarray(outs[0]).reshape(NCORES, 512, DIM)
    sc = np.asarray(outs[1]).reshape(NCORES, 128, 1) / 127.0
    res = np.zeros((B, S, DIM), dtype=np.float32)
    for c in range(NCORES):
        b, rk = divmod(c, 4)
        scl = np.tile(sc[c], (4, 1)).reshape(512, 1)
        res[b, rk * 512 : (rk + 1) * 512] = oi[c].astype(np.float32) * scl
    return res


def kernel(tokens, value_residual, episode_ids, Wq, Wkv, Wo, Wg, Wmix):
    nc = build_nc()
    in_maps = _prep_all(tokens, value_residual, episode_ids, Wq, Wkv, Wo, Wg, Wmix)
    return _execute(nc, in_maps)


# revision 16
# speedup vs baseline: 7.5299x; 1.0189x over previous
import numpy as np
import ml_dtypes

import jax
import concourse.bass as bass
from bass_rust import InstructionNameOrderedSet
import concourse.mybir as mybir
from concourse import tile
from concourse import bass2jax

BF16 = mybir.dt.bfloat16
F32 = mybir.dt.float32
I8 = mybir.dt.int8
AF = mybir.ActivationFunctionType

B, S, DIM, H, D = 2, 2048, 1024, 16, 64
WIN = 512
HPC = 4          # heads per core
NCORES = 8
NSB = S // 128   # 16 seq blocks
NKC = DIM // 128  # 8 contraction chunks
STRIPW = 640     # 128 keys attend to <=640 queries (dist 0..512 + 127)
WUP = 768        # uploaded weight cols per kc chunk: wq 256 | wk 256 | wv 256 (int8)
WSLW = 1280      # sbuf slab cols per kc chunk: wq | wk | wqrot | wkrot | wv

# int8 blob layout (elements): tok quarter | wall half | vr  (scales in blob16)
TOK_N = DIM * 512                 # 524288
WALL_N = 4 * 128 * WUP            # 393216
VR_N = HPC * S * D                # 524288
BLOB8_N = TOK_N + WALL_N + VR_N   # 1441792
# bf16 blob layout (elements): wo | ubq | ubr | gate1 | mraw | cvec [128,12]
WO_N = 128 * DIM                  # 131072
UB_N = 128 * NSB                  # 2048
GR_N = 4 * S                     # 8192
MR_N = 128 * 64                   # 8192
CV_N = 128 * 12                   # 1536
BLOB16_N = WO_N + 2 * UB_N + GR_N + MR_N + CV_N

_nc_cache = {}


def _patched_drain(self, tick_clock, wait_clock):
    # Tail drain: walrus limits sync waits per instruction, so convert the
    # multi-wait drain into a chain of single-wait sem waits on SyncE.
    from concourse.vector_clock import ScopedClock

    nc = self.nc
    probe = mybir.InstNoOp(name="__drain_probe", engine=mybir.EngineType.SP, ins=[], outs=[])
    wait_clock.add_sem_waits(probe, ScopedClock({None: tick_clock.global_clock}))
    id2h = {h.num: h for h in self.sems.allocated().values()}
    si = getattr(probe, "sync_info", None)
    if si is not None:
        for w in si.on_wait:
            h = id2h.get(w.id)
            if h is not None:
                nc.sync.wait_ge(h, w.wait_value)
    nc.sync.drain()
    nc.all_engine_barrier()
    popped = nc._tile_sem_poison_stack.pop()
    assert popped is self._sem_poison
    nc.clear_and_free_semaphores(list(self.sems.allocated().values()))
    nc.all_engine_barrier()


tile.TileContext._drain_and_barrier = _patched_drain


def _consts():
    bf = ml_dtypes.bfloat16
    pos = np.arange(S, dtype=np.float64)
    invf = 1.0 / (10000.0 ** (np.arange(0, D, 2, dtype=np.float64) / D))   # [32]
    ang = pos[None, :] * invf[:, None]                                     # [32,S]
    c32, s32 = np.cos(ang), np.sin(ang)
    cosm = np.tile(c32, (4, 1)).astype(bf)                                 # [128,S]
    sgn = np.concatenate([-s32, s32], axis=0)                              # [64,S]
    sinm = np.tile(sgn, (2, 1)).astype(bf)
    p = np.arange(128)
    j = np.arange(STRIPW)
    win = ((j[None, :] >= p[:, None]) & (j[None, :] - p[:, None] <= WIN)).astype(np.float32)
    iotaw = np.ascontiguousarray(np.broadcast_to(j[None, :].astype(np.float32), (128, STRIPW)))
    return cosm, sinm, win, iotaw


def build_nc():
    if "nc" in _nc_cache:
        return _nc_cache["nc"]
    nc = bass.Bass(num_devices=NCORES)

    # ---- per-core external I/O (core c = 4*b + g: batch b, head-group g) ----
    # blob8: tok quarter [DIM,512] int8 | wall half [4,128,WUP] int8
    blob8_d = nc.dram_tensor("blob8", [BLOB8_N], I8, kind="ExternalInput")
    # blob16: wo half | vr | ubq | ubr | gate1 | mraw | cvec
    blob16_d = nc.dram_tensor("blob16", [BLOB16_N], BF16, kind="ExternalInput")
    out_d = nc.dram_tensor("out", [512, DIM], I8, kind="ExternalOutput")
    outsc_d = nc.dram_tensor("outsc", [128, 1], F32, kind="ExternalOutput")

    # ---- NEFF-embedded constants (shipped at model load, not per call) ----
    cosm_np, sinm_np, win_np, iotaw_np = _consts()
    cos_d = nc.inline_tensor(cosm_np, name="cosk")
    sin_d = nc.inline_tensor(sinm_np, name="sink")
    win_d = nc.inline_tensor(win_np, name="wink")
    iota_d = nc.inline_tensor(iotaw_np, name="iotak")

    # ---- internal DRAM for collectives ----
    tok_stage = nc.dram_tensor("tok_stage", [TOK_N], I8, kind="Internal")
    wall_stage = nc.dram_tensor("wall_stage", [WALL_N], I8, kind="Internal")
    wo_stage = nc.dram_tensor("wo_stage", [WO_N], BF16, kind="Internal")
    tok_g = nc.dram_tensor("tok_g", [4 * TOK_N], I8, kind="Internal")
    wall_g = nc.dram_tensor("wall_g", [2 * WALL_N], I8, kind="Internal")
    wo_g = nc.dram_tensor("wo_g", [2 * WO_N], BF16, kind="Internal")
    pout_d = nc.dram_tensor("pout", [S, DIM], BF16, kind="Internal")
    rs_d = nc.dram_tensor("rsout", [512, DIM], BF16, kind="Internal")

    s_stage = nc.alloc_semaphore("s_stage")
    s_ag = nc.alloc_semaphore("s_ag")
    s_rs = nc.alloc_semaphore("s_rs")
    s_fin = nc.alloc_semaphore("s_fin")

    G4 = [[0, 1, 2, 3], [4, 5, 6, 7]]        # batch groups (head-parallel)
    G2 = [[0, 4], [1, 5], [2, 6], [3, 7]]    # same head-group pairs across batches

    # ---- pre-tile: stage inputs to internal DRAM, gather across cores ----
    nc.gpsimd.dma_start(out=tok_stage[:], in_=blob8_d[0:TOK_N]).then_inc(s_stage, 16)
    nc.gpsimd.dma_start(
        out=wall_stage[:], in_=blob8_d[TOK_N : TOK_N + WALL_N]
    ).then_inc(s_stage, 16)
    nc.gpsimd.dma_start(out=wo_stage[:], in_=blob16_d[0:WO_N]).then_inc(s_stage, 16)
    nc.gpsimd.wait_ge(s_stage, 48)
    nc.gpsimd.collective_compute(
        "AllGather", mybir.AluOpType.bypass, replica_groups=G4,
        ins=[tok_stage[:].opt()], outs=[tok_g[:].opt()],
    ).then_inc(s_ag, 1)
    nc.gpsimd.collective_compute(
        "AllGather", mybir.AluOpType.bypass, replica_groups=G2,
        ins=[wall_stage[:].opt()], outs=[wall_g[:].opt()],
    ).then_inc(s_ag, 1)
    nc.gpsimd.collective_compute(
        "AllGather", mybir.AluOpType.bypass, replica_groups=G2,
        ins=[wo_stage[:].opt()], outs=[wo_g[:].opt()],
    ).then_inc(s_ag, 1)
    nc.gpsimd.wait_ge(s_ag, 3)

    with tile.TileContext(nc) as tc:
        with (
            tc.tile_pool(name="big", bufs=1) as big,
            tc.tile_pool(name="stg", bufs=2) as stg,
            tc.tile_pool(name="pp", bufs=2, space=bass.MemorySpace.PSUM) as pp,
        ):
            # ---- resident SBUF slabs ----
            tok = big.tile([128, NKC * S], BF16, tag="tok")          # 32KB/p
            wsl = big.tile([128, NKC * WSLW], BF16, tag="wsl")       # 20KB/p
            wo_sb = big.tile([128, 2 * DIM], BF16, tag="wo")         # 4KB/p
            cosm = big.tile([128, S], BF16, tag="cos")
            sinm = big.tile([128, S], BF16, tag="sin")
            tokq8 = big.tile([128, NKC * S], I8, tag="tokq8")        # 16KB/p
            wslq8 = big.tile([128, NKC * WUP], I8, tag="wslq8")      # 6KB/p
            winsb = big.tile([128, STRIPW], F32, tag="winsb")
            iotasb = big.tile([128, STRIPW], F32, tag="iotasb")
            ubq_t = big.tile([128, NSB], BF16, tag="ubq")
            ubr_t = big.tile([128, NSB], BF16, tag="ubr")
            ubl = big.tile([128, NSB], F32, tag="ubl")
            vrq8 = [big.tile([128, NSB * D], I8, tag=f"vq{h}", name=f"vq{h}") for h in range(HPC)]
            vrs = [big.tile([128, NSB * D], BF16, tag=f"vr{h}", name=f"vr{h}") for h in range(HPC)]
            vaug = [big.tile([128, NSB * 65], BF16, tag=f"va{h}", name=f"va{h}") for h in range(HPC)]
            mixs = big.tile([128, 64], F32, tag="mix")               # sigmoid(mix)
            mixr = big.tile([128, 64], BF16, tag="mixr")             # uploaded raw mix
            gate1 = big.tile([1, HPC * S], BF16, tag="gate1")        # uploaded sigmoid(gate)
            cvec = big.tile([128, 12], BF16, tag="cvec")             # scale consts hi/lo
            cexp = big.tile([128, 1], F32, tag="cexp")               # exp logit scale
            csv = big.tile([128, 1], F32, tag="csv")                 # v dequant scale
            vsc = big.tile([128, 4], F32, tag="vsc")                 # vr dequant scales
            ones1 = big.tile([1, 64], BF16, tag="ones1")
            qkslab = big.tile([128, 8 * S], BF16, tag="qkslab")
            qraw = [qkslab[:, 0 * S : 1 * S], qkslab[:, 1 * S : 2 * S]]
            kraw = [qkslab[:, 2 * S : 3 * S], qkslab[:, 3 * S : 4 * S]]
            qrot = [qkslab[:, 4 * S : 5 * S], qkslab[:, 5 * S : 6 * S]]
            krot = [qkslab[:, 6 * S : 7 * S], qkslab[:, 7 * S : 8 * S]]
            qro, kro = qraw, kraw  # roped in place
            # PT ring: 5 live strips per head
            pts = [big.tile([128, 5 * STRIPW], BF16, tag=f"pt{h}", name=f"pt{h}") for h in range(HPC)]
            outg = [big.tile([128, S], BF16, tag=f"og{p}", name=f"og{p}") for p in range(2)]
            vtmp = big.tile([128, D], F32, tag="vtmp")
            dmy = big.tile([1, 128], F32, tag="dmy")
            dmyc = [0]

            pend = []

            def guard(inst):
                if pend:
                    s = InstructionNameOrderedSet()
                    for n in pend:
                        s.add(n)
                    inst.ins.add_nosync_dependencies_from(s)
                    pend.clear()
                return inst

            def absorb(*aps):
                for ap in aps:
                    i = dmyc[0] % 128
                    dmyc[0] += 1
                    ii = nc.vector.tensor_copy(dmy[0:1, i : i + 1], ap[0:1, 0:1])
                    pend.append(ii.ins.name)

            dmyA = big.tile([1, 128], F32, tag="dmyA")
            dmyAc = [0]

            def absorb_act(ap):
                i = dmyAc[0] % 128
                dmyAc[0] += 1
                ii = nc.scalar.copy(dmyA[0:1, i : i + 1], ap[0:1, 0:1])
                pend.append(ii.ins.name)

            bcb = big.tile([32, 1024], BF16, tag="bcb")
            bcbc = [0]
            crumb_st = {"last": None}

            def crumb(src_ap):
                crumb_st["last"] = src_ap[0:1, 0:1]

            def pe_absorb(ap=None):
                ap = ap if ap is not None else crumb_st["last"]
                if ap is None:
                    return
                if ap.partition_size() >= 32 and ap.dtype == BF16:
                    ii = nc.tensor.ldweights(ap[0:32, 0:1])
                else:
                    i = bcbc[0] % 1024
                    bcbc[0] += 1
                    nc.vector.tensor_copy(bcb[0:1, i : i + 1], ap[0:1, 0:1])
                    ii = nc.tensor.ldweights(bcb[0:32, i : i + 1])
                pend.append(ii.ins.name)

            # ---- loads (from gathered internal DRAM + inline consts) ----
            # tokens/weights: int8 staged resident, converted to bf16 on DVE
            tokv = tok.rearrange("p (k qq s) -> p k qq s", k=NKC, qq=4, s=512)
            tkv8 = tokq8.rearrange("p (k qq s) -> p k qq s", k=NKC, qq=4, s=512)
            for q in range(4):
                nc.gpsimd.dma_start(
                    out=tkv8[:, :, q, :],
                    in_=tok_g[q * TOK_N : (q + 1) * TOK_N].rearrange(
                        "(k p s) -> p k s", k=NKC, p=128, s=512
                    ),
                )
            for q in range(4):
                absorb(tokq8[:, q * 512 : q * 512 + 1])
                guard(nc.vector.tensor_copy(tokv[:, :, q, :], tkv8[:, :, q, :]))
            for kc in range(NKC):
                nc.gpsimd.dma_start(
                    out=wslq8[:, kc * WUP : (kc + 1) * WUP],
                    in_=wall_g[kc * 128 * WUP : (kc + 1) * 128 * WUP].rearrange(
                        "(p c) -> p c", p=128, c=WUP
                    ),
                )
            for kc in range(NKC):
                absorb(wslq8[:, kc * WUP : kc * WUP + 1])
                guard(nc.vector.tensor_copy(
                    wsl[:, kc * WSLW : kc * WSLW + 512],
                    wslq8[:, kc * WUP : kc * WUP + 512],
                ))
                guard(nc.vector.tensor_copy(
                    wsl[:, kc * WSLW + 1024 : kc * WSLW + 1280],
                    wslq8[:, kc * WUP + 512 : kc * WUP + 768],
                ))
            for half in range(2):
                nc.gpsimd.dma_start(
                    out=wo_sb[:, half * DIM : half * DIM + DIM],
                    in_=wo_g[half * WO_N : (half + 1) * WO_N].rearrange(
                        "(p d) -> p d", p=128, d=DIM
                    ),
                )
            nc.gpsimd.dma_start(out=cosm[:], in_=cos_d[:])
            nc.gpsimd.dma_start(out=sinm[:], in_=sin_d[:])
            nc.gpsimd.dma_start(out=winsb[:], in_=win_d[:])
            nc.gpsimd.dma_start(out=iotasb[:], in_=iota_d[:])
            off = WO_N
            nc.gpsimd.dma_start(
                out=ubq_t[:],
                in_=blob16_d[off : off + UB_N].rearrange("(p n) -> p n", p=128, n=NSB),
            )
            off += UB_N
            nc.gpsimd.dma_start(
                out=ubr_t[:],
                in_=blob16_d[off : off + UB_N].rearrange("(p n) -> p n", p=128, n=NSB),
            )
            off += UB_N
            nc.gpsimd.dma_start(
                out=gate1[:],
                in_=blob16_d[off : off + GR_N].rearrange("(o x) -> o x", o=1, x=GR_N),
            )
            off += GR_N
            nc.gpsimd.dma_start(
                out=mixr[:],
                in_=blob16_d[off : off + MR_N].rearrange("(p n) -> p n", p=128, n=64),
            )
            # vr: one strided int8 DMA per head, dequant on DVE
            vr0 = TOK_N + WALL_N
            for h in range(HPC):
                nc.gpsimd.dma_start(
                    out=vrq8[h].rearrange("p (sb d) -> p sb d", sb=NSB, d=D),
                    in_=blob8_d[vr0 + h * NSB * 128 * D : vr0 + (h + 1) * NSB * 128 * D]
                    .rearrange("(sb p d) -> p sb d", sb=NSB, p=128, d=D),
                )

            off += MR_N
            nc.gpsimd.dma_start(
                out=cvec[:],
                in_=blob16_d[off : off + CV_N].rearrange("(p n) -> p n", p=128, n=12),
            )
            nc.vector.memset(ones1[:], 1.0)
            absorb(cosm, sinm, gate1, cvec)
            guard(nc.vector.tensor_add(cexp[:], cvec[:, 0:1], cvec[:, 1:2]))
            guard(nc.vector.tensor_add(csv[:], cvec[:, 2:3], cvec[:, 3:4]))
            for h in range(HPC):
                guard(nc.vector.tensor_add(
                    vsc[:, h : h + 1], cvec[:, 4 + 2 * h : 5 + 2 * h],
                    cvec[:, 5 + 2 * h : 6 + 2 * h],
                ))
            for h in range(HPC):
                absorb(vrq8[h][:, 0:1])
                guard(nc.vector.tensor_scalar(
                    vrs[h][:], vrq8[h][:], vsc[:, h : h + 1], None, mybir.AluOpType.mult
                ))
            absorb_act(mixr[0:1, 0:1])
            absorb_act(cexp[0:1, 0:1])
            absorb_act(csv[0:1, 0:1])

            # ---- ub reconstruct + masks on device: msl = win * (iota <= ub) ----
            absorb(ubq_t, ubr_t)
            guard(nc.vector.scalar_tensor_tensor(
                ubl[:], ubq_t[:], 256.0, ubr_t[:],
                mybir.AluOpType.mult, mybir.AluOpType.add,
            ))
            absorb(winsb, iotasb)

            # ---- rot weights on device: wqrot/wkrot = 32-col half-swap of wq/wk ----
            for kc in range(NKC):
                base = kc * WSLW
                for h in range(HPC):
                    for dst0, src0 in ((0, 32), (32, 0)):
                        nc.vector.tensor_copy(
                            wsl[:, base + 512 + h * 64 + dst0 : base + 512 + h * 64 + dst0 + 32],
                            wsl[:, base + h * 64 + src0 : base + h * 64 + src0 + 32],
                        )
                        nc.vector.tensor_copy(
                            wsl[:, base + 768 + h * 64 + dst0 : base + 768 + h * 64 + dst0 + 32],
                            wsl[:, base + 256 + h * 64 + src0 : base + 256 + h * 64 + src0 + 32],
                        )

            def wchunk(kc, c0, c1):
                return wsl[:, kc * WSLW + c0 : kc * WSLW + c1]

            def tchunk(kc, s0, s1):
                return tok[:, kc * S + s0 : kc * S + s1]

            # ---- phase 1: T-orient projections: q, k (dual use) ----
            NS = 4  # seq chunks of 512
            for dest, c0 in (
                (qraw[0], 0), (qraw[1], 128), (kraw[0], 256), (kraw[1], 384),
                (qrot[0], 512), (qrot[1], 640), (krot[0], 768), (krot[1], 896),
            ):
                for ns in range(NS):
                    ps = pp.tile([128, 512], F32, tag="ps1", name="psA")
                    pe_absorb()
                    for kc in range(NKC):
                        guard(nc.tensor.matmul(
                            ps[:],
                            wchunk(kc, c0, c0 + 128),
                            tchunk(kc, ns * 512, ns * 512 + 512),
                            start=(kc == 0),
                            stop=(kc == NKC - 1),
                        ))
                    nc.vector.tensor_copy(dest[:, ns * 512 : ns * 512 + 512], ps[:])
                    crumb(dest[:, ns * 512 : ns * 512 + 512])

            # ---- phase 2: v (natural orient) + lerp with value residual ----
            for sb in range(NSB):
                ps = pp.tile([128, 256], F32, tag="ps1", name="psV")
                pe_absorb()
                for kc in range(NKC):
                    guard(nc.tensor.matmul(
                        ps[:],
                        tchunk(kc, sb * 128, sb * 128 + 128),
                        wchunk(kc, 1024, 1280),
                        start=(kc == 0),
                        stop=(kc == NKC - 1),
                    ))
                absorb_act(mixr[0:1, sb * 4 : sb * 4 + 1])
                guard(nc.scalar.activation(
                    mixs[:, sb * 4 : sb * 4 + 4], mixr[:, sb * 4 : sb * 4 + 4], AF.Sigmoid
                ))
                v_t = stg.tile([128, 256], F32, tag="vt", name="vt", bufs=2)
                absorb(ps[0:1, 0:1])
                guard(nc.vector.tensor_scalar(
                    v_t[:], ps[:], csv[:, 0:1], None, mybir.AluOpType.mult
                ))
                for h in range(HPC):
                    absorb(vrs[h][:, sb * D : sb * D + D])
                    guard(nc.vector.tensor_sub(
                        vtmp[:],
                        vrs[h][:, sb * D : sb * D + D],
                        v_t[:, h * D : h * D + D],
                    ))
                    absorb(mixs[:, sb * 4 + h : sb * 4 + h + 1])
                    # v' = mix*(vr - v) + v
                    guard(nc.vector.scalar_tensor_tensor(
                        vaug[h][:, sb * 65 : sb * 65 + 64],
                        vtmp[:],
                        mixs[:, sb * 4 + h : sb * 4 + h + 1],
                        v_t[:, h * D : h * D + D],
                        mybir.AluOpType.mult,
                        mybir.AluOpType.add,
                    ))
                    nc.vector.memset(vaug[h][:, sb * 65 + 64 : sb * 65 + 65], 1.0)
                crumb(vaug[HPC - 1][:, sb * 65 : sb * 65 + 64])

            # ---- phase 1b: RoPE via half-swap + cos/sin maps ----
            for raw, rot in (
                (qraw[0], qrot[0]),
                (qraw[1], qrot[1]),
                (kraw[0], krot[0]),
                (kraw[1], krot[1]),
            ):
                nc.vector.tensor_mul(rot[:], rot[:], sinm[:])
                nc.vector.tensor_mul(raw[:], raw[:], cosm[:])
                nc.vector.tensor_add(raw[:], raw[:], rot[:])
                crumb(raw[:])

            # ---- phase 3: attention ----
            ptw_hist, ring_hist, fbs_hist, og_hist = [], [], [], []
            for kb in range(NSB):
                Wn = min(STRIPW, S - kb * 128)
                mtile = stg.tile([128, STRIPW], BF16, tag="mt", name="mt", bufs=2)
                guard(nc.vector.scalar_tensor_tensor(
                    mtile[:],
                    iotasb[:],
                    ubl[:, kb : kb + 1],
                    winsb[:],
                    mybir.AluOpType.is_le,
                    mybir.AluOpType.mult,
                ))
                for h in range(HPC):
                    p, hh = divmod(h, 2)
                    b0 = hh * 64
                    ptv = pts[h][:, (kb % 5) * STRIPW : (kb % 5) * STRIPW + STRIPW]
                    sim = pp.tile([128, STRIPW], F32, tag="psS", name="psS")
                    pe_absorb(kro[p])
                    pe_absorb(qro[p])
                    if len(ptw_hist) >= 1:
                        pe_absorb(ptw_hist[-1])
                    for c0 in range(0, Wn, 512):
                        c1 = min(c0 + 512, Wn)
                        guard(nc.tensor.matmul(
                            sim[:, c0:c1],
                            kro[p][b0 : b0 + 64, kb * 128 : kb * 128 + 128],
                            qro[p][b0 : b0 + 64, kb * 128 + c0 : kb * 128 + c1],
                            start=True,
                            stop=True,
                        ))
                    ptw = stg.tile([128, STRIPW], BF16, tag="ptw", name="ptw", bufs=2)
                    if ring_hist:
                        absorb_act(ring_hist[-1][0:1, 0:1])
                    absorb_act(sim[0:1, 0:1])
                    guard(nc.scalar.activation(
                        ptw[:, 0:Wn], sim[:, 0:Wn], AF.Exp, scale=cexp[:, 0:1]
                    ))
                    ptw_hist.append(ptw)
                    absorb(ptw[0:1, 0:1], ptv[0:1, 0:1])
                    guard(nc.vector.tensor_mul(
                        ptv[:, 0:Wn],
                        ptw[:, 0:Wn],
                        mtile[:, 0:Wn],
                    ))
                    ring_hist.append(ptv)
                    # AV for q-block qb = kb
                    av = pp.tile([65, 128], F32, tag="psAV", name="psAV", bufs=1)
                    pe_absorb(ptv)
                    if og_hist:
                        pe_absorb(og_hist[-1])
                    if fbs_hist:
                        pe_absorb(fbs_hist[-1][0:1, 0:1])
                    srcs = list(range(max(0, kb - 4), kb + 1))
                    for j, sc in enumerate(srcs):
                        off2 = (kb - sc) * 128
                        psrc = pts[h][:, (sc % 5) * STRIPW + off2 : (sc % 5) * STRIPW + off2 + 128]
                        guard(nc.tensor.matmul(
                            av[:],
                            vaug[h][:, sc * 65 : sc * 65 + 65],
                            psrc,
                            start=(j == 0),
                            stop=(j == len(srcs) - 1),
                        ))
                    # normalize + gate
                    rec_sb = big.tile([1, 128], F32, tag="recsb", name="recsb")
                    f_row = big.tile([1, 128], BF16, tag="frow", name="frow")
                    gsl = gate1[0:1, h * S + kb * 128 : h * S + kb * 128 + 128]
                    nc.vector.reciprocal(rec_sb[:], av[64:65, :])
                    absorb(gsl)
                    guard(nc.vector.tensor_mul(f_row[:], rec_sb[:], gsl))
                    pe_absorb(f_row[0:1, 0:1])
                    if fbs_hist:
                        pe_absorb(fbs_hist[-1][0:1, 0:1])
                    fps = pp.tile([64, 128], F32, tag="fps", name="fps", bufs=1)
                    guard(nc.tensor.matmul(fps[:], ones1[:], f_row[:], start=True, stop=True))
                    fbs = stg.tile([64, 128], F32, tag="fbs", name="fbs", bufs=1)
                    nc.vector.tensor_copy(fbs[:], fps[:])
                    fbs_hist.append(fbs)
                    guard(nc.vector.tensor_mul(
                        outg[p][b0 : b0 + 64, kb * 128 : kb * 128 + 128],
                        av[0:64, :],
                        fbs[:],
                    ))
                    og_hist.append(outg[p][b0 : b0 + 1, kb * 128 : kb * 128 + 1])

            # ---- phase 4: Wo -> partial out (internal DRAM) ----
            ost_hist = []
            crumb(outg[0][:, S - 128 : S])
            crumb(outg[1][:, S - 128 : S])
            for g8 in range(4):
                slab = qkslab[:, g8 * 4 * DIM : (g8 + 1) * 4 * DIM]
                for j in range(4):
                    sb = g8 * 4 + j
                    for half in range(2):
                        ps = pp.tile([128, 512], F32, tag="ps1", name="psO")
                        pe_absorb()
                        if ost_hist:
                            pe_absorb(ost_hist[-1])
                        for kc in range(2):
                            guard(nc.tensor.matmul(
                                ps[:],
                                outg[kc][:, sb * 128 : sb * 128 + 128],
                                wo_sb[:, kc * DIM + half * 512 : kc * DIM + half * 512 + 512],
                                start=(kc == 0),
                                stop=(kc == 1),
                            ))
                        dst = slab[:, j * DIM + half * 512 : j * DIM + half * 512 + 512]
                        absorb(ps[0:1, 0:1])
                        guard(nc.vector.tensor_copy(dst, ps[:]))
                        ost_hist.append(dst)
                nc.sync.dma_start(
                    out=pout_d[g8 * 512 : g8 * 512 + 512, :].rearrange(
                        "(sb p) d -> p sb d", p=128
                    ),
                    in_=slab.rearrange("p (sb d) -> p sb d", d=DIM),
                )

    # ---- post-tile (drain guarantees all DMAs done): reduce partials ----
    nc.gpsimd.collective_compute(
        "ReduceScatter", mybir.AluOpType.add, replica_groups=G4,
        ins=[pout_d[:].opt()], outs=[rs_d[:].opt()],
    ).then_inc(s_rs, 1)
    nc.gpsimd.wait_ge(s_rs, 1)
    with tile.TileContext(nc) as tc2:
        with tc2.tile_pool(name="qz", bufs=1) as qz:
            rsb = qz.tile([128, 4 * DIM], BF16, tag="rsb")
            amx = qz.tile([128, 1], F32, tag="amx")
            rcpq = qz.tile([128, 1], F32, tag="rcpq")
            oi8 = qz.tile([128, 4 * DIM], I8, tag="oi8")
            nc.gpsimd.dma_start(
                out=rsb.rearrange("p (sb d) -> p sb d", d=DIM),
                in_=rs_d[:].rearrange("(sb p) d -> p sb d", p=128),
            )
            nc.vector.tensor_reduce(
                amx[:], rsb[:], mybir.AxisListType.XYZW, mybir.AluOpType.max,
                apply_absolute_value=True,
            )
            nc.vector.reciprocal(rcpq[:], amx[:])
            nc.vector.tensor_scalar(
                oi8[:], rsb[:], rcpq[:, 0:1], 127.0,
                mybir.AluOpType.mult, mybir.AluOpType.mult,
            )
            nc.sync.dma_start(
                out=out_d[:].rearrange("(sb p) d -> p sb d", p=128),
                in_=oi8.rearrange("p (sb d) -> p sb d", d=DIM),
            )
            nc.sync.dma_start(out=outsc_d[:], in_=amx[:])

    _nc_cache["nc"] = nc
    return nc


def _q8(x):
    s = float(np.abs(x).max()) / 127.0
    return np.clip(np.round(x / s), -127, 127).astype(np.int8), s


def _hilo(x):
    bf = ml_dtypes.bfloat16
    hi = np.float32(bf(x))
    lo = np.float32(bf(np.float32(x) - hi))
    return bf(hi), bf(lo)


def _prep_all(tokens, value_residual, episode_ids, Wq, Wkv, Wo, Wg, Wmix):
    bf = ml_dtypes.bfloat16
    perm = np.concatenate([np.arange(0, D, 2), np.arange(1, D, 2)])

    tokens = np.asarray(tokens, dtype=np.float32)
    st = float(np.abs(tokens).max()) / 127.0
    tokq = [
        np.ascontiguousarray(
            np.clip(np.round(tokens[b].T / st), -127, 127).astype(np.int8)
        )
        for b in range(B)
    ]                                                                         # [DIM,S] i8

    Wqp = np.asarray(Wq).reshape(DIM, H, D)[:, :, perm]
    Wkp = np.asarray(Wkv)[:, : H * D].reshape(DIM, H, D)[:, :, perm]
    Wvp = np.asarray(Wkv)[:, H * D :].reshape(DIM, H, D)
    Wop = np.asarray(Wo).reshape(H, D, DIM)
    packs = []
    for g in range(4):
        hs = slice(4 * g, 4 * g + 4)
        wq_q, swq = _q8(Wqp[:, hs].reshape(DIM, 256))
        wk_q, swk = _q8(Wkp[:, hs].reshape(DIM, 256))
        wv_q, swv = _q8(Wvp[:, hs].reshape(DIM, 256))
        wall = np.concatenate([wq_q, wk_q, wv_q], axis=1)                     # [DIM,768] i8
        wall = np.ascontiguousarray(wall.reshape(NKC, 128, WUP))
        wo = np.ascontiguousarray(
            Wop[hs].reshape(256, DIM).astype(bf).reshape(2, 128, DIM)
        )
        cexp = st * st * swq * swk * (D ** -0.5)
        csv = st * swv
        packs.append((wall, wo, cexp, csv))

    # gate/mix projections on host (tiny, keeps sigmoid paths off fp8)
    graw_all = np.einsum("bnd,dh->bhn", tokens, np.asarray(Wg, dtype=np.float32))   # [B,H,S]
    gate1_all = 1.0 / (1.0 + np.exp(-graw_all))
    mraw_all = np.einsum("bnd,dh->bnh", tokens, np.asarray(Wmix, dtype=np.float32)) # [B,S,H]

    ubqs, ubrs = [], []
    kk = np.arange(S)
    for b in range(B):
        ids = np.asarray(episode_ids[b])
        ee = np.searchsorted(ids, ids, side="right") - 1                      # [S]
        ubl = ee - (kk // 128) * 128                                          # int
        q = ubl // 256
        r = ubl - 256 * q
        ubqs.append(np.ascontiguousarray(q.reshape(NSB, 128).T.astype(bf)))   # [128,NSB]
        ubrs.append(np.ascontiguousarray(r.reshape(NSB, 128).T.astype(bf)))

    vr_f = np.asarray(value_residual, dtype=np.float32)
    svr = np.abs(vr_f).max(axis=(2, 3)) / 127.0                               # [B,H]
    vrq = np.clip(np.round(vr_f / svr[:, :, None, None]), -127, 127).astype(np.int8)

    in_maps = []
    for c in range(NCORES):
        b, g = divmod(c, 4)
        wall, wo, cexp, csv = packs[g]
        hs = slice(4 * g, 4 * g + 4)
        cv = np.empty((128, 12), dtype=bf)
        eh, el = _hilo(cexp)
        vh, vl = _hilo(csv)
        cv[:, 0], cv[:, 1], cv[:, 2], cv[:, 3] = eh, el, vh, vl
        for h in range(HPC):
            sh_, sl_ = _hilo(float(svr[b, 4 * g + h]))
            cv[:, 4 + 2 * h], cv[:, 5 + 2 * h] = sh_, sl_

        g1 = gate1_all[b, hs].astype(bf)                                      # [4,S]
        mraw = np.ascontiguousarray(
            mraw_all[b, :, hs].reshape(NSB, 128, 4).transpose(1, 0, 2).reshape(128, 64)
        ).astype(bf)                                                          # [128,64]
        blob8 = np.concatenate([
            tokq[b][:, g * 512 : (g + 1) * 512].ravel(),
            wall[4 * b : 4 * b + 4].ravel(),
            vrq[b, hs].ravel(),
        ])
        blob16 = np.concatenate([
            wo[b].ravel(),
            ubqs[b].ravel(), ubrs[b].ravel(),
            g1.ravel(), mraw.ravel(), cv.ravel(),
        ])
        in_maps.append({"blob8": blob8, "blob16": blob16})
    return in_maps


# ---- cached PJRT dispatch: jit built once, zero output-buffers device-resident ----
_runner_cache = {}


def _get_runner(nc):
    if "fn" in _runner_cache:
        return _runner_cache
    from jax.experimental.shard_map import shard_map

    bass2jax.install_neuronx_cc_hook()
    partition_name = nc.partition_id_tensor.name if nc.partition_id_tensor else None
    in_names, out_names, out_avals = [], [], []
    for alloc in nc.m.functions[0].allocations:
        if not isinstance(alloc, mybir.MemoryLocationSet):
            continue
        name = alloc.memorylocations[0].name
        if alloc.kind == "ExternalInput":
            if name != partition_name:
                in_names.append(name)
        elif alloc.kind == "ExternalOutput":
            out_avals.append(
                jax.core.ShapedArray(tuple(alloc.tensor_shape), mybir.dt.np(alloc.dtype))
            )
            out_names.append(name)
    in_names_all = list(in_names) + list(out_names)
    if partition_name is not None:
        in_names_all.append(partition_name)

    def _body(*args):
        operands = list(args)
        if partition_name is not None:
            operands.append(bass2jax.partition_id_tensor())
        outs = bass2jax._bass_exec_p.bind(
            *operands,
            out_avals=tuple(out_avals),
            in_names=tuple(in_names_all),
            out_names=tuple(out_names),
            lowering_input_output_aliases=(),
            sim_require_finite=True,
            sim_require_nnan=True,
            nc=nc,
        )
        return tuple(outs)

    devices = jax.devices()[:NCORES]
    mesh = jax.sharding.Mesh(np.asarray(devices), ("core",))
    P = jax.sharding.PartitionSpec
    n_in = len(in_names) + len(out_names)
    fn = jax.jit(
        shard_map(
            _body, mesh=mesh, in_specs=(P("core"),) * n_in,
            out_specs=(P("core"),) * len(out_names), check_rep=False,
        ),
        keep_unused=True,
    )
    sh = jax.sharding.NamedSharding(mesh, P("core"))
    zeros_dev = [
        jax.device_put(np.zeros((NCORES * a.shape[0], *a.shape[1:]), a.dtype), sh)
        for a in out_avals
    ]
    jax.block_until_ready(zeros_dev)
    _runner_cache.update(fn=fn, in_names=in_names, zeros_dev=zeros_dev)
    return _runner_cache


def _execute(nc, in_maps):
    r = _get_runner(nc)
    concat_in = [
        np.concatenate([np.asarray(m[name]) for m in in_maps], axis=0)
        for name in r["in_names"]
    ]
    outs = r["fn"](*concat_in, *r["zeros_dev"])
    outs[0].copy_to_host_async()
    outs[1].copy_to_host_async()
    oi = np.asarray(outs[0]).reshape(NCORES, 512, DIM)
    sc = np.asarray(outs[1]).reshape(NCORES, 1, 128, 1) / 127.0
    # core c = (b, rk) row-major == row-major [2,4] blocks of res
    res = np.empty((B, S, DIM), dtype=np.float32)
    scl = np.broadcast_to(sc, (NCORES, 4, 128, 1)).reshape(NCORES, 512, 1)
    np.multiply(oi, scl, out=res.reshape(NCORES, 512, DIM))
    return res


def kernel(tokens, value_residual, episode_ids, Wq, Wkv, Wo, Wg, Wmix):
    nc = build_nc()
    in_maps = _prep_all(tokens, value_residual, episode_ids, Wq, Wkv, Wo, Wg, Wmix)
    return _execute(nc, in_maps)


# revision 17
# speedup vs baseline: 7.5657x; 1.0048x over previous
import numpy as np
import ml_dtypes

import jax
import concourse.bass as bass
from bass_rust import InstructionNameOrderedSet
import concourse.mybir as mybir
from concourse import tile
from concourse import bass2jax

BF16 = mybir.dt.bfloat16
F32 = mybir.dt.float32
I8 = mybir.dt.int8
AF = mybir.ActivationFunctionType

B, S, DIM, H, D = 2, 2048, 1024, 16, 64
WIN = 512
HPC = 4          # heads per core
NCORES = 8
NSB = S // 128   # 16 seq blocks
NKC = DIM // 128  # 8 contraction chunks
STRIPW = 640     # 128 keys attend to <=640 queries (dist 0..512 + 127)
WUP = 768        # uploaded weight cols per kc chunk: wq 256 | wk 256 | wv 256 (int8)
WSLW = 1280      # sbuf slab cols per kc chunk: wq | wk | wqrot | wkrot | wv

# int8 blob layout (elements): tok quarter | wall half | vr  (scales in blob16)
TOK_N = DIM * 512                 # 524288
WALL_N = 4 * 128 * WUP            # 393216
VR_N = HPC * S * D                # 524288
BLOB8_N = TOK_N + WALL_N + VR_N   # 1441792
# bf16 blob layout (elements): wo | ubq | ubr | gate1 | mraw | cvec [128,12]
WO_N = 128 * DIM                  # 131072
UB_N = 128 * NSB                  # 2048
GR_N = 4 * S                     # 8192
MR_N = 128 * 64                   # 8192
CV_N = 128 * 12                   # 1536
BLOB16_N = WO_N + 2 * UB_N + GR_N + MR_N + CV_N

_nc_cache = {}


def _patched_drain(self, tick_clock, wait_clock):
    # Tail drain: walrus limits sync waits per instruction, so convert the
    # multi-wait drain into a chain of single-wait sem waits on SyncE.
    from concourse.vector_clock import ScopedClock

    nc = self.nc
    probe = mybir.InstNoOp(name="__drain_probe", engine=mybir.EngineType.SP, ins=[], outs=[])
    wait_clock.add_sem_waits(probe, ScopedClock({None: tick_clock.global_clock}))
    id2h = {h.num: h for h in self.sems.allocated().values()}
    si = getattr(probe, "sync_info", None)
    if si is not None:
        for w in si.on_wait:
            h = id2h.get(w.id)
            if h is not None:
                nc.sync.wait_ge(h, w.wait_value)
    nc.sync.drain()
    nc.all_engine_barrier()
    popped = nc._tile_sem_poison_stack.pop()
    assert popped is self._sem_poison
    nc.clear_and_free_semaphores(list(self.sems.allocated().values()))
    nc.all_engine_barrier()


tile.TileContext._drain_and_barrier = _patched_drain


def _consts():
    bf = ml_dtypes.bfloat16
    pos = np.arange(S, dtype=np.float64)
    invf = 1.0 / (10000.0 ** (np.arange(0, D, 2, dtype=np.float64) / D))   # [32]
    ang = pos[None, :] * invf[:, None]                                     # [32,S]
    c32, s32 = np.cos(ang), np.sin(ang)
    cosm = np.tile(c32, (4, 1)).astype(bf)                                 # [128,S]
    sgn = np.concatenate([-s32, s32], axis=0)                              # [64,S]
    sinm = np.tile(sgn, (2, 1)).astype(bf)
    p = np.arange(128)
    j = np.arange(STRIPW)
    win = ((j[None, :] >= p[:, None]) & (j[None, :] - p[:, None] <= WIN)).astype(np.float32)
    iotaw = np.ascontiguousarray(np.broadcast_to(j[None, :].astype(np.float32), (128, STRIPW)))
    return cosm, sinm, win, iotaw


def build_nc():
    if "nc" in _nc_cache:
        return _nc_cache["nc"]
    nc = bass.Bass(num_devices=NCORES)

    # ---- per-core external I/O (core c = 4*b + g: batch b, head-group g) ----
    # blob8: tok quarter [DIM,512] int8 | wall half [4,128,WUP] int8
    blob8_d = nc.dram_tensor("blob8", [BLOB8_N], I8, kind="ExternalInput")
    # blob16: wo half | vr | ubq | ubr | gate1 | mraw | cvec
    blob16_d = nc.dram_tensor("blob16", [BLOB16_N], BF16, kind="ExternalInput")
    out_d = nc.dram_tensor("out", [512, DIM], I8, kind="ExternalOutput")
    outsc_d = nc.dram_tensor("outsc", [128, 1], F32, kind="ExternalOutput")

    # ---- NEFF-embedded constants (shipped at model load, not per call) ----
    cosm_np, sinm_np, win_np, iotaw_np = _consts()
    cos_d = nc.inline_tensor(cosm_np, name="cosk")
    sin_d = nc.inline_tensor(sinm_np, name="sink")
    win_d = nc.inline_tensor(win_np, name="wink")
    iota_d = nc.inline_tensor(iotaw_np, name="iotak")

    # ---- internal DRAM for collectives ----
    tok_stage = nc.dram_tensor("tok_stage", [TOK_N], I8, kind="Internal")
    wall_stage = nc.dram_tensor("wall_stage", [WALL_N], I8, kind="Internal")
    wo_stage = nc.dram_tensor("wo_stage", [WO_N], BF16, kind="Internal")
    tok_g = nc.dram_tensor("tok_g", [4 * TOK_N], I8, kind="Internal")
    wall_g = nc.dram_tensor("wall_g", [2 * WALL_N], I8, kind="Internal")
    wo_g = nc.dram_tensor("wo_g", [2 * WO_N], BF16, kind="Internal")
    pout_d = nc.dram_tensor("pout", [S, DIM], BF16, kind="Internal")
    rs_d = nc.dram_tensor("rsout", [512, DIM], BF16, kind="Internal")

    s_stage = nc.alloc_semaphore("s_stage")
    s_ag = nc.alloc_semaphore("s_ag")
    s_rs = nc.alloc_semaphore("s_rs")
    s_fin = nc.alloc_semaphore("s_fin")

    G4 = [[0, 1, 2, 3], [4, 5, 6, 7]]        # batch groups (head-parallel)
    G2 = [[0, 4], [1, 5], [2, 6], [3, 7]]    # same head-group pairs across batches

    # ---- pre-tile: stage inputs to internal DRAM, gather across cores ----
    nc.gpsimd.dma_start(out=tok_stage[:], in_=blob8_d[0:TOK_N]).then_inc(s_stage, 16)
    nc.gpsimd.dma_start(
        out=wall_stage[:], in_=blob8_d[TOK_N : TOK_N + WALL_N]
    ).then_inc(s_stage, 16)
    nc.gpsimd.dma_start(out=wo_stage[:], in_=blob16_d[0:WO_N]).then_inc(s_stage, 16)
    nc.gpsimd.wait_ge(s_stage, 48)
    nc.gpsimd.collective_compute(
        "AllGather", mybir.AluOpType.bypass, replica_groups=G4,
        ins=[tok_stage[:].opt()], outs=[tok_g[:].opt()],
    ).then_inc(s_ag, 1)
    nc.gpsimd.collective_compute(
        "AllGather", mybir.AluOpType.bypass, replica_groups=G2,
        ins=[wall_stage[:].opt()], outs=[wall_g[:].opt()],
    ).then_inc(s_ag, 1)
    nc.gpsimd.collective_compute(
        "AllGather", mybir.AluOpType.bypass, replica_groups=G2,
        ins=[wo_stage[:].opt()], outs=[wo_g[:].opt()],
    ).then_inc(s_ag, 1)
    nc.gpsimd.wait_ge(s_ag, 3)

    with tile.TileContext(nc) as tc:
        with (
            tc.tile_pool(name="big", bufs=1) as big,
            tc.tile_pool(name="stg", bufs=2) as stg,
            tc.tile_pool(name="pp", bufs=2, space=bass.MemorySpace.PSUM) as pp,
        ):
            # ---- resident SBUF slabs ----
            tok = big.tile([128, NKC * S], BF16, tag="tok")          # 32KB/p
            wsl = big.tile([128, NKC * WSLW], BF16, tag="wsl")       # 20KB/p
            wo_sb = big.tile([128, 2 * DIM], BF16, tag="wo")         # 4KB/p
            cosm = big.tile([128, S], BF16, tag="cos")
            sinm = big.tile([128, S], BF16, tag="sin")
            tokq8 = big.tile([128, NKC * S], I8, tag="tokq8")        # 16KB/p
            wslq8 = big.tile([128, NKC * WUP], I8, tag="wslq8")      # 6KB/p
            winsb = big.tile([128, STRIPW], F32, tag="winsb")
            iotasb = big.tile([128, STRIPW], F32, tag="iotasb")
            ubq_t = big.tile([128, NSB], BF16, tag="ubq")
            ubr_t = big.tile([128, NSB], BF16, tag="ubr")
            ubl = big.tile([128, NSB], F32, tag="ubl")
            vrq8 = [big.tile([128, NSB * D], I8, tag=f"vq{h}", name=f"vq{h}") for h in range(HPC)]
            vrs = [big.tile([128, NSB * D], BF16, tag=f"vr{h}", name=f"vr{h}") for h in range(HPC)]
            vaug = [big.tile([128, NSB * 65], BF16, tag=f"va{h}", name=f"va{h}") for h in range(HPC)]
            mixs = big.tile([128, 64], F32, tag="mix")               # sigmoid(mix)
            mixr = big.tile([128, 64], BF16, tag="mixr")             # uploaded raw mix
            gate1 = big.tile([1, HPC * S], BF16, tag="gate1")        # uploaded sigmoid(gate)
            cvec = big.tile([128, 12], BF16, tag="cvec")             # scale consts hi/lo
            cexp = big.tile([128, 1], F32, tag="cexp")               # exp logit scale
            csv = big.tile([128, 1], F32, tag="csv")                 # v dequant scale
            vsc = big.tile([128, 4], F32, tag="vsc")                 # vr dequant scales
            ones1 = big.tile([1, 64], BF16, tag="ones1")
            qkslab = big.tile([128, 8 * S], BF16, tag="qkslab")
            qraw = [qkslab[:, 0 * S : 1 * S], qkslab[:, 1 * S : 2 * S]]
            kraw = [qkslab[:, 2 * S : 3 * S], qkslab[:, 3 * S : 4 * S]]
            qrot = [qkslab[:, 4 * S : 5 * S], qkslab[:, 5 * S : 6 * S]]
            krot = [qkslab[:, 6 * S : 7 * S], qkslab[:, 7 * S : 8 * S]]
            qro, kro = qraw, kraw  # roped in place
            # PT ring: 5 live strips per head
            pts = [big.tile([128, 5 * STRIPW], BF16, tag=f"pt{h}", name=f"pt{h}") for h in range(HPC)]
            outg = [big.tile([128, S], BF16, tag=f"og{p}", name=f"og{p}") for p in range(2)]
            vtmp = big.tile([128, D], F32, tag="vtmp")
            dmy = big.tile([1, 128], F32, tag="dmy")
            dmyc = [0]

            pend = []

            def guard(inst):
                if pend:
                    s = InstructionNameOrderedSet()
                    for n in pend:
                        s.add(n)
                    inst.ins.add_nosync_dependencies_from(s)
                    pend.clear()
                return inst

            def absorb(*aps):
                for ap in aps:
                    i = dmyc[0] % 128
                    dmyc[0] += 1
                    ii = nc.vector.tensor_copy(dmy[0:1, i : i + 1], ap[0:1, 0:1])
                    pend.append(ii.ins.name)

            dmyA = big.tile([1, 128], F32, tag="dmyA")
            dmyAc = [0]

            def absorb_act(ap):
                i = dmyAc[0] % 128
                dmyAc[0] += 1
                ii = nc.scalar.copy(dmyA[0:1, i : i + 1], ap[0:1, 0:1])
                pend.append(ii.ins.name)

            bcb = big.tile([32, 1024], BF16, tag="bcb")
            bcbc = [0]
            crumb_st = {"last": None}

            def crumb(src_ap):
                crumb_st["last"] = src_ap[0:1, 0:1]

            def pe_absorb(ap=None):
                ap = ap if ap is not None else crumb_st["last"]
                if ap is None:
                    return
                if ap.partition_size() >= 32 and ap.dtype == BF16:
                    ii = nc.tensor.ldweights(ap[0:32, 0:1])
                else:
                    i = bcbc[0] % 1024
                    bcbc[0] += 1
                    nc.vector.tensor_copy(bcb[0:1, i : i + 1], ap[0:1, 0:1])
                    ii = nc.tensor.ldweights(bcb[0:32, i : i + 1])
                pend.append(ii.ins.name)

            # ---- loads (from gathered internal DRAM + inline consts) ----
            # tokens/weights: int8 staged resident, converted to bf16 on DVE
            tokv = tok.rearrange("p (k qq s) -> p k qq s", k=NKC, qq=4, s=512)
            tkv8 = tokq8.rearrange("p (k qq s) -> p k qq s", k=NKC, qq=4, s=512)
            for q in range(4):
                nc.gpsimd.dma_start(
                    out=tkv8[:, :, q, :],
                    in_=tok_g[q * TOK_N : (q + 1) * TOK_N].rearrange(
                        "(k p s) -> p k s", k=NKC, p=128, s=512
                    ),
                )
            for q in range(4):
                absorb(tokq8[:, q * 512 : q * 512 + 1])
                guard(nc.vector.tensor_copy(tokv[:, :, q, :], tkv8[:, :, q, :]))
            for kc in range(NKC):
                nc.gpsimd.dma_start(
                    out=wslq8[:, kc * WUP : (kc + 1) * WUP],
                    in_=wall_g[kc * 128 * WUP : (kc + 1) * 128 * WUP].rearrange(
                        "(p c) -> p c", p=128, c=WUP
                    ),
                )
            for kc in range(NKC):
                absorb(wslq8[:, kc * WUP : kc * WUP + 1])
                guard(nc.vector.tensor_copy(
                    wsl[:, kc * WSLW : kc * WSLW + 512],
                    wslq8[:, kc * WUP : kc * WUP + 512],
                ))
                guard(nc.vector.tensor_copy(
                    wsl[:, kc * WSLW + 1024 : kc * WSLW + 1280],
                    wslq8[:, kc * WUP + 512 : kc * WUP + 768],
                ))
            for half in range(2):
                nc.gpsimd.dma_start(
                    out=wo_sb[:, half * DIM : half * DIM + DIM],
                    in_=wo_g[half * WO_N : (half + 1) * WO_N].rearrange(
                        "(p d) -> p d", p=128, d=DIM
                    ),
                )
            nc.gpsimd.dma_start(out=cosm[:], in_=cos_d[:])
            nc.gpsimd.dma_start(out=sinm[:], in_=sin_d[:])
            nc.gpsimd.dma_start(out=winsb[:], in_=win_d[:])
            nc.gpsimd.dma_start(out=iotasb[:], in_=iota_d[:])
            off = WO_N
            nc.gpsimd.dma_start(
                out=ubq_t[:],
                in_=blob16_d[off : off + UB_N].rearrange("(p n) -> p n", p=128, n=NSB),
            )
            off += UB_N
            nc.gpsimd.dma_start(
                out=ubr_t[:],
                in_=blob16_d[off : off + UB_N].rearrange("(p n) -> p n", p=128, n=NSB),
            )
            off += UB_N
            nc.gpsimd.dma_start(
                out=gate1[:],
                in_=blob16_d[off : off + GR_N].rearrange("(o x) -> o x", o=1, x=GR_N),
            )
            off += GR_N
            nc.gpsimd.dma_start(
                out=mixr[:],
                in_=blob16_d[off : off + MR_N].rearrange("(p n) -> p n", p=128, n=64),
            )
            # vr: one strided int8 DMA per head, dequant on DVE
            vr0 = TOK_N + WALL_N
            for h in range(HPC):
                nc.gpsimd.dma_start(
                    out=vrq8[h].rearrange("p (sb d) -> p sb d", sb=NSB, d=D),
                    in_=blob8_d[vr0 + h * NSB * 128 * D : vr0 + (h + 1) * NSB * 128 * D]
                    .rearrange("(sb p d) -> p sb d", sb=NSB, p=128, d=D),
                )

            off += MR_N
            nc.gpsimd.dma_start(
                out=cvec[:],
                in_=blob16_d[off : off + CV_N].rearrange("(p n) -> p n", p=128, n=12),
            )
            nc.vector.memset(ones1[:], 1.0)
            absorb(cosm, sinm, gate1, cvec)
            guard(nc.vector.tensor_add(cexp[:], cvec[:, 0:1], cvec[:, 1:2]))
            guard(nc.vector.tensor_add(csv[:], cvec[:, 2:3], cvec[:, 3:4]))
            for h in range(HPC):
                guard(nc.vector.tensor_add(
                    vsc[:, h : h + 1], cvec[:, 4 + 2 * h : 5 + 2 * h],
                    cvec[:, 5 + 2 * h : 6 + 2 * h],
                ))
            for h in range(HPC):
                absorb(vrq8[h][:, 0:1])
                guard(nc.vector.tensor_scalar(
                    vrs[h][:], vrq8[h][:], vsc[:, h : h + 1], None, mybir.AluOpType.mult
                ))
            absorb_act(mixr[0:1, 0:1])
            absorb_act(cexp[0:1, 0:1])
            absorb_act(csv[0:1, 0:1])

            # ---- ub reconstruct + masks on device: msl = win * (iota <= ub) ----
            absorb(ubq_t, ubr_t)
            guard(nc.vector.scalar_tensor_tensor(
                ubl[:], ubq_t[:], 256.0, ubr_t[:],
                mybir.AluOpType.mult, mybir.AluOpType.add,
            ))
            absorb(winsb, iotasb)

            # ---- rot weights on device: wqrot/wkrot = 32-col half-swap of wq/wk ----
            for kc in range(NKC):
                base = kc * WSLW
                for h in range(HPC):
                    for dst0, src0 in ((0, 32), (32, 0)):
                        nc.vector.tensor_copy(
                            wsl[:, base + 512 + h * 64 + dst0 : base + 512 + h * 64 + dst0 + 32],
                            wsl[:, base + h * 64 + src0 : base + h * 64 + src0 + 32],
                        )
                        nc.vector.tensor_copy(
                            wsl[:, base + 768 + h * 64 + dst0 : base + 768 + h * 64 + dst0 + 32],
                            wsl[:, base + 256 + h * 64 + src0 : base + 256 + h * 64 + src0 + 32],
                        )

            def wchunk(kc, c0, c1):
                return wsl[:, kc * WSLW + c0 : kc * WSLW + c1]

            def tchunk(kc, s0, s1):
                return tok[:, kc * S + s0 : kc * S + s1]

            # ---- phase 1: T-orient projections: q, k (dual use) ----
            NS = 4  # seq chunks of 512
            for dest, c0 in (
                (qraw[0], 0), (qraw[1], 128), (kraw[0], 256), (kraw[1], 384),
                (qrot[0], 512), (qrot[1], 640), (krot[0], 768), (krot[1], 896),
            ):
                for ns in range(NS):
                    ps = pp.tile([128, 512], F32, tag="ps1", name="psA")
                    pe_absorb()
                    for kc in range(NKC):
                        guard(nc.tensor.matmul(
                            ps[:],
                            wchunk(kc, c0, c0 + 128),
                            tchunk(kc, ns * 512, ns * 512 + 512),
                            start=(kc == 0),
                            stop=(kc == NKC - 1),
                        ))
                    nc.vector.tensor_copy(dest[:, ns * 512 : ns * 512 + 512], ps[:])
                    crumb(dest[:, ns * 512 : ns * 512 + 512])

            # ---- phase 2: v (natural orient) + lerp with value residual ----
            for sb in range(NSB):
                ps = pp.tile([128, 256], F32, tag="ps1", name="psV")
                pe_absorb()
                for kc in range(NKC):
                    guard(nc.tensor.matmul(
                        ps[:],
                        tchunk(kc, sb * 128, sb * 128 + 128),
                        wchunk(kc, 1024, 1280),
                        start=(kc == 0),
                        stop=(kc == NKC - 1),
                    ))
                absorb_act(mixr[0:1, sb * 4 : sb * 4 + 1])
                guard(nc.scalar.activation(
                    mixs[:, sb * 4 : sb * 4 + 4], mixr[:, sb * 4 : sb * 4 + 4], AF.Sigmoid
                ))
                v_t = stg.tile([128, 256], F32, tag="vt", name="vt", bufs=2)
                absorb(ps[0:1, 0:1])
                guard(nc.vector.tensor_scalar(
                    v_t[:], ps[:], csv[:, 0:1], None, mybir.AluOpType.mult
                ))
                for h in range(HPC):
                    absorb(vrs[h][:, sb * D : sb * D + D])
                    guard(nc.vector.tensor_sub(
                        vtmp[:],
                        vrs[h][:, sb * D : sb * D + D],
                        v_t[:, h * D : h * D + D],
                    ))
                    absorb(mixs[:, sb * 4 + h : sb * 4 + h + 1])
                    # v' = mix*(vr - v) + v
                    guard(nc.vector.scalar_tensor_tensor(
                        vaug[h][:, sb * 65 : sb * 65 + 64],
                        vtmp[:],
                        mixs[:, sb * 4 + h : sb * 4 + h + 1],
                        v_t[:, h * D : h * D + D],
                        mybir.AluOpType.mult,
                        mybir.AluOpType.add,
                    ))
                    nc.vector.memset(vaug[h][:, sb * 65 + 64 : sb * 65 + 65], 1.0)
                crumb(vaug[HPC - 1][:, sb * 65 : sb * 65 + 64])

            # ---- phase 1b: RoPE via half-swap + cos/sin maps ----
            for raw, rot in (
                (qraw[0], qrot[0]),
                (qraw[1], qrot[1]),
                (kraw[0], krot[0]),
                (kraw[1], krot[1]),
            ):
                nc.vector.tensor_mul(rot[:], rot[:], sinm[:])
                nc.vector.tensor_mul(raw[:], raw[:], cosm[:])
                nc.vector.tensor_add(raw[:], raw[:], rot[:])
                crumb(raw[:])

            # ---- phase 3: attention ----
            ptw_hist, ring_hist, fbs_hist, og_hist = [], [], [], []
            for kb in range(NSB):
                Wn = min(STRIPW, S - kb * 128)
                mtile = stg.tile([128, STRIPW], BF16, tag="mt", name="mt", bufs=2)
                guard(nc.vector.scalar_tensor_tensor(
                    mtile[:],
                    iotasb[:],
                    ubl[:, kb : kb + 1],
                    winsb[:],
                    mybir.AluOpType.is_le,
                    mybir.AluOpType.mult,
                ))
                for h in range(HPC):
                    p, hh = divmod(h, 2)
                    b0 = hh * 64
                    ptv = pts[h][:, (kb % 5) * STRIPW : (kb % 5) * STRIPW + STRIPW]
                    sim = pp.tile([128, STRIPW], F32, tag="psS", name="psS")
                    pe_absorb(kro[p])
                    pe_absorb(qro[p])
                    if len(ptw_hist) >= 1:
                        pe_absorb(ptw_hist[-1])
                    for c0 in range(0, Wn, 512):
                        c1 = min(c0 + 512, Wn)
                        guard(nc.tensor.matmul(
                            sim[:, c0:c1],
                            kro[p][b0 : b0 + 64, kb * 128 : kb * 128 + 128],
                            qro[p][b0 : b0 + 64, kb * 128 + c0 : kb * 128 + c1],
                            start=True,
                            stop=True,
                        ))
                    ptw = stg.tile([128, STRIPW], BF16, tag="ptw", name="ptw", bufs=2)
                    if ring_hist:
                        absorb_act(ring_hist[-1][0:1, 0:1])
                    absorb_act(sim[0:1, 0:1])
                    guard(nc.scalar.activation(
                        ptw[:, 0:Wn], sim[:, 0:Wn], AF.Exp, scale=cexp[:, 0:1]
                    ))
                    ptw_hist.append(ptw)
                    absorb(ptw[0:1, 0:1], ptv[0:1, 0:1])
                    guard(nc.vector.tensor_mul(
                        ptv[:, 0:Wn],
                        ptw[:, 0:Wn],
                        mtile[:, 0:Wn],
                    ))
                    ring_hist.append(ptv)
                    # AV for q-block qb = kb
                    av = pp.tile([65, 128], F32, tag="psAV", name="psAV", bufs=1)
                    pe_absorb(ptv)
                    if og_hist:
                        pe_absorb(og_hist[-1])
                    if fbs_hist:
                        pe_absorb(fbs_hist[-1][0:1, 0:1])
                    srcs = list(range(max(0, kb - 4), kb + 1))
                    for j, sc in enumerate(srcs):
                        off2 = (kb - sc) * 128
                        psrc = pts[h][:, (sc % 5) * STRIPW + off2 : (sc % 5) * STRIPW + off2 + 128]
                        guard(nc.tensor.matmul(
                            av[:],
                            vaug[h][:, sc * 65 : sc * 65 + 65],
                            psrc,
                            start=(j == 0),
                            stop=(j == len(srcs) - 1),
                        ))
                    # normalize + gate
                    rec_sb = big.tile([1, 128], F32, tag="recsb", name="recsb")
                    f_row = big.tile([1, 128], BF16, tag="frow", name="frow")
                    gsl = gate1[0:1, h * S + kb * 128 : h * S + kb * 128 + 128]
                    nc.vector.reciprocal(rec_sb[:], av[64:65, :])
                    absorb(gsl)
                    guard(nc.vector.tensor_mul(f_row[:], rec_sb[:], gsl))
                    pe_absorb(f_row[0:1, 0:1])
                    if fbs_hist:
                        pe_absorb(fbs_hist[-1][0:1, 0:1])
                    fps = pp.tile([64, 128], F32, tag="fps", name="fps", bufs=1)
                    guard(nc.tensor.matmul(fps[:], ones1[:], f_row[:], start=True, stop=True))
                    fbs = stg.tile([64, 128], F32, tag="fbs", name="fbs", bufs=1)
                    nc.vector.tensor_copy(fbs[:], fps[:])
                    fbs_hist.append(fbs)
                    guard(nc.vector.tensor_mul(
                        outg[p][b0 : b0 + 64, kb * 128 : kb * 128 + 128],
                        av[0:64, :],
                        fbs[:],
                    ))
                    og_hist.append(outg[p][b0 : b0 + 1, kb * 128 : kb * 128 + 1])

            # ---- phase 4: Wo -> partial out (internal DRAM) ----
            ost_hist = []
            crumb(outg[0][:, S - 128 : S])
            crumb(outg[1][:, S - 128 : S])
            for g8 in range(4):
                slab = qkslab[:, g8 * 4 * DIM : (g8 + 1) * 4 * DIM]
                for j in range(4):
                    sb = g8 * 4 + j
                    for half in range(2):
                        ps = pp.tile([128, 512], F32, tag="ps1", name="psO")
                        pe_absorb()
                        if ost_hist:
                            pe_absorb(ost_hist[-1])
                        for kc in range(2):
                            guard(nc.tensor.matmul(
                                ps[:],
                                outg[kc][:, sb * 128 : sb * 128 + 128],
                                wo_sb[:, kc * DIM + half * 512 : kc * DIM + half * 512 + 512],
                                start=(kc == 0),
                                stop=(kc == 1),
                            ))
                        dst = slab[:, j * DIM + half * 512 : j * DIM + half * 512 + 512]
                        absorb(ps[0:1, 0:1])
                        guard(nc.vector.tensor_copy(dst, ps[:]))
                        ost_hist.append(dst)
                nc.sync.dma_start(
                    out=pout_d[g8 * 512 : g8 * 512 + 512, :].rearrange(
                        "(sb p) d -> p sb d", p=128
                    ),
                    in_=slab.rearrange("p (sb d) -> p sb d", d=DIM),
                )

    # ---- post-tile (drain guarantees all DMAs done): reduce partials ----
    nc.gpsimd.collective_compute(
        "ReduceScatter", mybir.AluOpType.add, replica_groups=G4,
        ins=[pout_d[:].opt()], outs=[rs_d[:].opt()],
    ).then_inc(s_rs, 1)
    nc.gpsimd.wait_ge(s_rs, 1)
    with tile.TileContext(nc) as tc2:
        with tc2.tile_pool(name="qz", bufs=1) as qz:
            rsb = qz.tile([128, 4 * DIM], BF16, tag="rsb")
            amx = qz.tile([128, 1], F32, tag="amx")
            rcpq = qz.tile([128, 1], F32, tag="rcpq")
            oi8 = qz.tile([128, 4 * DIM], I8, tag="oi8")
            nc.gpsimd.dma_start(
                out=rsb.rearrange("p (sb d) -> p sb d", d=DIM),
                in_=rs_d[:].rearrange("(sb p) d -> p sb d", p=128),
            )
            nc.vector.tensor_reduce(
                amx[:], rsb[:], mybir.AxisListType.XYZW, mybir.AluOpType.max,
                apply_absolute_value=True,
            )
            nc.vector.reciprocal(rcpq[:], amx[:])
            nc.vector.tensor_scalar(
                oi8[:], rsb[:], rcpq[:, 0:1], 127.0,
                mybir.AluOpType.mult, mybir.AluOpType.mult,
            )
            nc.sync.dma_start(
                out=out_d[:].rearrange("(sb p) d -> p sb d", p=128),
                in_=oi8.rearrange("p (sb d) -> p sb d", d=DIM),
            )
            nc.sync.dma_start(out=outsc_d[:], in_=amx[:])

    _nc_cache["nc"] = nc
    return nc


def _q8(x):
    s = float(np.abs(x).max()) / 127.0
    return np.clip(np.round(x / s), -127, 127).astype(np.int8), s


def _hilo(x):
    bf = ml_dtypes.bfloat16
    hi = np.float32(bf(x))
    lo = np.float32(bf(np.float32(x) - hi))
    return bf(hi), bf(lo)


def _prep_all(tokens, value_residual, episode_ids, Wq, Wkv, Wo, Wg, Wmix):
    bf = ml_dtypes.bfloat16
    perm = np.concatenate([np.arange(0, D, 2), np.arange(1, D, 2)])

    tokens = np.asarray(tokens, dtype=np.float32)
    st = float(np.abs(tokens).max()) / 127.0
    tokq = [
        np.ascontiguousarray(
            np.clip(np.round(tokens[b].T / st), -127, 127).astype(np.int8)
        )
        for b in range(B)
    ]                                                                         # [DIM,S] i8

    Wqp = np.asarray(Wq).reshape(DIM, H, D)[:, :, perm]
    Wkp = np.asarray(Wkv)[:, : H * D].reshape(DIM, H, D)[:, :, perm]
    Wvp = np.asarray(Wkv)[:, H * D :].reshape(DIM, H, D)
    Wop = np.asarray(Wo).reshape(H, D, DIM)
    packs = []
    for g in range(4):
        hs = slice(4 * g, 4 * g + 4)
        wq_q, swq = _q8(Wqp[:, hs].reshape(DIM, 256))
        wk_q, swk = _q8(Wkp[:, hs].reshape(DIM, 256))
        wv_q, swv = _q8(Wvp[:, hs].reshape(DIM, 256))
        wall = np.concatenate([wq_q, wk_q, wv_q], axis=1)                     # [DIM,768] i8
        wall = np.ascontiguousarray(wall.reshape(NKC, 128, WUP))
        wo = np.ascontiguousarray(
            Wop[hs].reshape(256, DIM).astype(bf).reshape(2, 128, DIM)
        )
        cexp = st * st * swq * swk * (D ** -0.5)
        csv = st * swv
        packs.append((wall, wo, cexp, csv))

    # gate/mix projections on host (tiny, keeps sigmoid paths off fp8)
    graw_all = np.einsum("bnd,dh->bhn", tokens, np.asarray(Wg, dtype=np.float32))   # [B,H,S]
    gate1_all = 1.0 / (1.0 + np.exp(-graw_all))
    mraw_all = np.einsum("bnd,dh->bnh", tokens, np.asarray(Wmix, dtype=np.float32)) # [B,S,H]

    ubqs, ubrs = [], []
    kk = np.arange(S)
    for b in range(B):
        ids = np.asarray(episode_ids[b])
        ee = np.searchsorted(ids, ids, side="right") - 1                      # [S]
        ubl = ee - (kk // 128) * 128                                          # int
        q = ubl // 256
        r = ubl - 256 * q
        ubqs.append(np.ascontiguousarray(q.reshape(NSB, 128).T.astype(bf)))   # [128,NSB]
        ubrs.append(np.ascontiguousarray(r.reshape(NSB, 128).T.astype(bf)))

    vr_f = np.asarray(value_residual, dtype=np.float32)
    svr = np.abs(vr_f).max(axis=(2, 3)) / 127.0                               # [B,H]
    vrq = np.clip(np.round(vr_f / svr[:, :, None, None]), -127, 127).astype(np.int8)

    in_maps = []
    for c in range(NCORES):
        b, g = divmod(c, 4)
        wall, wo, cexp, csv = packs[g]
        hs = slice(4 * g, 4 * g + 4)
        cv = np.empty((128, 12), dtype=bf)
        eh, el = _hilo(cexp)
        vh, vl = _hilo(csv)
        cv[:, 0], cv[:, 1], cv[:, 2], cv[:, 3] = eh, el, vh, vl
        for h in range(HPC):
            sh_, sl_ = _hilo(float(svr[b, 4 * g + h]))
            cv[:, 4 + 2 * h], cv[:, 5 + 2 * h] = sh_, sl_

        g1 = gate1_all[b, hs].astype(bf)                                      # [4,S]
        mraw = np.ascontiguousarray(
            mraw_all[b, :, hs].reshape(NSB, 128, 4).transpose(1, 0, 2).reshape(128, 64)
        ).astype(bf)                                                          # [128,64]
        blob8 = np.concatenate([
            tokq[b][:, g * 512 : (g + 1) * 512].ravel(),
            wall[4 * b : 4 * b + 4].ravel(),
            vrq[b, hs].ravel(),
        ])
        blob16 = np.concatenate([
            wo[b].ravel(),
            ubqs[b].ravel(), ubrs[b].ravel(),
            g1.ravel(), mraw.ravel(), cv.ravel(),
        ])
        in_maps.append({"blob8": blob8, "blob16": blob16})
    return in_maps


# ---- cached PJRT dispatch: jit built once, zero output-buffers device-resident ----
_runner_cache = {}


def _get_runner(nc):
    if "fn" in _runner_cache:
        return _runner_cache
    from jax.experimental.shard_map import shard_map

    bass2jax.install_neuronx_cc_hook()
    partition_name = nc.partition_id_tensor.name if nc.partition_id_tensor else None
    in_names, out_names, out_avals = [], [], []
    for alloc in nc.m.functions[0].allocations:
        if not isinstance(alloc, mybir.MemoryLocationSet):
            continue
        name = alloc.memorylocations[0].name
        if alloc.kind == "ExternalInput":
            if name != partition_name:
                in_names.append(name)
        elif alloc.kind == "ExternalOutput":
            out_avals.append(
                jax.core.ShapedArray(tuple(alloc.tensor_shape), mybir.dt.np(alloc.dtype))
            )
            out_names.append(name)
    in_names_all = list(in_names) + list(out_names)
    if partition_name is not None:
        in_names_all.append(partition_name)

    def _body(*args):
        operands = list(args)
        if partition_name is not None:
            operands.append(bass2jax.partition_id_tensor())
        outs = bass2jax._bass_exec_p.bind(
            *operands,
            out_avals=tuple(out_avals),
            in_names=tuple(in_names_all),
            out_names=tuple(out_names),
            lowering_input_output_aliases=(),
            sim_require_finite=True,
            sim_require_nnan=True,
            nc=nc,
        )
        return tuple(outs)

    devices = jax.devices()[:NCORES]
    mesh = jax.sharding.Mesh(np.asarray(devices), ("core",))
    P = jax.sharding.PartitionSpec
    n_in = len(in_names) + len(out_names)
    fn = jax.jit(
        shard_map(
            _body, mesh=mesh, in_specs=(P("core"),) * n_in,
            out_specs=(P("core"),) * len(out_names), check_rep=False,
        ),
        keep_unused=True,
    )
    sh = jax.sharding.NamedSharding(mesh, P("core"))
    zeros_dev = [
        jax.device_put(np.zeros((NCORES * a.shape[0], *a.shape[1:]), a.dtype), sh)
        for a in out_avals
    ]
    jax.block_until_ready(zeros_dev)
    _runner_cache.update(fn=fn, in_names=in_names, zeros_dev=zeros_dev)
    return _runner_cache


def _execute(nc, in_maps):
    r = _get_runner(nc)
    bufs = r.setdefault("concat_bufs", {})
    concat_in = []
    for name in r["in_names"]:
        parts = [np.asarray(m[name]) for m in in_maps]
        n = parts[0].shape[0]
        buf = bufs.get(name)
        if buf is None or buf.shape[0] != NCORES * n:
            buf = np.empty((NCORES * n, *parts[0].shape[1:]), parts[0].dtype)
            bufs[name] = buf
        for c, p in enumerate(parts):
            buf[c * n : (c + 1) * n] = p
        concat_in.append(buf)
    outs = r["fn"](*concat_in, *r["zeros_dev"])
    outs[0].copy_to_host_async()
    outs[1].copy_to_host_async()
    oi = np.asarray(outs[0]).reshape(NCORES, 512, DIM)
    sc = np.asarray(outs[1]).reshape(NCORES, 1, 128, 1) / 127.0
    # core c = (b, rk) row-major == row-major [2,4] blocks of res
    res = np.empty((B, S, DIM), dtype=np.float32)
    scl = np.broadcast_to(sc, (NCORES, 4, 128, 1)).reshape(NCORES, 512, 1)
    np.multiply(oi, scl, out=res.reshape(NCORES, 512, DIM))
    return res


def kernel(tokens, value_residual, episode_ids, Wq, Wkv, Wo, Wg, Wmix):
    nc = build_nc()
    in_maps = _prep_all(tokens, value_residual, episode_ids, Wq, Wkv, Wo, Wg, Wmix)
    return _execute(nc, in_maps)
